# revision 17
# baseline (speedup 1.0000x reference)
"""MBart MoE decoder layer on 8 trn2 NeuronCores.

Sharding: 8 cores = 8 (sequence, expert-slot) pairs. Core c handles
sequence b=c//2, expert slot j=c%2 (each sequence is lang-routed to at
most 2 distinct experts; routing is computed on the host from `langs`).
Each core computes the full attention path for its sequence (replicated
across the pair) and one expert FFN over all 256 tokens; the host sums
the pair's partial outputs (expert-sharded combine) and transposes back
to token-major. Expert weights are gathered per-core on the host, so a
core only receives the one expert it needs.

On-device layout is feature-major [D, tokens]: projections take weights
as lhsT (feature-major out) or activations as lhsT (token-major out), so
no activation transposes are needed anywhere. LN gains/biases are folded
into the downstream weights on the host; softmax uses transposed scores
[keys, queries] with the attention mask added via an identity-matmul
into PSUM (host classifies each 128x128 mask block as zero / add / skip,
so causal dead blocks are never computed) and denominators accumulated
via a ones-matmul, then broadcast over partitions with a rank-1 matmul
for one full-lane reciprocal per head pair.
"""

import os
import sys
from contextlib import ExitStack

for _p in ("/opt/trn_rl_repo",):
    if _p not in sys.path:
        sys.path.append(_p)

import numpy as np
import ml_dtypes

import concourse.bass as bass
import concourse.tile as tile
import concourse.mybir as mybir
from concourse import bacc, bass_utils

B, S, SK = 4, 256, 512
D, NH, NKV, HD = 1024, 16, 4, 64
DE, NE = 4096, 8
LN_EPS = 1e-5
REP = NH // NKV
DC = D // 128    # 8 feature chunks
FC = DE // 128   # 32 ffn chunks
SC = S // 128    # 2 self-attn key chunks
KC = SK // 128   # 4 cross-attn key chunks
QC = S // 128    # 2 query halves
KVW = NKV * HD   # 256
GRP = 4          # ffn chunks per MoE weight group
NGRP = FC // GRP

MODE = os.environ.get("KERNEL_MM_DTYPE", "bf16")  # "bf16" | "f32r" | "f32"

_CACHE: dict = {}
_TRACE_DIR = None   # set by test harness for profiling runs
_LAST_EXEC_NS = None

# packed attention-weight column layout: qw | kw(dup) | vw
W_Q, W_K, W_V = 0, D, D + 2 * KVW
WPACK = D + 2 * KVW + KVW  # 1792

# packed per-partition bias column layout
_BIAS_COLS = {}
_off = 0
for _n, _w in [("qb", DC), ("kb", 4), ("vb", KVW), ("ob", DC),
               ("q2b", DC), ("k2b", 4), ("v2b", KVW), ("o2b", DC),
               ("b1", FC), ("b3", FC), ("c", 1)]:
    _BIAS_COLS[_n] = (_off, _w)
    _off += _w
BIAS_W = _off


def _build(mode, sa_cls, ca_cls):
    """sa_cls/ca_cls: block classes per (kc, qhalf): 0=no-mask, 1=mask-add,
    2=fully-masked(skip)."""
    st = {"bf16": mybir.dt.bfloat16, "f32r": mybir.dt.float32r,
          "f32": mybir.dt.float32}[mode]
    f32 = mybir.dt.float32
    same_st = mode == "f32"
    A = mybir.ActivationFunctionType
    OP = mybir.AluOpType

    nc = bacc.Bacc("TRN2", target_bir_lowering=False, debug=False, num_devices=8)
    import os as _os
    _SPLIT = _os.environ.get("KERNEL_DMA_SPLIT", "1") == "1"
    eng_b = nc.scalar if _SPLIT else nc.sync
    eng_s = nc.gpsimd if _SPLIT else nc.sync

    def mm(psum, lhsT, rhs, start, stop):
        nc.tensor.matmul(psum, lhsT, rhs, start=start, stop=stop)

    di = {}

    def din(name, shape, dtype=None):
        di[name] = nc.dram_tensor(name, list(shape), dtype or st, kind="ExternalInput")
        return di[name]

    din("xT", (128, DC * S), f32)
    if not same_st:
        din("xT_st", (128, DC * S))
    din("encT", (128, DC * SK))
    need_samask = any(c == 1 for c in sa_cls)
    need_camask = any(c == 1 for c in ca_cls)
    if need_samask:
        din("maskT", (S, S))
    if need_camask:
        din("encmaskT", (SK, S))
    din("id128", (128, 128))
    din("ones_col", (128, 1))
    din("ones_row", (1, 128))
    din("wqkv", (128, DC * WPACK))
    din("wca", (128, DC * WPACK))
    din("ow", (128, DC * D))
    din("o2w", (128, DC * D))
    din("biases", (128, BIAS_W), f32)
    din("w13", (128, NGRP * DC * 1024))  # per grp, per k: [w1 512 | w3 512]
    din("w2", (128, FC * D))
    out_res = nc.dram_tensor("out_res", [D, S], f32, kind="ExternalOutput")
    out_ffn = nc.dram_tensor("out_ffn", [S, D], f32, kind="ExternalOutput")

    with tile.TileContext(nc) as tc, ExitStack() as ctx:
        cp = ctx.enter_context(tc.tile_pool(name="consts", bufs=1))
        pers = ctx.enter_context(tc.tile_pool(name="pers", bufs=1))

        ones128 = cp.tile([128, 1], st, tag="ones128", name="ones128")
        eng_b.dma_start(ones128[:], di["ones_col"].ap())
        ones1r = cp.tile([1, 128], st, tag="ones1r", name="ones1r")
        eng_b.dma_start(ones1r[:], di["ones_row"].ap())
        eps_t = cp.tile([128, 1], f32, tag="eps_t", name="eps_t")
        nc.vector.memset(eps_t, LN_EPS)
        id128 = cp.tile([128, 128], st, tag="id128", name="id128")
        eng_b.dma_start(id128[:], di["id128"].ap())
        maskT = encmaskT = None
        if need_samask:
            maskT = cp.tile([128, SC, S], st, tag="maskT", name="maskT")
            for kc in range(SC):
                eng_b.dma_start(maskT[:, kc, :],
                                  di["maskT"].ap()[kc * 128:(kc + 1) * 128, :])
        if need_camask:
            encmaskT = cp.tile([128, KC, S], st, tag="encmaskT", name="encmaskT")
            for kc in range(KC):
                eng_b.dma_start(encmaskT[:, kc, :],
                                  di["encmaskT"].ap()[kc * 128:(kc + 1) * 128, :])

        bias_t = cp.tile([128, BIAS_W], f32, tag="bias_t", name="bias_t")
        eng_b.dma_start(bias_t[:], di["biases"].ap())

        def bias(nm):
            off, w = _BIAS_COLS[nm]
            return bias_t[:, off:off + w]

        def load_chunks(dram, nchunk, width, tag, pool, dtype=st, engine=None):
            t = pool.tile([128, nchunk * width], dtype, tag=tag, name=tag)
            (engine or nc.sync).dma_start(t[:], dram.ap())
            return [t[:, k * width:(k + 1) * width] for k in range(nchunk)]

        def layernorm(src_f32, src_st, out_tag, pool):
            """src: DC chunks [128,S] f32 (+st copies). Returns DC normalized
            chunks [128,S] st (gain/bias folded downstream by host)."""
            with tc.tile_pool(name=f"{out_tag}_lt", bufs=2) as lp, \
                 tc.tile_pool(name=f"{out_tag}_lp", bufs=1, space="PSUM") as sp, \
                 tc.tile_pool(name=f"{out_tag}_lb", bufs=1, space="PSUM") as bp:
                sum_ps = sp.tile([1, S], f32, tag="lnsum", name="lnsum")
                sq_ps = sp.tile([1, S], f32, tag="lnsq", name="lnsq")
                for k in range(DC):
                    sq = lp.tile([128, S], st, tag="lnsqt", name="lnsqt")
                    nc.vector.tensor_tensor(sq[:], src_f32[k][:], src_f32[k][:],
                                            OP.mult)
                    mm(sum_ps[:], ones128[:], src_st[k][:], k == 0, k == DC - 1)
                    mm(sq_ps[:], ones128[:], sq[:], k == 0, k == DC - 1)
                s_sb = lp.tile([1, S], st, tag="ln_ssb", name="ln_ssb")
                nc.vector.tensor_single_scalar(s_sb[:], sum_ps[:], 1.0 / D, OP.mult)
                q_sb = lp.tile([1, S], st, tag="ln_qsb", name="ln_qsb")
                nc.vector.tensor_single_scalar(q_sb[:], sq_ps[:], 1.0 / D, OP.mult)
                s_bc = bp.tile([128, S], f32, tag="ln_sbc", name="ln_sbc")
                q_bc = bp.tile([128, S], f32, tag="ln_qbc", name="ln_qbc")
                mm(s_bc[:], ones1r[:], s_sb[:], True, True)   # mean, bcast
                mm(q_bc[:], ones1r[:], q_sb[:], True, True)   # E[x^2], bcast
                # full-lane stats math on [128,S]
                s_sbuf = lp.tile([128, S], f32, tag="ln_ssbuf", name="ln_ssbuf")
                nc.vector.tensor_copy(s_sbuf[:], s_bc[:])
                var = lp.tile([128, S], f32, tag="ln_var", name="ln_var")
                nc.vector.scalar_tensor_tensor(var[:], s_bc[:], 0.0, s_sbuf[:],
                                               OP.bypass, OP.mult)
                nc.vector.tensor_sub(var[:], q_bc[:], var[:])
                v_t = lp.tile([128, S], f32, tag="ln_vt", name="ln_vt")
                nc.scalar.activation(v_t[:], var[:], A.Abs_reciprocal_sqrt,
                                     bias=eps_t[:])
                u_t = lp.tile([128, S], f32, tag="ln_ut", name="ln_ut")
                nc.vector.tensor_tensor(u_t[:], s_sbuf[:], v_t[:], OP.mult)
                outs = []
                for k in range(DC):
                    o = pool.tile([128, S], st, tag=f"{out_tag}{k}",
                                  name=f"{out_tag}{k}")
                    nc.vector.tensor_tensor(o[:], src_f32[k][:], v_t[:], OP.mult)
                    nc.vector.tensor_sub(o[:], o[:], u_t[:])
                    outs.append(o)
                return outs

        def cast_st(src, tag, pool):
            if same_st:
                return src
            outs = []
            for k, t in enumerate(src):
                o = pool.tile([128, t.shape[-1]], st, tag=f"{tag}{k}",
                              name=f"{tag}{k}")
                nc.vector.tensor_copy(o[:], t[:])
                outs.append(o)
            return outs

        def project_fm(w_slices, rhs_chunks, nout, bias_ap, out_tag, pool,
                       extra=None, out_dt=None, width=None):
            """out^T[dout_chunk] = sum_k w_slices[k][:, m*128:...].T @ rhs[k]."""
            W = width or S
            outs = []
            with tc.tile_pool(name=f"{out_tag}_ps", bufs=3, space="PSUM") as pp:
                for mI in range(nout):
                    ps = pp.tile([128, W], f32, tag="proj", name="proj")
                    for k in range(DC):
                        mm(ps[:], w_slices[k][:, mI * 128:(mI + 1) * 128],
                           rhs_chunks[k][:], k == 0, k == DC - 1)
                    o = pool.tile([128, W], out_dt or st, tag=f"{out_tag}{mI}",
                                  name=f"{out_tag}{mI}")
                    if extra is not None:
                        extra(mI, ps, o)
                    elif bias_ap is not None:
                        nc.vector.tensor_scalar(o[:], ps[:],
                                                bias_ap[:, mI:mI + 1], None,
                                                OP.add)
                    else:
                        nc.vector.tensor_copy(o[:], ps[:])
                    outs.append(o)
            return outs

        def project_tm(act_chunks, w_slices, ntok, bias_bcast, out_tag, pool):
            """token-major V with a ones column appended per kv head:
            out[tok_chunk] = [V_kv | 1] blocks of 65 columns."""
            outs = []
            with tc.tile_pool(name=f"{out_tag}_ps", bufs=3, space="PSUM") as pp:
                for t in range(ntok):
                    ps = pp.tile([128, KVW], f32, tag="projtm", name="projtm")
                    for k in range(DC):
                        mm(ps[:], act_chunks[k][:, t * 128:(t + 1) * 128],
                           w_slices[k][:], k == 0, k == DC - 1)
                    o = pool.tile([128, NKV, HD + 1], st, tag=f"{out_tag}{t}",
                                  name=f"{out_tag}{t}")
                    nc.vector.tensor_add(
                        o[:, :, 0:HD],
                        ps[:].rearrange("p (kv d) -> p kv d", kv=NKV),
                        bias_bcast[:].rearrange("p (kv d) -> p kv d", kv=NKV))
                    for kv in range(NKV):
                        nc.vector.tensor_copy(o[:, kv, HD:HD + 1], ones128[:])
                    outs.append(o)
            return outs

        def attend(qT, kT, vtm, n_kc, mask_tile, cls, out_tag, pool):
            """Transposed-score attention. cls[kc*QC + qh] in {0,1,2}.
            vtm blocks are [V_kv | ones] of 65 cols, so each O' matmul also
            accumulates the softmax denominator into row 64."""
            outs = []
            # per kc: active query range (contiguous union of non-skip halves)
            qr = []
            for kc in range(n_kc):
                act = [qh for qh in range(QC) if cls[kc * QC + qh] != 2]
                assert act and act == list(range(act[0], act[-1] + 1))
                qr.append((act[0] * 128, (act[-1] + 1) * 128))
            with tc.tile_pool(name=f"{out_tag}_sp", bufs=3, space="PSUM") as stp, \
                 tc.tile_pool(name=f"{out_tag}_op", bufs=2, space="PSUM") as opp, \
                 tc.tile_pool(name=f"{out_tag}_bp", bufs=1, space="PSUM") as bpp, \
                 tc.tile_pool(name=f"{out_tag}_et", bufs=6) as epool, \
                 tc.tile_pool(name=f"{out_tag}_dt", bufs=3) as dpool:
                for c in range(DC):
                    o_ps_h = [opp.tile([65, S], f32, tag=f"oph{hh}",
                                       name=f"oph{hh}") for hh in range(2)]
                    kv = (2 * c) // REP      # same kv head for both of the pair
                    for kc in range(n_kc):
                        q0, q1 = qr[kc]
                        adds = [q for q in range(QC) if cls[kc * QC + q] == 1]
                        st_h = []
                        e_h = []
                        for hh in range(2):
                            qh_ap = qT[c][hh * 64:(hh + 1) * 64, :]
                            kh = kT[kv][hh * 64:(hh + 1) * 64, :]
                            st_ps = stp.tile([128, S], f32, tag="st",
                                             name="st")
                            mm(st_ps[:, q0:q1], kh[:, kc * 128:(kc + 1) * 128],
                               qh_ap[:, q0:q1], True, not adds)
                            st_h.append(st_ps)
                        for hh in range(2):
                            for i, q in enumerate(adds):
                                mm(st_h[hh][:, q * 128:(q + 1) * 128], id128[:],
                                   mask_tile[:, kc, q * 128:(q + 1) * 128],
                                   False, i == len(adds) - 1)
                        for hh in range(2):
                            e = epool.tile([128, S], st, tag="e", name="e")
                            nc.scalar.activation(e[:, q0:q1],
                                                 st_h[hh][:, q0:q1], A.Exp)
                            e_h.append(e)
                        for hh in range(2):
                            mm(o_ps_h[hh][:, q0:q1],
                               vtm[kc][:, kv, :], e_h[hh][:, q0:q1],
                               kc == 0, kc == n_kc - 1)
                    den_pair = dpool.tile([1, 2 * S], st, tag="den_pair",
                                          name="den_pair")
                    for hh in range(2):
                        nc.vector.tensor_copy(den_pair[:, hh * S:(hh + 1) * S],
                                              o_ps_h[hh][64:65, :])
                    r_ps = bpp.tile([128, 2 * S], f32, tag="rbc", name="rbc")
                    mm(r_ps[:], ones1r[:], den_pair[:], True, True)
                    # 1/x as (1/sqrt(x))^2 on ACT; avoids the slow DVE recip
                    rsq = dpool.tile([128, 2 * S], f32, tag="rsq", name="rsq")
                    nc.scalar.activation(rsq[:], r_ps[:], A.Abs_reciprocal_sqrt)
                    rbi = dpool.tile([128, 2 * S], f32, tag="rbi", name="rbi")
                    nc.vector.tensor_tensor(rbi[:], rsq[:], rsq[:], OP.mult)
                    o = pool.tile([128, S], st, tag=f"{out_tag}{c}",
                                  name=f"{out_tag}{c}")
                    for hh in range(2):
                        nc.vector.tensor_tensor(
                            o[hh * 64:(hh + 1) * 64, :], o_ps_h[hh][0:64, :],
                            rbi[hh * 64:(hh + 1) * 64, hh * S:(hh + 1) * S],
                            OP.mult)
                    outs.append(o)
            return outs

        h1 = [pers.tile([128, S], f32, tag=f"h1T{k}", name=f"h1T{k}")
              for k in range(DC)]
        h2 = [pers.tile([128, S], f32, tag=f"h2T{k}", name=f"h2T{k}")
              for k in range(DC)]

        # ---------------- self attention ----------------
        with tc.tile_pool(name="sa_acts", bufs=1) as sa:
            xT = load_chunks(di["xT"], DC, S, "xT", sa, f32)
            xT_st = xT if same_st else load_chunks(di["xT_st"], DC, S, "xTs", sa)
            n1 = layernorm(xT, xT_st, "n1T", sa)
            with tc.tile_pool(name="wqkvp", bufs=1) as wp:
                wt = load_chunks(di["wqkv"], DC, WPACK, "wqkv", wp)
                qT = project_fm([t[:, W_Q:W_Q + D] for t in wt], n1, DC,
                                bias("qb"), "qT", sa)
                kT = project_fm([t[:, W_K:W_K + 2 * KVW] for t in wt], n1, 4,
                                bias("kb"), "kT", sa)
                v_tm = project_tm(n1, [t[:, W_V:W_V + KVW] for t in wt], SC,
                                  bias("vb"), "v_tm", sa)
            with tc.tile_pool(name="wop", bufs=1) as wp:
                ow_t = load_chunks(di["ow"], DC, D, "ow", wp)
                sa_out = attend(qT, kT, v_tm, SC, maskT, sa_cls, "saT", sa)

                def o_epil(mI, ps, o):
                    nc.vector.scalar_tensor_tensor(o[:], ps[:],
                                                   bias("ob")[:, mI:mI + 1],
                                                   xT[mI][:], OP.add, OP.add)
                project_fm(ow_t, sa_out, DC, None, "h1w", _FixedPool(h1),
                           extra=o_epil, out_dt=f32)

        # ---------------- cross attention ----------------
        with tc.tile_pool(name="ca_acts", bufs=1) as ca:
            encT = load_chunks(di["encT"], DC, SK, "encT", ca, engine=eng_b)
            h1_st = cast_st(h1, "h1s", ca)
            with tc.tile_pool(name="wcap", bufs=1) as wp:
                wt = load_chunks(di["wca"], DC, WPACK, "wca", wp, engine=eng_b)
                k2T = project_fm([t[:, W_K:W_K + 2 * KVW] for t in wt], encT, 4,
                                 bias("k2b"), "k2T", ca, width=SK)
                v2_tm = project_tm(encT, [t[:, W_V:W_V + KVW] for t in wt], KC,
                                   bias("v2b"), "v2_tm", ca)
                n2 = layernorm(h1, h1_st, "n2T", ca)
                q2T = project_fm([t[:, W_Q:W_Q + D] for t in wt], n2, DC,
                                 bias("q2b"), "q2T", ca)
            with tc.tile_pool(name="wo2p", bufs=1) as wp:
                o2w_t = load_chunks(di["o2w"], DC, D, "o2w", wp, engine=eng_b)
                ca_out = attend(q2T, k2T, v2_tm, KC, encmaskT, ca_cls, "caT", ca)

                def o2_epil(mI, ps, o):
                    nc.vector.scalar_tensor_tensor(o[:], ps[:],
                                                   bias("o2b")[:, mI:mI + 1],
                                                   h1[mI][:], OP.add, OP.add)
                project_fm(o2w_t, ca_out, DC, None, "h2w", _FixedPool(h2),
                           extra=o2_epil, out_dt=f32)

        # residual output (host: out_b = res.T + ffn_j0 + ffn_j1)
        for k in range(DC):
            eng_s.dma_start(out_res.ap()[k * 128:(k + 1) * 128, :], h2[k][:])

        # ---------------- MoE expert ----------------
        with tc.tile_pool(name="moe_acts", bufs=1) as mo:
            h2_st = cast_st(h2, "h2s", mo)
            n3 = layernorm(h2, h2_st, "n3T", mo)

            mT = [mo.tile([128, S], st, tag=f"mT{m}", name=f"mT{m}")
                  for m in range(FC)]
            with tc.tile_pool(name="w13p", bufs=2) as wp, \
                 tc.tile_pool(name="gh_ps", bufs=3, space="PSUM") as gp, \
                 tc.tile_pool(name="gelu_t", bufs=3) as gt:
                gw = GRP * 128
                GW = DC * 1024
                for g in range(NGRP):
                    wgt = wp.tile([128, GW], st, tag="w13g", name="w13g")
                    nc.sync.dma_start(wgt[:], di["w13"].ap()[:, g * GW:(g + 1) * GW])
                    wg = [wgt[:, k * 1024:(k + 1) * 1024] for k in range(DC)]
                    for mi in range(GRP):
                        mI = g * GRP + mi
                        g_ps = gp.tile([128, S], f32, tag="g_ps", name="g_ps")
                        h_ps = gp.tile([128, S], f32, tag="h_ps", name="h_ps")
                        for k in range(DC):
                            mm(g_ps[:], wg[k][:, mi * 128:(mi + 1) * 128],
                               n3[k][:], k == 0, k == DC - 1)
                        for k in range(DC):
                            mm(h_ps[:], wg[k][:, gw + mi * 128:gw + (mi + 1) * 128],
                               n3[k][:], k == 0, k == DC - 1)
                        ge = gt.tile([128, S], f32, tag="ge", name="ge")
                        nc.scalar.activation(ge[:], g_ps[:], A.Gelu,
                                             bias=bias("b1")[:, mI:mI + 1])
                        nc.vector.scalar_tensor_tensor(mT[mI][:], h_ps[:],
                                                       bias("b3")[:, mI:mI + 1],
                                                       ge[:], OP.add, OP.mult)

            # down-proj, token-major out: y[t,n] = sum_f M^T[f,t].T @ w2[f,n]
            with tc.tile_pool(name="w2p", bufs=3) as wp, \
                 tc.tile_pool(name="y_ps", bufs=1, space="PSUM") as yp, \
                 tc.tile_pool(name="outp", bufs=3) as op_:
                y_ps = [[yp.tile([128, 512], f32, tag=f"y{t}{n}", name=f"y{t}{n}")
                         for n in range(2)] for t in range(QC)]
                for k2 in range(FC):
                    w2t = wp.tile([128, D], st, tag="w2t", name="w2t")
                    nc.sync.dma_start(w2t[:], di["w2"].ap()[:, k2 * D:(k2 + 1) * D])
                    for t in range(QC):
                        for n in range(2):
                            mm(y_ps[t][n][:], mT[k2][:, t * 128:(t + 1) * 128],
                               w2t[:, n * 512:(n + 1) * 512],
                               k2 == 0, k2 == FC - 1)
                for t in range(QC):
                    for n in range(2):
                        o = op_.tile([128, 512], f32, tag="o_out", name="o_out")
                        nc.vector.tensor_scalar_mul(o[:], y_ps[t][n][:],
                                                    bias("c")[:, 0:1])
                        eng_s.dma_start(
                            out_ffn.ap()[t * 128:(t + 1) * 128,
                                         n * 512:(n + 1) * 512], o[:])

    nc.compile()
    return nc


class _FixedPool:
    """Adapter letting project_fm write into pre-allocated tiles."""

    def __init__(self, tiles):
        self._tiles = list(tiles)
        self._i = 0

    def tile(self, shape, dtype, tag=None, name=None):
        t = self._tiles[self._i]
        self._i += 1
        return t


def _routing(langs):
    """Per-sequence expert slots [(expert_idx, coef) x2], matching the
    reference: coef[e,b] = any(langs[b]==4+e) * (1/count(langs[b]>3))."""
    langs = np.asarray(langs)
    slots = []
    for b in range(langs.shape[0]):
        row = [int(v) for v in langs[b]]
        cnt = sum(1 for v in row if v > 3)
        rw = 1.0 if cnt == 0 else 1.0 / cnt
        seen = []
        for v in row:
            if v > 3 and 0 <= v - 4 < NE and (v - 4) not in seen:
                seen.append(v - 4)
        sl = [(e, rw) for e in seen]
        while len(sl) < 2:
            sl.append((0, 0.0))
        slots.append(sl[:2])
    return slots


def _mask_classes(maskT, n_kc):
    """Classify each [128 keys x 128 queries] block of a transposed mask:
    0 all-zero (no add), 1 general (add), 2 fully masked (skip compute).
    Keeps at least one active key block per query and contiguous active
    ranges per key chunk."""
    cls = []
    for kc in range(n_kc):
        for qh in range(QC):
            blk = maskT[kc * 128:(kc + 1) * 128, qh * 128:(qh + 1) * 128]
            if np.all(blk == 0):
                cls.append(0)
            elif np.all(blk <= -1e8):
                cls.append(2)
            else:
                cls.append(1)
    for qh in range(QC):
        if all(cls[kc * QC + qh] == 2 for kc in range(n_kc)):
            for kc in range(n_kc):
                cls[kc * QC + qh] = 1
    for kc in range(n_kc):
        act = [q for q in range(QC) if cls[kc * QC + q] != 2]
        if not act or act != list(range(act[0], act[-1] + 1)):
            for q in range(QC):
                if cls[kc * QC + q] == 2:
                    cls[kc * QC + q] = 1
    return tuple(cls)


def kernel(**inputs):
    mode = MODE
    np_dt = ml_dtypes.bfloat16 if mode == "bf16" else np.float32
    f32 = np.float32

    inp = {k: np.asarray(v) for k, v in inputs.items()}
    x = inp["hidden_states"].astype(f32)
    enc = inp["encoder_hidden_states"].astype(f32)
    mask = inp["attention_mask"].astype(f32)
    encmask = inp["encoder_attention_mask"].astype(f32)
    g1, b1 = inp["ln1_g"].astype(f32), inp["ln1_b"].astype(f32)
    g2, b2 = inp["ln2_g"].astype(f32), inp["ln2_b"].astype(f32)
    g3, b3 = inp["ln3_g"].astype(f32), inp["ln3_b"].astype(f32)

    def dup_kv(w):
        return np.concatenate([np.tile(w[:, 64 * j:64 * (j + 1)], (1, 2))
                               for j in range(NKV)], axis=1)

    def dup_kv_b(v):
        return np.concatenate([np.tile(v[64 * j:64 * (j + 1)], 2)
                               for j in range(NKV)])

    sc = HD ** -0.5
    qw_f = g1[:, None] * inp["sa_q_w"] * sc
    qb_f = (b1 @ inp["sa_q_w"] + inp["sa_q_b"]) * sc
    kw_f = dup_kv(g1[:, None] * inp["sa_k_w"])
    kb_f = dup_kv_b(b1 @ inp["sa_k_w"] + inp["sa_k_b"])
    vw_f = g1[:, None] * inp["sa_v_w"]
    vb_f = b1 @ inp["sa_v_w"] + inp["sa_v_b"]
    q2w_f = g2[:, None] * inp["ca_q_w"] * sc
    q2b_f = (b2 @ inp["ca_q_w"] + inp["ca_q_b"]) * sc
    k2w_f = dup_kv(inp["ca_k_w"])
    k2b_f = dup_kv_b(inp["ca_k_b"])
    w1_f = inp["moe_w1"] * g3[None, :, None]
    b1_f = np.einsum("d,edf->ef", b3, inp["moe_w1"]).astype(f32)
    w3_f = inp["moe_w3"] * g3[None, :, None]
    b3_f = np.einsum("d,edf->ef", b3, inp["moe_w3"]).astype(f32)

    maskT0 = np.ascontiguousarray(mask[:, 0].transpose(0, 2, 1))     # [B,S,S]
    encmaskT0 = np.ascontiguousarray(encmask[:, 0].transpose(0, 2, 1))
    sa_cls = _mask_classes(maskT0[0], SC)
    ca_cls = _mask_classes(encmaskT0[0], KC)
    for b in range(1, B):
        if _mask_classes(maskT0[b], SC) != sa_cls or \
           _mask_classes(encmaskT0[b], KC) != ca_cls:
            sa_cls = tuple(1 for _ in range(SC * QC))
            ca_cls = tuple(1 for _ in range(KC * QC))
            break

    key = (mode, sa_cls, ca_cls)
    if key not in _CACHE:
        _CACHE[key] = _build(mode, sa_cls, ca_cls)
    nc = _CACHE[key]

    def col128(v):
        return np.asarray(v, f32).reshape(-1, 128).T

    def pack_k(w):
        w = np.asarray(w)
        return np.concatenate([w[k * 128:(k + 1) * 128, :]
                               for k in range(w.shape[0] // 128)], axis=1)

    slots = _routing(inp["langs"])
    wqkv = np.concatenate([qw_f, kw_f, vw_f], axis=1).astype(np_dt)
    wca = np.concatenate([q2w_f, k2w_f, inp["ca_v_w"]], axis=1).astype(np_dt)

    bias_common = np.zeros((128, BIAS_W), f32)
    for nm, v in [("qb", col128(qb_f)), ("kb", col128(kb_f)),
                  ("vb", np.broadcast_to(vb_f.astype(f32), (128, KVW))),
                  ("ob", col128(inp["sa_o_b"])),
                  ("q2b", col128(q2b_f)), ("k2b", col128(k2b_f)),
                  ("v2b", np.broadcast_to(inp["ca_v_b"].astype(f32), (128, KVW))),
                  ("o2b", col128(inp["ca_o_b"]))]:
        off, w = _BIAS_COLS[nm]
        bias_common[:, off:off + w] = v

    in_maps = []
    for c in range(8):
        b, j = c // 2, c % 2
        e, coef = slots[b][j]
        xT = pack_k(np.ascontiguousarray(x[b].T))
        # interleave w1/w3 by group: [w1 grp g | w3 grp g] blocks of 512 cols
        gw = GRP * 128
        w13 = np.empty((128, NGRP * DC * 1024), f32)
        for g in range(NGRP):
            for k in range(DC):
                c0 = (g * DC + k) * 1024
                w13[:, c0:c0 + 512] = w1_f[e][k * 128:(k + 1) * 128,
                                              g * 512:(g + 1) * 512]
                w13[:, c0 + 512:c0 + 1024] = w3_f[e][k * 128:(k + 1) * 128,
                                                     g * 512:(g + 1) * 512]
        bt = bias_common.copy()
        for nm, v in [("b1", col128(b1_f[e])), ("b3", col128(b3_f[e]))]:
            off, w = _BIAS_COLS[nm]
            bt[:, off:off + w] = v
        bt[:, _BIAS_COLS["c"][0]] = coef
        m = {
            "xT": xT,
            "encT": pack_k(np.ascontiguousarray(enc[b].T)).astype(np_dt),
            "id128": np.eye(128, dtype=f32).astype(np_dt),
            "ones_col": np.ones((128, 1), f32).astype(np_dt),
            "ones_row": np.ones((1, 128), f32).astype(np_dt),
            "wqkv": pack_k(wqkv), "wca": pack_k(wca),
            "ow": pack_k(inp["sa_o_w"].astype(np_dt)),
            "o2w": pack_k(inp["ca_o_w"].astype(np_dt)),
            "biases": bt,
            "w13": w13.astype(np_dt),
            "w2": pack_k(np.ascontiguousarray(inp["moe_w2"][e])).astype(np_dt),
        }
        if mode != "f32":
            m["xT_st"] = xT.astype(np_dt)
        if any(cc == 1 for cc in sa_cls):
            m["maskT"] = maskT0[b].astype(np_dt)
        if any(cc == 1 for cc in ca_cls):
            m["encmaskT"] = encmaskT0[b].astype(np_dt)
        in_maps.append(m)

    kw = {}
    if _TRACE_DIR:
        kw = dict(trace=True, tmpdir=_TRACE_DIR, trace_cores=[0])
    res = bass_utils.run_bass_kernel_spmd(nc, in_maps, core_ids=list(range(8)), **kw)
    global _LAST_EXEC_NS
    _LAST_EXEC_NS = res.exec_time_ns
    return np.stack([
        res.results[2 * b]["out_res"].T
        + res.results[2 * b]["out_ffn"]
        + res.results[2 * b + 1]["out_ffn"]
        for b in range(B)
    ]).astype(f32)



# revision 18
# speedup vs baseline: 1.0329x; 1.0329x over previous
"""MBart MoE decoder layer on 8 trn2 NeuronCores.

Sharding: 8 cores = 8 (sequence, expert-slot) pairs. Core c handles
sequence b=c//2, expert slot j=c%2 (each sequence is lang-routed to at
most 2 distinct experts; routing is computed on the host from `langs`).
Each core computes the full attention path for its sequence (replicated
across the pair) and one expert FFN over all 256 tokens; the host sums
the pair's partial outputs (expert-sharded combine) and transposes back
to token-major. Expert weights are gathered per-core on the host, so a
core only receives the one expert it needs.

On-device layout is feature-major [D, tokens]: projections take weights
as lhsT (feature-major out) or activations as lhsT (token-major out), so
no activation transposes are needed anywhere. LN gains/biases are folded
into the downstream weights on the host; softmax uses transposed scores
[keys, queries] with the attention mask added via an identity-matmul
into PSUM (host classifies each 128x128 mask block as zero / add / skip,
so causal dead blocks are never computed) and denominators accumulated
via a ones-matmul, then broadcast over partitions with a rank-1 matmul
for one full-lane reciprocal per head pair.
"""

import os
import sys
from contextlib import ExitStack

for _p in ("/opt/trn_rl_repo",):
    if _p not in sys.path:
        sys.path.append(_p)

import numpy as np
import ml_dtypes

import concourse.bass as bass
import concourse.tile as tile
import concourse.mybir as mybir
from concourse import bacc, bass_utils

B, S, SK = 4, 256, 512
D, NH, NKV, HD = 1024, 16, 4, 64
DE, NE = 4096, 8
LN_EPS = 1e-5
REP = NH // NKV
DC = D // 128    # 8 feature chunks
FC = DE // 128   # 32 ffn chunks
SC = S // 128    # 2 self-attn key chunks
KC = SK // 128   # 4 cross-attn key chunks
QC = S // 128    # 2 query halves
KVW = NKV * HD   # 256
GRP = 4          # ffn chunks per MoE weight group
NGRP = FC // GRP

MODE = os.environ.get("KERNEL_MM_DTYPE", "bf16")  # "bf16" | "f32r" | "f32"

_CACHE: dict = {}
_TRACE_DIR = None   # set by test harness for profiling runs
_LAST_EXEC_NS = None

# packed attention-weight column layout: qw | kw(dup) | vw
W_Q, W_K, W_V = 0, D, D + 2 * KVW
WPACK = D + 2 * KVW + KVW  # 1792

# packed per-partition bias column layout
_BIAS_COLS = {}
_off = 0
for _n, _w in [("qb", DC), ("kb", 4), ("vb", KVW), ("ob", DC),
               ("q2b", DC), ("k2b", 4), ("v2b", KVW), ("o2b", DC),
               ("b1", FC), ("b3", FC), ("c", 1)]:
    _BIAS_COLS[_n] = (_off, _w)
    _off += _w
BIAS_W = _off


def _build(mode, sa_cls, ca_cls):
    """sa_cls/ca_cls: block classes per (kc, qhalf): 0=no-mask, 1=mask-add,
    2=fully-masked(skip)."""
    st = {"bf16": mybir.dt.bfloat16, "f32r": mybir.dt.float32r,
          "f32": mybir.dt.float32}[mode]
    f32 = mybir.dt.float32
    same_st = mode == "f32"
    A = mybir.ActivationFunctionType
    OP = mybir.AluOpType

    nc = bacc.Bacc("TRN2", target_bir_lowering=False, debug=False, num_devices=8)
    import os as _os
    _SPLIT = _os.environ.get("KERNEL_DMA_SPLIT", "1") == "1"
    eng_b = nc.scalar if _SPLIT else nc.sync
    eng_s = nc.gpsimd if _SPLIT else nc.sync

    def mm(psum, lhsT, rhs, start, stop):
        nc.tensor.matmul(psum, lhsT, rhs, start=start, stop=stop)

    di = {}

    def din(name, shape, dtype=None):
        di[name] = nc.dram_tensor(name, list(shape), dtype or st, kind="ExternalInput")
        return di[name]

    din("xT", (128, DC * S), f32)
    if not same_st:
        din("xT_st", (128, DC * S))
    din("encT", (128, DC * SK))
    need_samask = any(c == 1 for c in sa_cls)
    need_camask = any(c == 1 for c in ca_cls)
    if need_samask:
        din("maskT", (S, S))
    if need_camask:
        din("encmaskT", (SK, S))
    din("id128", (128, 128))
    din("ones_col", (128, 1))
    din("ones_row", (1, 128))
    din("wqkv", (128, DC * WPACK))
    din("wca", (128, DC * WPACK))
    din("ow", (128, DC * D))
    din("o2w", (128, DC * D))
    din("biases", (128, BIAS_W), f32)
    din("w13", (128, NGRP * DC * 1024))  # per grp, per k: [w1 512 | w3 512]
    din("w2", (128, FC * D))
    out_res = nc.dram_tensor("out_res", [D, S], f32, kind="ExternalOutput")
    out_ffn = nc.dram_tensor("out_ffn", [S, D], f32, kind="ExternalOutput")

    with tile.TileContext(nc) as tc, ExitStack() as ctx:
        cp = ctx.enter_context(tc.tile_pool(name="consts", bufs=1))
        pers = ctx.enter_context(tc.tile_pool(name="pers", bufs=1))

        ones128 = cp.tile([128, 1], st, tag="ones128", name="ones128")
        eng_b.dma_start(ones128[:], di["ones_col"].ap())
        ones1r = cp.tile([1, 128], st, tag="ones1r", name="ones1r")
        eng_b.dma_start(ones1r[:], di["ones_row"].ap())
        eps_t = cp.tile([128, 1], f32, tag="eps_t", name="eps_t")
        nc.vector.memset(eps_t, LN_EPS)
        id128 = cp.tile([128, 128], st, tag="id128", name="id128")
        eng_b.dma_start(id128[:], di["id128"].ap())
        maskT = encmaskT = None
        if need_samask:
            maskT = cp.tile([128, SC, S], st, tag="maskT", name="maskT")
            for kc in range(SC):
                eng_b.dma_start(maskT[:, kc, :],
                                  di["maskT"].ap()[kc * 128:(kc + 1) * 128, :])
        if need_camask:
            encmaskT = cp.tile([128, KC, S], st, tag="encmaskT", name="encmaskT")
            for kc in range(KC):
                eng_b.dma_start(encmaskT[:, kc, :],
                                  di["encmaskT"].ap()[kc * 128:(kc + 1) * 128, :])

        bias_t = cp.tile([128, BIAS_W], f32, tag="bias_t", name="bias_t")
        eng_b.dma_start(bias_t[:], di["biases"].ap())

        def bias(nm):
            off, w = _BIAS_COLS[nm]
            return bias_t[:, off:off + w]

        def load_chunks(dram, nchunk, width, tag, pool, dtype=st, engine=None):
            t = pool.tile([128, nchunk * width], dtype, tag=tag, name=tag)
            (engine or nc.sync).dma_start(t[:], dram.ap())
            return [t[:, k * width:(k + 1) * width] for k in range(nchunk)]

        def layernorm(src_f32, src_st, out_tag, pool):
            """src: DC chunks [128,S] f32 (+st copies). Returns DC normalized
            chunks [128,S] st (gain/bias folded downstream by host)."""
            with tc.tile_pool(name=f"{out_tag}_lt", bufs=2) as lp, \
                 tc.tile_pool(name=f"{out_tag}_lp", bufs=1, space="PSUM") as sp, \
                 tc.tile_pool(name=f"{out_tag}_lb", bufs=1, space="PSUM") as bp:
                sum_ps = sp.tile([1, S], f32, tag="lnsum", name="lnsum")
                sq_ps = sp.tile([1, S], f32, tag="lnsq", name="lnsq")
                for k in range(DC):
                    sq = lp.tile([128, S], st, tag="lnsqt", name="lnsqt")
                    nc.vector.tensor_tensor(sq[:], src_f32[k][:], src_f32[k][:],
                                            OP.mult)
                    mm(sum_ps[:], ones128[:], src_st[k][:], k == 0, k == DC - 1)
                    mm(sq_ps[:], ones128[:], sq[:], k == 0, k == DC - 1)
                s_sb = lp.tile([1, S], st, tag="ln_ssb", name="ln_ssb")
                nc.vector.tensor_single_scalar(s_sb[:], sum_ps[:], 1.0 / D, OP.mult)
                q_sb = lp.tile([1, S], st, tag="ln_qsb", name="ln_qsb")
                nc.vector.tensor_single_scalar(q_sb[:], sq_ps[:], 1.0 / D, OP.mult)
                s_bc = bp.tile([128, S], f32, tag="ln_sbc", name="ln_sbc")
                q_bc = bp.tile([128, S], f32, tag="ln_qbc", name="ln_qbc")
                mm(s_bc[:], ones1r[:], s_sb[:], True, True)   # mean, bcast
                mm(q_bc[:], ones1r[:], q_sb[:], True, True)   # E[x^2], bcast
                # full-lane stats math on [128,S]
                s_sbuf = lp.tile([128, S], f32, tag="ln_ssbuf", name="ln_ssbuf")
                nc.vector.tensor_copy(s_sbuf[:], s_bc[:])
                var = lp.tile([128, S], f32, tag="ln_var", name="ln_var")
                nc.vector.scalar_tensor_tensor(var[:], s_bc[:], 0.0, s_sbuf[:],
                                               OP.bypass, OP.mult)
                nc.vector.tensor_sub(var[:], q_bc[:], var[:])
                v_t = lp.tile([128, S], f32, tag="ln_vt", name="ln_vt")
                nc.scalar.activation(v_t[:], var[:], A.Abs_reciprocal_sqrt,
                                     bias=eps_t[:])
                u_t = lp.tile([128, S], f32, tag="ln_ut", name="ln_ut")
                nc.vector.tensor_tensor(u_t[:], s_sbuf[:], v_t[:], OP.mult)
                outs = []
                for k in range(DC):
                    o = pool.tile([128, S], st, tag=f"{out_tag}{k}",
                                  name=f"{out_tag}{k}")
                    nc.vector.tensor_tensor(o[:], src_f32[k][:], v_t[:], OP.mult)
                    nc.vector.tensor_sub(o[:], o[:], u_t[:])
                    outs.append(o)
                return outs

        def cast_st(src, tag, pool):
            if same_st:
                return src
            outs = []
            for k, t in enumerate(src):
                o = pool.tile([128, t.shape[-1]], st, tag=f"{tag}{k}",
                              name=f"{tag}{k}")
                nc.vector.tensor_copy(o[:], t[:])
                outs.append(o)
            return outs

        def project_fm(w_slices, rhs_chunks, nout, bias_ap, out_tag, pool,
                       extra=None, out_dt=None, width=None):
            """out^T[dout_chunk] = sum_k w_slices[k][:, m*128:...].T @ rhs[k]."""
            W = width or S
            outs = []
            with tc.tile_pool(name=f"{out_tag}_ps", bufs=3, space="PSUM") as pp:
                for mI in range(nout):
                    ps = pp.tile([128, W], f32, tag="proj", name="proj")
                    for k in range(DC):
                        mm(ps[:], w_slices[k][:, mI * 128:(mI + 1) * 128],
                           rhs_chunks[k][:], k == 0, k == DC - 1)
                    o = pool.tile([128, W], out_dt or st, tag=f"{out_tag}{mI}",
                                  name=f"{out_tag}{mI}")
                    if extra is not None:
                        extra(mI, ps, o)
                    elif bias_ap is not None:
                        nc.vector.tensor_scalar(o[:], ps[:],
                                                bias_ap[:, mI:mI + 1], None,
                                                OP.add)
                    else:
                        nc.vector.tensor_copy(o[:], ps[:])
                    outs.append(o)
            return outs

        def project_tm(act_chunks, w_slices, ntok, bias_bcast, out_tag, pool):
            """token-major V with a ones column appended per kv head:
            out[tok_chunk] = [V_kv | 1] blocks of 65 columns."""
            outs = []
            with tc.tile_pool(name=f"{out_tag}_ps", bufs=3, space="PSUM") as pp:
                for t in range(ntok):
                    ps = pp.tile([128, KVW], f32, tag="projtm", name="projtm")
                    for k in range(DC):
                        mm(ps[:], act_chunks[k][:, t * 128:(t + 1) * 128],
                           w_slices[k][:], k == 0, k == DC - 1)
                    o = pool.tile([128, NKV, HD + 1], st, tag=f"{out_tag}{t}",
                                  name=f"{out_tag}{t}")
                    nc.vector.tensor_add(
                        o[:, :, 0:HD],
                        ps[:].rearrange("p (kv d) -> p kv d", kv=NKV),
                        bias_bcast[:].rearrange("p (kv d) -> p kv d", kv=NKV))
                    for kv in range(NKV):
                        nc.vector.tensor_copy(o[:, kv, HD:HD + 1], ones128[:])
                    outs.append(o)
            return outs

        def attend(qT, kT, vtm, n_kc, mask_tile, cls, out_tag, pool):
            """Transposed-score attention. cls[kc*QC + qh] in {0,1,2}.
            vtm blocks are [V_kv | ones] of 65 cols, so each O' matmul also
            accumulates the softmax denominator into row 64."""
            outs = []
            # per kc: active query range (contiguous union of non-skip halves)
            qr = []
            for kc in range(n_kc):
                act = [qh for qh in range(QC) if cls[kc * QC + qh] != 2]
                assert act and act == list(range(act[0], act[-1] + 1))
                qr.append((act[0] * 128, (act[-1] + 1) * 128))
            with tc.tile_pool(name=f"{out_tag}_sp", bufs=3, space="PSUM") as stp, \
                 tc.tile_pool(name=f"{out_tag}_op", bufs=2, space="PSUM") as opp, \
                 tc.tile_pool(name=f"{out_tag}_bp", bufs=1, space="PSUM") as bpp, \
                 tc.tile_pool(name=f"{out_tag}_et", bufs=6) as epool, \
                 tc.tile_pool(name=f"{out_tag}_dt", bufs=3) as dpool:
                for c in range(DC):
                    o_ps_h = [opp.tile([65, S], f32, tag=f"oph{hh}",
                                       name=f"oph{hh}") for hh in range(2)]
                    kv = (2 * c) // REP      # same kv head for both of the pair
                    for kc in range(n_kc):
                        q0, q1 = qr[kc]
                        adds = [q for q in range(QC) if cls[kc * QC + q] == 1]
                        st_h = []
                        e_h = []
                        for hh in range(2):
                            qh_ap = qT[c][hh * 64:(hh + 1) * 64, :]
                            kh = kT[kv][hh * 64:(hh + 1) * 64, :]
                            st_ps = stp.tile([128, S], f32, tag="st",
                                             name="st")
                            mm(st_ps[:, q0:q1], kh[:, kc * 128:(kc + 1) * 128],
                               qh_ap[:, q0:q1], True, not adds)
                            st_h.append(st_ps)
                        for hh in range(2):
                            for i, q in enumerate(adds):
                                mm(st_h[hh][:, q * 128:(q + 1) * 128], id128[:],
                                   mask_tile[:, kc, q * 128:(q + 1) * 128],
                                   False, i == len(adds) - 1)
                        for hh in range(2):
                            e = epool.tile([128, S], st, tag="e", name="e")
                            nc.scalar.activation(e[:, q0:q1],
                                                 st_h[hh][:, q0:q1], A.Exp)
                            e_h.append(e)
                        for hh in range(2):
                            mm(o_ps_h[hh][:, q0:q1],
                               vtm[kc][:, kv, :], e_h[hh][:, q0:q1],
                               kc == 0, kc == n_kc - 1)
                    den_pair = dpool.tile([1, 2 * S], st, tag="den_pair",
                                          name="den_pair")
                    for hh in range(2):
                        nc.vector.tensor_copy(den_pair[:, hh * S:(hh + 1) * S],
                                              o_ps_h[hh][64:65, :])
                    r_ps = bpp.tile([128, 2 * S], f32, tag="rbc", name="rbc")
                    mm(r_ps[:], ones1r[:], den_pair[:], True, True)
                    # 1/x as (1/sqrt(x))^2 on ACT; avoids the slow DVE recip
                    rsq = dpool.tile([128, 2 * S], f32, tag="rsq", name="rsq")
                    nc.scalar.activation(rsq[:], r_ps[:], A.Abs_reciprocal_sqrt)
                    rbi = dpool.tile([128, 2 * S], f32, tag="rbi", name="rbi")
                    nc.vector.tensor_tensor(rbi[:], rsq[:], rsq[:], OP.mult)
                    o = pool.tile([128, S], st, tag=f"{out_tag}{c}",
                                  name=f"{out_tag}{c}")
                    for hh in range(2):
                        nc.vector.tensor_tensor(
                            o[hh * 64:(hh + 1) * 64, :], o_ps_h[hh][0:64, :],
                            rbi[hh * 64:(hh + 1) * 64, hh * S:(hh + 1) * S],
                            OP.mult)
                    outs.append(o)
            return outs

        h1 = [pers.tile([128, S], f32, tag=f"h1T{k}", name=f"h1T{k}")
              for k in range(DC)]
        h2 = [pers.tile([128, S], f32, tag=f"h2T{k}", name=f"h2T{k}")
              for k in range(DC)]

        # w2 fully resident before the MoE starts (4 sliced DMAs on the
        # store queue, issued up front; overlaps the attention phases)
        w2_all = pers.tile([128, FC * D], st, tag="w2_all", name="w2_all")
        for q in range(4):
            w = FC * D // 4
            eng_s.dma_start(w2_all[:, q * w:(q + 1) * w],
                            di["w2"].ap()[:, q * w:(q + 1) * w])

        # ---------------- self attention ----------------
        with tc.tile_pool(name="sa_acts", bufs=1) as sa:
            xT = load_chunks(di["xT"], DC, S, "xT", sa, f32)
            xT_st = xT if same_st else load_chunks(di["xT_st"], DC, S, "xTs", sa)
            n1 = layernorm(xT, xT_st, "n1T", sa)
            with tc.tile_pool(name="wqkvp", bufs=1) as wp:
                wt = load_chunks(di["wqkv"], DC, WPACK, "wqkv", wp)
                qT = project_fm([t[:, W_Q:W_Q + D] for t in wt], n1, DC,
                                bias("qb"), "qT", sa)
                kT = project_fm([t[:, W_K:W_K + 2 * KVW] for t in wt], n1, 4,
                                bias("kb"), "kT", sa)
                v_tm = project_tm(n1, [t[:, W_V:W_V + KVW] for t in wt], SC,
                                  bias("vb"), "v_tm", sa)
            with tc.tile_pool(name="wop", bufs=1) as wp:
                ow_t = load_chunks(di["ow"], DC, D, "ow", wp)
                sa_out = attend(qT, kT, v_tm, SC, maskT, sa_cls, "saT", sa)

                def o_epil(mI, ps, o):
                    nc.vector.scalar_tensor_tensor(o[:], ps[:],
                                                   bias("ob")[:, mI:mI + 1],
                                                   xT[mI][:], OP.add, OP.add)
                project_fm(ow_t, sa_out, DC, None, "h1w", _FixedPool(h1),
                           extra=o_epil, out_dt=f32)

        # ---------------- cross attention ----------------
        with tc.tile_pool(name="ca_acts", bufs=1) as ca:
            encT = load_chunks(di["encT"], DC, SK, "encT", ca, engine=eng_b)
            h1_st = cast_st(h1, "h1s", ca)
            with tc.tile_pool(name="wcap", bufs=1) as wp:
                wt = load_chunks(di["wca"], DC, WPACK, "wca", wp, engine=eng_b)
                k2T = project_fm([t[:, W_K:W_K + 2 * KVW] for t in wt], encT, 4,
                                 bias("k2b"), "k2T", ca, width=SK)
                v2_tm = project_tm(encT, [t[:, W_V:W_V + KVW] for t in wt], KC,
                                   bias("v2b"), "v2_tm", ca)
                n2 = layernorm(h1, h1_st, "n2T", ca)
                q2T = project_fm([t[:, W_Q:W_Q + D] for t in wt], n2, DC,
                                 bias("q2b"), "q2T", ca)
            with tc.tile_pool(name="wo2p", bufs=1) as wp:
                o2w_t = load_chunks(di["o2w"], DC, D, "o2w", wp, engine=eng_b)
                ca_out = attend(q2T, k2T, v2_tm, KC, encmaskT, ca_cls, "caT", ca)

                def o2_epil(mI, ps, o):
                    nc.vector.scalar_tensor_tensor(o[:], ps[:],
                                                   bias("o2b")[:, mI:mI + 1],
                                                   h1[mI][:], OP.add, OP.add)
                project_fm(o2w_t, ca_out, DC, None, "h2w", _FixedPool(h2),
                           extra=o2_epil, out_dt=f32)

        # residual output (host: out_b = res.T + ffn_j0 + ffn_j1)
        for k in range(DC):
            eng_s.dma_start(out_res.ap()[k * 128:(k + 1) * 128, :], h2[k][:])

        # ---------------- MoE expert ----------------
        with tc.tile_pool(name="moe_acts", bufs=1) as mo:
            h2_st = cast_st(h2, "h2s", mo)
            n3 = layernorm(h2, h2_st, "n3T", mo)

            with tc.tile_pool(name="w13p", bufs=3) as wp, \
                 tc.tile_pool(name="mTp", bufs=6) as mp, \
                 tc.tile_pool(name="gh_ps", bufs=3, space="PSUM") as gp, \
                 tc.tile_pool(name="y_ps", bufs=1, space="PSUM") as yp, \
                 tc.tile_pool(name="gelu_t", bufs=3) as gt, \
                 tc.tile_pool(name="outp", bufs=2) as op_:
                y_ps = [[yp.tile([128, 512], f32, tag=f"y{t}{n}", name=f"y{t}{n}")
                         for n in range(2)] for t in range(QC)]
                gw = GRP * 128
                GW = DC * 1024
                for g in range(NGRP):
                    wgt = wp.tile([128, GW], st, tag="w13g", name="w13g")
                    nc.sync.dma_start(wgt[:], di["w13"].ap()[:, g * GW:(g + 1) * GW])
                    wg = [wgt[:, k * 1024:(k + 1) * 1024] for k in range(DC)]
                    for mi in range(GRP):
                        mI = g * GRP + mi
                        # one PSUM bank: gelu-arg in [0:S], mult-arg in [S:2S]
                        gh = gp.tile([128, 2 * S], f32, tag="gh", name="gh")
                        for k in range(DC):
                            mm(gh[:, 0:S], wg[k][:, mi * 128:(mi + 1) * 128],
                               n3[k][:], k == 0, k == DC - 1)
                        for k in range(DC):
                            mm(gh[:, S:2 * S],
                               wg[k][:, gw + mi * 128:gw + (mi + 1) * 128],
                               n3[k][:], k == 0, k == DC - 1)
                        ge = gt.tile([128, S], f32, tag="ge", name="ge")
                        nc.scalar.activation(ge[:], gh[:, 0:S], A.Gelu,
                                             bias=bias("b1")[:, mI:mI + 1])
                        mT = mp.tile([128, S], st, tag="mT", name="mT")
                        nc.vector.scalar_tensor_tensor(mT[:], gh[:, S:2 * S],
                                                       bias("b3")[:, mI:mI + 1],
                                                       ge[:], OP.add, OP.mult)
                        # fused down-projection: w2 already resident
                        for t in range(QC):
                            for n in range(2):
                                mm(y_ps[t][n][:], mT[:, t * 128:(t + 1) * 128],
                                   w2_all[:, mI * D + n * 512:
                                          mI * D + (n + 1) * 512],
                                   mI == 0, mI == FC - 1)
                for t in range(QC):
                    for n in range(2):
                        o = op_.tile([128, 512], f32, tag="o_out", name="o_out")
                        nc.vector.tensor_scalar_mul(o[:], y_ps[t][n][:],
                                                    bias("c")[:, 0:1])
                        eng_s.dma_start(
                            out_ffn.ap()[t * 128:(t + 1) * 128,
                                         n * 512:(n + 1) * 512], o[:])

    nc.compile()
    return nc


class _FixedPool:
    """Adapter letting project_fm write into pre-allocated tiles."""

    def __init__(self, tiles):
        self._tiles = list(tiles)
        self._i = 0

    def tile(self, shape, dtype, tag=None, name=None):
        t = self._tiles[self._i]
        self._i += 1
        return t


def _routing(langs):
    """Per-sequence expert slots [(expert_idx, coef) x2], matching the
    reference: coef[e,b] = any(langs[b]==4+e) * (1/count(langs[b]>3))."""
    langs = np.asarray(langs)
    slots = []
    for b in range(langs.shape[0]):
        row = [int(v) for v in langs[b]]
        cnt = sum(1 for v in row if v > 3)
        rw = 1.0 if cnt == 0 else 1.0 / cnt
        seen = []
        for v in row:
            if v > 3 and 0 <= v - 4 < NE and (v - 4) not in seen:
                seen.append(v - 4)
        sl = [(e, rw) for e in seen]
        while len(sl) < 2:
            sl.append((0, 0.0))
        slots.append(sl[:2])
    return slots


def _mask_classes(maskT, n_kc):
    """Classify each [128 keys x 128 queries] block of a transposed mask:
    0 all-zero (no add), 1 general (add), 2 fully masked (skip compute).
    Keeps at least one active key block per query and contiguous active
    ranges per key chunk."""
    cls = []
    for kc in range(n_kc):
        for qh in range(QC):
            blk = maskT[kc * 128:(kc + 1) * 128, qh * 128:(qh + 1) * 128]
            if np.all(blk == 0):
                cls.append(0)
            elif np.all(blk <= -1e8):
                cls.append(2)
            else:
                cls.append(1)
    for qh in range(QC):
        if all(cls[kc * QC + qh] == 2 for kc in range(n_kc)):
            for kc in range(n_kc):
                cls[kc * QC + qh] = 1
    for kc in range(n_kc):
        act = [q for q in range(QC) if cls[kc * QC + q] != 2]
        if not act or act != list(range(act[0], act[-1] + 1)):
            for q in range(QC):
                if cls[kc * QC + q] == 2:
                    cls[kc * QC + q] = 1
    return tuple(cls)


def kernel(**inputs):
    mode = MODE
    np_dt = ml_dtypes.bfloat16 if mode == "bf16" else np.float32
    f32 = np.float32

    inp = {k: np.asarray(v) for k, v in inputs.items()}
    x = inp["hidden_states"].astype(f32)
    enc = inp["encoder_hidden_states"].astype(f32)
    mask = inp["attention_mask"].astype(f32)
    encmask = inp["encoder_attention_mask"].astype(f32)
    g1, b1 = inp["ln1_g"].astype(f32), inp["ln1_b"].astype(f32)
    g2, b2 = inp["ln2_g"].astype(f32), inp["ln2_b"].astype(f32)
    g3, b3 = inp["ln3_g"].astype(f32), inp["ln3_b"].astype(f32)

    def dup_kv(w):
        return np.concatenate([np.tile(w[:, 64 * j:64 * (j + 1)], (1, 2))
                               for j in range(NKV)], axis=1)

    def dup_kv_b(v):
        return np.concatenate([np.tile(v[64 * j:64 * (j + 1)], 2)
                               for j in range(NKV)])

    sc = HD ** -0.5
    qw_f = g1[:, None] * inp["sa_q_w"] * sc
    qb_f = (b1 @ inp["sa_q_w"] + inp["sa_q_b"]) * sc
    kw_f = dup_kv(g1[:, None] * inp["sa_k_w"])
    kb_f = dup_kv_b(b1 @ inp["sa_k_w"] + inp["sa_k_b"])
    vw_f = g1[:, None] * inp["sa_v_w"]
    vb_f = b1 @ inp["sa_v_w"] + inp["sa_v_b"]
    q2w_f = g2[:, None] * inp["ca_q_w"] * sc
    q2b_f = (b2 @ inp["ca_q_w"] + inp["ca_q_b"]) * sc
    k2w_f = dup_kv(inp["ca_k_w"])
    k2b_f = dup_kv_b(inp["ca_k_b"])
    w1_f = inp["moe_w1"] * g3[None, :, None]
    b1_f = np.einsum("d,edf->ef", b3, inp["moe_w1"]).astype(f32)
    w3_f = inp["moe_w3"] * g3[None, :, None]
    b3_f = np.einsum("d,edf->ef", b3, inp["moe_w3"]).astype(f32)

    maskT0 = np.ascontiguousarray(mask[:, 0].transpose(0, 2, 1))     # [B,S,S]
    encmaskT0 = np.ascontiguousarray(encmask[:, 0].transpose(0, 2, 1))
    sa_cls = _mask_classes(maskT0[0], SC)
    ca_cls = _mask_classes(encmaskT0[0], KC)
    for b in range(1, B):
        if _mask_classes(maskT0[b], SC) != sa_cls or \
           _mask_classes(encmaskT0[b], KC) != ca_cls:
            sa_cls = tuple(1 for _ in range(SC * QC))
            ca_cls = tuple(1 for _ in range(KC * QC))
            break

    key = (mode, sa_cls, ca_cls)
    if key not in _CACHE:
        _CACHE[key] = _build(mode, sa_cls, ca_cls)
    nc = _CACHE[key]

    def col128(v):
        return np.asarray(v, f32).reshape(-1, 128).T

    def pack_k(w):
        w = np.asarray(w)
        return np.concatenate([w[k * 128:(k + 1) * 128, :]
                               for k in range(w.shape[0] // 128)], axis=1)

    slots = _routing(inp["langs"])
    wqkv = np.concatenate([qw_f, kw_f, vw_f], axis=1).astype(np_dt)
    wca = np.concatenate([q2w_f, k2w_f, inp["ca_v_w"]], axis=1).astype(np_dt)

    bias_common = np.zeros((128, BIAS_W), f32)
    for nm, v in [("qb", col128(qb_f)), ("kb", col128(kb_f)),
                  ("vb", np.broadcast_to(vb_f.astype(f32), (128, KVW))),
                  ("ob", col128(inp["sa_o_b"])),
                  ("q2b", col128(q2b_f)), ("k2b", col128(k2b_f)),
                  ("v2b", np.broadcast_to(inp["ca_v_b"].astype(f32), (128, KVW))),
                  ("o2b", col128(inp["ca_o_b"]))]:
        off, w = _BIAS_COLS[nm]
        bias_common[:, off:off + w] = v

    in_maps = []
    for c in range(8):
        b, j = c // 2, c % 2
        e, coef = slots[b][j]
        xT = pack_k(np.ascontiguousarray(x[b].T))
        # interleave w1/w3 by group: [w1 grp g | w3 grp g] blocks of 512 cols
        gw = GRP * 128
        w13 = np.empty((128, NGRP * DC * 1024), f32)
        for g in range(NGRP):
            for k in range(DC):
                c0 = (g * DC + k) * 1024
                w13[:, c0:c0 + 512] = w1_f[e][k * 128:(k + 1) * 128,
                                              g * 512:(g + 1) * 512]
                w13[:, c0 + 512:c0 + 1024] = w3_f[e][k * 128:(k + 1) * 128,
                                                     g * 512:(g + 1) * 512]
        bt = bias_common.copy()
        for nm, v in [("b1", col128(b1_f[e])), ("b3", col128(b3_f[e]))]:
            off, w = _BIAS_COLS[nm]
            bt[:, off:off + w] = v
        bt[:, _BIAS_COLS["c"][0]] = coef
        m = {
            "xT": xT,
            "encT": pack_k(np.ascontiguousarray(enc[b].T)).astype(np_dt),
            "id128": np.eye(128, dtype=f32).astype(np_dt),
            "ones_col": np.ones((128, 1), f32).astype(np_dt),
            "ones_row": np.ones((1, 128), f32).astype(np_dt),
            "wqkv": pack_k(wqkv), "wca": pack_k(wca),
            "ow": pack_k(inp["sa_o_w"].astype(np_dt)),
            "o2w": pack_k(inp["ca_o_w"].astype(np_dt)),
            "biases": bt,
            "w13": w13.astype(np_dt),
            "w2": pack_k(np.ascontiguousarray(inp["moe_w2"][e])).astype(np_dt),
        }
        if mode != "f32":
            m["xT_st"] = xT.astype(np_dt)
        if any(cc == 1 for cc in sa_cls):
            m["maskT"] = maskT0[b].astype(np_dt)
        if any(cc == 1 for cc in ca_cls):
            m["encmaskT"] = encmaskT0[b].astype(np_dt)
        in_maps.append(m)

    kw = {}
    if _TRACE_DIR:
        kw = dict(trace=True, tmpdir=_TRACE_DIR, trace_cores=[0])
    res = bass_utils.run_bass_kernel_spmd(nc, in_maps, core_ids=list(range(8)), **kw)
    global _LAST_EXEC_NS
    _LAST_EXEC_NS = res.exec_time_ns
    return np.stack([
        res.results[2 * b]["out_res"].T
        + res.results[2 * b]["out_ffn"]
        + res.results[2 * b + 1]["out_ffn"]
        for b in range(B)
    ]).astype(f32)



# revision 21
# speedup vs baseline: 1.0696x; 1.0355x over previous
"""MBart MoE decoder layer on 8 trn2 NeuronCores.

Sharding: 8 cores = 8 (sequence, expert-slot) pairs. Core c handles
sequence b=c//2, expert slot j=c%2 (each sequence is lang-routed to at
most 2 distinct experts; routing is computed on the host from `langs`).
Each core computes the full attention path for its sequence (replicated
across the pair) and one expert FFN over all 256 tokens; the host sums
the pair's partial outputs (expert-sharded combine) and transposes back
to token-major. Expert weights are gathered per-core on the host, so a
core only receives the one expert it needs.

On-device layout is feature-major [D, tokens]: projections take weights
as lhsT (feature-major out) or activations as lhsT (token-major out), so
no activation transposes are needed anywhere. LN gains/biases are folded
into the downstream weights on the host; softmax uses transposed scores
[keys, queries] with the attention mask added via an identity-matmul
into PSUM (host classifies each 128x128 mask block as zero / add / skip,
so causal dead blocks are never computed) and denominators accumulated
via a ones-matmul, then broadcast over partitions with a rank-1 matmul
for one full-lane reciprocal per head pair.
"""

import os
import sys
from contextlib import ExitStack

for _p in ("/opt/trn_rl_repo",):
    if _p not in sys.path:
        sys.path.append(_p)

import numpy as np
import ml_dtypes

import concourse.bass as bass
import concourse.tile as tile
import concourse.mybir as mybir
from concourse import bacc, bass_utils

B, S, SK = 4, 256, 512
D, NH, NKV, HD = 1024, 16, 4, 64
DE, NE = 4096, 8
LN_EPS = 1e-5
REP = NH // NKV
DC = D // 128    # 8 feature chunks
FC = DE // 128   # 32 ffn chunks
SC = S // 128    # 2 self-attn key chunks
KC = SK // 128   # 4 cross-attn key chunks
QC = S // 128    # 2 query halves
KVW = NKV * HD   # 256
GRP = 4          # ffn chunks per MoE weight group
NGRP = FC // GRP

MODE = os.environ.get("KERNEL_MM_DTYPE", "bf16")  # "bf16" | "f32r" | "f32"

_CACHE: dict = {}
_TRACE_DIR = None   # set by test harness for profiling runs
_LAST_EXEC_NS = None

# packed attention-weight column layout: qw | kw(dup) | vw
W_Q, W_K, W_V = 0, D, D + 2 * KVW
WPACK = D + 2 * KVW + KVW  # 1792

# packed per-partition bias column layout
_BIAS_COLS = {}
_off = 0
for _n, _w in [("qb", DC), ("kb", 4), ("vb", KVW), ("ob", DC),
               ("q2b", DC), ("k2b", 4), ("v2b", KVW), ("o2b", DC),
               ("b1", FC), ("b3", FC), ("c", 1)]:
    _BIAS_COLS[_n] = (_off, _w)
    _off += _w
BIAS_W = _off


def _build(mode, sa_cls, ca_cls):
    """sa_cls/ca_cls: block classes per (kc, qhalf): 0=no-mask, 1=mask-add,
    2=fully-masked(skip)."""
    st = {"bf16": mybir.dt.bfloat16, "f32r": mybir.dt.float32r,
          "f32": mybir.dt.float32}[mode]
    f32 = mybir.dt.float32
    same_st = mode == "f32"
    A = mybir.ActivationFunctionType
    OP = mybir.AluOpType

    nc = bacc.Bacc("TRN2", target_bir_lowering=False, debug=False, num_devices=8)
    import os as _os
    _SPLIT = _os.environ.get("KERNEL_DMA_SPLIT", "1") == "1"
    eng_b = nc.scalar if _SPLIT else nc.sync
    eng_s = nc.gpsimd if _SPLIT else nc.sync

    def mm(psum, lhsT, rhs, start, stop):
        nc.tensor.matmul(psum, lhsT, rhs, start=start, stop=stop)

    di = {}

    def din(name, shape, dtype=None):
        di[name] = nc.dram_tensor(name, list(shape), dtype or st, kind="ExternalInput")
        return di[name]

    din("xT", (128, DC * S), f32)
    if not same_st:
        din("xT_st", (128, DC * S))
    din("encT", (128, DC * SK))
    need_samask = any(c == 1 for c in sa_cls)
    need_camask = any(c == 1 for c in ca_cls)
    if need_samask:
        din("maskT", (S, S))
    if need_camask:
        din("encmaskT", (SK, S))
    din("id128", (128, 128))
    din("ones_col", (128, 1))
    din("ones_row", (1, 128))
    din("wqkv", (128, DC * WPACK))
    din("wca", (128, DC * WPACK))
    din("ow", (128, DC * D))
    din("o2w", (128, DC * D))
    din("biases", (128, BIAS_W), f32)
    din("w13", (128, NGRP * DC * 1024))  # per grp, per k: [w1 512 | w3 512]
    din("w2", (128, FC * D))
    out_res = nc.dram_tensor("out_res", [D, S], f32, kind="ExternalOutput")
    out_ffn = nc.dram_tensor("out_ffn", [S, D], f32, kind="ExternalOutput")

    with tile.TileContext(nc) as tc, ExitStack() as ctx:
        cp = ctx.enter_context(tc.tile_pool(name="consts", bufs=1))
        pers = ctx.enter_context(tc.tile_pool(name="pers", bufs=1))

        ones128 = cp.tile([128, 1], st, tag="ones128", name="ones128")
        eng_b.dma_start(ones128[:], di["ones_col"].ap())
        ones1r = cp.tile([1, 128], st, tag="ones1r", name="ones1r")
        eng_b.dma_start(ones1r[:], di["ones_row"].ap())
        eps_t = cp.tile([128, 1], f32, tag="eps_t", name="eps_t")
        nc.vector.memset(eps_t, LN_EPS)
        id128 = cp.tile([128, 128], st, tag="id128", name="id128")
        eng_b.dma_start(id128[:], di["id128"].ap())
        maskT = encmaskT = None
        if need_samask:
            maskT = cp.tile([128, SC, S], st, tag="maskT", name="maskT")
            for kc in range(SC):
                eng_b.dma_start(maskT[:, kc, :],
                                  di["maskT"].ap()[kc * 128:(kc + 1) * 128, :])
        if need_camask:
            encmaskT = cp.tile([128, KC, S], st, tag="encmaskT", name="encmaskT")
            for kc in range(KC):
                eng_b.dma_start(encmaskT[:, kc, :],
                                  di["encmaskT"].ap()[kc * 128:(kc + 1) * 128, :])

        bias_t = cp.tile([128, BIAS_W], f32, tag="bias_t", name="bias_t")
        eng_b.dma_start(bias_t[:], di["biases"].ap())

        def bias(nm):
            off, w = _BIAS_COLS[nm]
            return bias_t[:, off:off + w]

        def load_chunks(dram, nchunk, width, tag, pool, dtype=st, engine=None):
            t = pool.tile([128, nchunk * width], dtype, tag=tag, name=tag)
            (engine or nc.sync).dma_start(t[:], dram.ap())
            return [t[:, k * width:(k + 1) * width] for k in range(nchunk)]

        def layernorm(src_f32, src_st, out_tag, pool):
            """src: DC chunks [128,S] f32 (+st copies). Returns DC normalized
            chunks [128,S] st (gain/bias folded downstream by host)."""
            with tc.tile_pool(name=f"{out_tag}_lt", bufs=2) as lp, \
                 tc.tile_pool(name=f"{out_tag}_lp", bufs=1, space="PSUM") as sp, \
                 tc.tile_pool(name=f"{out_tag}_lb", bufs=1, space="PSUM") as bp:
                sum_ps = sp.tile([1, S], f32, tag="lnsum", name="lnsum")
                sq_ps = sp.tile([1, S], f32, tag="lnsq", name="lnsq")
                for k in range(DC):
                    sq = lp.tile([128, S], st, tag="lnsqt", name="lnsqt")
                    nc.vector.tensor_tensor(sq[:], src_f32[k][:], src_f32[k][:],
                                            OP.mult)
                    mm(sum_ps[:], ones128[:], src_st[k][:], k == 0, k == DC - 1)
                    mm(sq_ps[:], ones128[:], sq[:], k == 0, k == DC - 1)
                s_sb = lp.tile([1, S], st, tag="ln_ssb", name="ln_ssb")
                nc.vector.tensor_single_scalar(s_sb[:], sum_ps[:], 1.0 / D, OP.mult)
                q_sb = lp.tile([1, S], st, tag="ln_qsb", name="ln_qsb")
                nc.vector.tensor_single_scalar(q_sb[:], sq_ps[:], 1.0 / D, OP.mult)
                s_bc = bp.tile([128, S], f32, tag="ln_sbc", name="ln_sbc")
                q_bc = bp.tile([128, S], f32, tag="ln_qbc", name="ln_qbc")
                mm(s_bc[:], ones1r[:], s_sb[:], True, True)   # mean, bcast
                mm(q_bc[:], ones1r[:], q_sb[:], True, True)   # E[x^2], bcast
                # full-lane stats math on [128,S]
                s_sbuf = lp.tile([128, S], f32, tag="ln_ssbuf", name="ln_ssbuf")
                nc.vector.tensor_copy(s_sbuf[:], s_bc[:])
                var = lp.tile([128, S], f32, tag="ln_var", name="ln_var")
                nc.vector.scalar_tensor_tensor(var[:], s_bc[:], 0.0, s_sbuf[:],
                                               OP.bypass, OP.mult)
                nc.vector.tensor_sub(var[:], q_bc[:], var[:])
                v_t = lp.tile([128, S], f32, tag="ln_vt", name="ln_vt")
                nc.scalar.activation(v_t[:], var[:], A.Abs_reciprocal_sqrt,
                                     bias=eps_t[:])
                u_t = lp.tile([128, S], f32, tag="ln_ut", name="ln_ut")
                nc.vector.tensor_tensor(u_t[:], s_sbuf[:], v_t[:], OP.mult)
                outs = []
                for k in range(DC):
                    o = pool.tile([128, S], st, tag=f"{out_tag}{k}",
                                  name=f"{out_tag}{k}")
                    nc.vector.tensor_tensor(o[:], src_f32[k][:], v_t[:], OP.mult)
                    nc.vector.tensor_sub(o[:], o[:], u_t[:])
                    outs.append(o)
                return outs

        def cast_st(src, tag, pool):
            if same_st:
                return src
            outs = []
            for k, t in enumerate(src):
                o = pool.tile([128, t.shape[-1]], st, tag=f"{tag}{k}",
                              name=f"{tag}{k}")
                nc.vector.tensor_copy(o[:], t[:])
                outs.append(o)
            return outs

        def project_fm(w_slices, rhs_chunks, nout, bias_ap, out_tag, pool,
                       extra=None, out_dt=None, width=None):
            """out^T[dout_chunk] = sum_k w_slices[k][:, m*128:...].T @ rhs[k]."""
            W = width or S
            outs = []
            with tc.tile_pool(name=f"{out_tag}_ps", bufs=3, space="PSUM") as pp:
                for mI in range(nout):
                    ps = pp.tile([128, W], f32, tag="proj", name="proj")
                    for k in range(DC):
                        mm(ps[:], w_slices[k][:, mI * 128:(mI + 1) * 128],
                           rhs_chunks[k][:], k == 0, k == DC - 1)
                    o = pool.tile([128, W], out_dt or st, tag=f"{out_tag}{mI}",
                                  name=f"{out_tag}{mI}")
                    if extra is not None:
                        extra(mI, ps, o)
                    elif bias_ap is not None:
                        nc.vector.tensor_scalar(o[:], ps[:],
                                                bias_ap[:, mI:mI + 1], None,
                                                OP.add)
                    else:
                        nc.vector.tensor_copy(o[:], ps[:])
                    outs.append(o)
            return outs

        def project_tm(act_chunks, w_slices, ntok, bias_bcast, out_tag, pool):
            """token-major V with a ones column appended per kv head:
            out[tok_chunk] = [V_kv | 1] blocks of 65 columns."""
            outs = []
            with tc.tile_pool(name=f"{out_tag}_ps", bufs=3, space="PSUM") as pp:
                for t in range(ntok):
                    ps = pp.tile([128, KVW], f32, tag="projtm", name="projtm")
                    for k in range(DC):
                        mm(ps[:], act_chunks[k][:, t * 128:(t + 1) * 128],
                           w_slices[k][:], k == 0, k == DC - 1)
                    o = pool.tile([128, NKV, HD + 1], st, tag=f"{out_tag}{t}",
                                  name=f"{out_tag}{t}")
                    nc.vector.tensor_add(
                        o[:, :, 0:HD],
                        ps[:].rearrange("p (kv d) -> p kv d", kv=NKV),
                        bias_bcast[:].rearrange("p (kv d) -> p kv d", kv=NKV))
                    for kv in range(NKV):
                        nc.vector.tensor_copy(o[:, kv, HD:HD + 1], ones128[:])
                    outs.append(o)
            return outs

        def attend(qT, kT, vtm, n_kc, mask_tile, cls, out_tag, pool):
            """Baseline-structure attention; mask applied multiplicatively on
            the DVE (host ships exp(mask)) instead of via id128 matmuls."""
            outs = []
            qr = []
            for kc in range(n_kc):
                act = [qh for qh in range(QC) if cls[kc * QC + qh] != 2]
                assert act and act == list(range(act[0], act[-1] + 1))
                qr.append((act[0] * 128, (act[-1] + 1) * 128))
            with tc.tile_pool(name=f"{out_tag}_sp", bufs=3, space="PSUM") as stp, \
                 tc.tile_pool(name=f"{out_tag}_op", bufs=2, space="PSUM") as opp, \
                 tc.tile_pool(name=f"{out_tag}_bp", bufs=1, space="PSUM") as bpp, \
                 tc.tile_pool(name=f"{out_tag}_et", bufs=6) as epool, \
                 tc.tile_pool(name=f"{out_tag}_dt", bufs=3) as dpool:
                for c in range(DC):
                    o_ps_h = [opp.tile([65, S], f32, tag=f"oph{hh}",
                                       name=f"oph{hh}") for hh in range(2)]
                    kv = (2 * c) // REP      # same kv head for both of the pair
                    for kc in range(n_kc):
                        q0, q1 = qr[kc]
                        adds = [q for q in range(QC) if cls[kc * QC + q] == 1]
                        st_h = []
                        e_h = []
                        for hh in range(2):
                            qh_ap = qT[c][hh * 64:(hh + 1) * 64, :]
                            kh = kT[kv][hh * 64:(hh + 1) * 64, :]
                            st_ps = stp.tile([128, S], f32, tag="st",
                                             name="st")
                            mm(st_ps[:, q0:q1], kh[:, kc * 128:(kc + 1) * 128],
                               qh_ap[:, q0:q1], True, True)
                            st_h.append(st_ps)
                        for hh in range(2):
                            e = epool.tile([128, S], st, tag="e", name="e")
                            nc.scalar.activation(e[:, q0:q1],
                                                 st_h[hh][:, q0:q1], A.Exp)
                            for q in adds:
                                nc.vector.tensor_tensor(
                                    e[:, q * 128:(q + 1) * 128],
                                    e[:, q * 128:(q + 1) * 128],
                                    mask_tile[:, kc, q * 128:(q + 1) * 128],
                                    OP.mult)
                            e_h.append(e)
                        for hh in range(2):
                            mm(o_ps_h[hh][:, q0:q1],
                               vtm[kc][:, kv, :], e_h[hh][:, q0:q1],
                               kc == 0, kc == n_kc - 1)
                    den_pair = dpool.tile([1, 2 * S], st, tag="den_pair",
                                          name="den_pair")
                    for hh in range(2):
                        nc.vector.tensor_copy(den_pair[:, hh * S:(hh + 1) * S],
                                              o_ps_h[hh][64:65, :])
                    r_ps = bpp.tile([128, 2 * S], f32, tag="rbc", name="rbc")
                    mm(r_ps[:], ones1r[:], den_pair[:], True, True)
                    # 1/x as (1/sqrt(x))^2 on ACT; avoids the slow DVE recip
                    rsq = dpool.tile([128, 2 * S], f32, tag="rsq", name="rsq")
                    nc.scalar.activation(rsq[:], r_ps[:], A.Abs_reciprocal_sqrt)
                    rbi = dpool.tile([128, 2 * S], f32, tag="rbi", name="rbi")
                    nc.vector.tensor_tensor(rbi[:], rsq[:], rsq[:], OP.mult)
                    o = pool.tile([128, S], st, tag=f"{out_tag}{c}",
                                  name=f"{out_tag}{c}")
                    for hh in range(2):
                        nc.vector.tensor_tensor(
                            o[hh * 64:(hh + 1) * 64, :], o_ps_h[hh][0:64, :],
                            rbi[hh * 64:(hh + 1) * 64, hh * S:(hh + 1) * S],
                            OP.mult)
                    outs.append(o)
            return outs

        h1 = [pers.tile([128, S], f32, tag=f"h1T{k}", name=f"h1T{k}")
              for k in range(DC)]
        h2 = [pers.tile([128, S], f32, tag=f"h2T{k}", name=f"h2T{k}")
              for k in range(DC)]

        # w2 fully resident before the MoE starts (4 sliced DMAs on the
        # store queue, issued up front; overlaps the attention phases)
        w2_all = pers.tile([128, FC * D], st, tag="w2_all", name="w2_all")
        for q in range(4):
            w = FC * D // 4
            eng_s.dma_start(w2_all[:, q * w:(q + 1) * w],
                            di["w2"].ap()[:, q * w:(q + 1) * w])

        # ---------------- self attention ----------------
        with tc.tile_pool(name="sa_acts", bufs=1) as sa:
            xT = load_chunks(di["xT"], DC, S, "xT", sa, f32)
            xT_st = xT if same_st else load_chunks(di["xT_st"], DC, S, "xTs", sa)
            n1 = layernorm(xT, xT_st, "n1T", sa)
            with tc.tile_pool(name="wqkvp", bufs=1) as wp:
                wt = load_chunks(di["wqkv"], DC, WPACK, "wqkv", wp)
                qT = project_fm([t[:, W_Q:W_Q + D] for t in wt], n1, DC,
                                bias("qb"), "qT", sa)
                kT = project_fm([t[:, W_K:W_K + 2 * KVW] for t in wt], n1, 4,
                                bias("kb"), "kT", sa)
                v_tm = project_tm(n1, [t[:, W_V:W_V + KVW] for t in wt], SC,
                                  bias("vb"), "v_tm", sa)
            with tc.tile_pool(name="wop", bufs=1) as wp:
                ow_t = load_chunks(di["ow"], DC, D, "ow", wp)
                sa_out = attend(qT, kT, v_tm, SC, maskT, sa_cls, "saT", sa)

                def o_epil(mI, ps, o):
                    nc.vector.scalar_tensor_tensor(o[:], ps[:],
                                                   bias("ob")[:, mI:mI + 1],
                                                   xT[mI][:], OP.add, OP.add)
                project_fm(ow_t, sa_out, DC, None, "h1w", _FixedPool(h1),
                           extra=o_epil, out_dt=f32)

        # ---------------- cross attention ----------------
        with tc.tile_pool(name="ca_acts", bufs=1) as ca:
            encT = load_chunks(di["encT"], DC, SK, "encT", ca, engine=eng_b)
            h1_st = cast_st(h1, "h1s", ca)
            with tc.tile_pool(name="wcap", bufs=1) as wp:
                wt = load_chunks(di["wca"], DC, WPACK, "wca", wp, engine=eng_b)
                k2T = project_fm([t[:, W_K:W_K + 2 * KVW] for t in wt], encT, 4,
                                 bias("k2b"), "k2T", ca, width=SK)
                v2_tm = project_tm(encT, [t[:, W_V:W_V + KVW] for t in wt], KC,
                                   bias("v2b"), "v2_tm", ca)
                n2 = layernorm(h1, h1_st, "n2T", ca)
                q2T = project_fm([t[:, W_Q:W_Q + D] for t in wt], n2, DC,
                                 bias("q2b"), "q2T", ca)
            with tc.tile_pool(name="wo2p", bufs=1) as wp:
                o2w_t = load_chunks(di["o2w"], DC, D, "o2w", wp, engine=eng_b)
                ca_out = attend(q2T, k2T, v2_tm, KC, encmaskT, ca_cls, "caT", ca)

                def o2_epil(mI, ps, o):
                    nc.vector.scalar_tensor_tensor(o[:], ps[:],
                                                   bias("o2b")[:, mI:mI + 1],
                                                   h1[mI][:], OP.add, OP.add)
                project_fm(o2w_t, ca_out, DC, None, "h2w", _FixedPool(h2),
                           extra=o2_epil, out_dt=f32)

        # residual output (host: out_b = res.T + ffn_j0 + ffn_j1)
        for k in range(DC):
            eng_s.dma_start(out_res.ap()[k * 128:(k + 1) * 128, :], h2[k][:])

        # ---------------- MoE expert ----------------
        with tc.tile_pool(name="moe_acts", bufs=1) as mo:
            h2_st = cast_st(h2, "h2s", mo)
            n3 = layernorm(h2, h2_st, "n3T", mo)

            with tc.tile_pool(name="w13p", bufs=3) as wp, \
                 tc.tile_pool(name="mTp", bufs=6) as mp, \
                 tc.tile_pool(name="gh_ps", bufs=3, space="PSUM") as gp, \
                 tc.tile_pool(name="y_ps", bufs=1, space="PSUM") as yp, \
                 tc.tile_pool(name="gelu_t", bufs=3) as gt, \
                 tc.tile_pool(name="outp", bufs=2) as op_:
                y_ps = [[yp.tile([128, 512], f32, tag=f"y{t}{n}", name=f"y{t}{n}")
                         for n in range(2)] for t in range(QC)]
                gw = GRP * 128
                GW = DC * 1024
                for g in range(NGRP):
                    wgt = wp.tile([128, GW], st, tag="w13g", name="w13g")
                    nc.sync.dma_start(wgt[:], di["w13"].ap()[:, g * GW:(g + 1) * GW])
                    wg = [wgt[:, k * 1024:(k + 1) * 1024] for k in range(DC)]
                    for mi in range(GRP):
                        mI = g * GRP + mi
                        # one PSUM bank: gelu-arg in [0:S], mult-arg in [S:2S]
                        gh = gp.tile([128, 2 * S], f32, tag="gh", name="gh")
                        for k in range(DC):
                            mm(gh[:, 0:S], wg[k][:, mi * 128:(mi + 1) * 128],
                               n3[k][:], k == 0, k == DC - 1)
                        for k in range(DC):
                            mm(gh[:, S:2 * S],
                               wg[k][:, gw + mi * 128:gw + (mi + 1) * 128],
                               n3[k][:], k == 0, k == DC - 1)
                        ge = gt.tile([128, S], f32, tag="ge", name="ge")
                        nc.scalar.activation(ge[:], gh[:, 0:S], A.Gelu,
                                             bias=bias("b1")[:, mI:mI + 1])
                        mT = mp.tile([128, S], st, tag="mT", name="mT")
                        nc.vector.scalar_tensor_tensor(mT[:], gh[:, S:2 * S],
                                                       bias("b3")[:, mI:mI + 1],
                                                       ge[:], OP.add, OP.mult)
                        # fused down-projection: w2 already resident
                        for t in range(QC):
                            for n in range(2):
                                mm(y_ps[t][n][:], mT[:, t * 128:(t + 1) * 128],
                                   w2_all[:, mI * D + n * 512:
                                          mI * D + (n + 1) * 512],
                                   mI == 0, mI == FC - 1)
                for t in range(QC):
                    for n in range(2):
                        o = op_.tile([128, 512], f32, tag="o_out", name="o_out")
                        nc.vector.tensor_scalar_mul(o[:], y_ps[t][n][:],
                                                    bias("c")[:, 0:1])
                        eng_s.dma_start(
                            out_ffn.ap()[t * 128:(t + 1) * 128,
                                         n * 512:(n + 1) * 512], o[:])

    nc.compile()
    return nc


class _FixedPool:
    """Adapter letting project_fm write into pre-allocated tiles."""

    def __init__(self, tiles):
        self._tiles = list(tiles)
        self._i = 0

    def tile(self, shape, dtype, tag=None, name=None):
        t = self._tiles[self._i]
        self._i += 1
        return t


def _routing(langs):
    """Per-sequence expert slots [(expert_idx, coef) x2], matching the
    reference: coef[e,b] = any(langs[b]==4+e) * (1/count(langs[b]>3))."""
    langs = np.asarray(langs)
    slots = []
    for b in range(langs.shape[0]):
        row = [int(v) for v in langs[b]]
        cnt = sum(1 for v in row if v > 3)
        rw = 1.0 if cnt == 0 else 1.0 / cnt
        seen = []
        for v in row:
            if v > 3 and 0 <= v - 4 < NE and (v - 4) not in seen:
                seen.append(v - 4)
        sl = [(e, rw) for e in seen]
        while len(sl) < 2:
            sl.append((0, 0.0))
        slots.append(sl[:2])
    return slots


def _mask_classes(maskT, n_kc):
    """Classify each [128 keys x 128 queries] block of a transposed mask:
    0 all-zero (no add), 1 general (add), 2 fully masked (skip compute).
    Keeps at least one active key block per query and contiguous active
    ranges per key chunk."""
    cls = []
    for kc in range(n_kc):
        for qh in range(QC):
            blk = maskT[kc * 128:(kc + 1) * 128, qh * 128:(qh + 1) * 128]
            if np.all(blk == 0):
                cls.append(0)
            elif np.all(blk <= -1e8):
                cls.append(2)
            else:
                cls.append(1)
    for qh in range(QC):
        if all(cls[kc * QC + qh] == 2 for kc in range(n_kc)):
            for kc in range(n_kc):
                cls[kc * QC + qh] = 1
    for kc in range(n_kc):
        act = [q for q in range(QC) if cls[kc * QC + q] != 2]
        if not act or act != list(range(act[0], act[-1] + 1)):
            for q in range(QC):
                if cls[kc * QC + q] == 2:
                    cls[kc * QC + q] = 1
    return tuple(cls)


def kernel(**inputs):
    mode = MODE
    np_dt = ml_dtypes.bfloat16 if mode == "bf16" else np.float32
    f32 = np.float32

    inp = {k: np.asarray(v) for k, v in inputs.items()}
    x = inp["hidden_states"].astype(f32)
    enc = inp["encoder_hidden_states"].astype(f32)
    mask = inp["attention_mask"].astype(f32)
    encmask = inp["encoder_attention_mask"].astype(f32)
    g1, b1 = inp["ln1_g"].astype(f32), inp["ln1_b"].astype(f32)
    g2, b2 = inp["ln2_g"].astype(f32), inp["ln2_b"].astype(f32)
    g3, b3 = inp["ln3_g"].astype(f32), inp["ln3_b"].astype(f32)

    def dup_kv(w):
        return np.concatenate([np.tile(w[:, 64 * j:64 * (j + 1)], (1, 2))
                               for j in range(NKV)], axis=1)

    def dup_kv_b(v):
        return np.concatenate([np.tile(v[64 * j:64 * (j + 1)], 2)
                               for j in range(NKV)])

    sc = HD ** -0.5
    qw_f = g1[:, None] * inp["sa_q_w"] * sc
    qb_f = (b1 @ inp["sa_q_w"] + inp["sa_q_b"]) * sc
    kw_f = dup_kv(g1[:, None] * inp["sa_k_w"])
    kb_f = dup_kv_b(b1 @ inp["sa_k_w"] + inp["sa_k_b"])
    vw_f = g1[:, None] * inp["sa_v_w"]
    vb_f = b1 @ inp["sa_v_w"] + inp["sa_v_b"]
    q2w_f = g2[:, None] * inp["ca_q_w"] * sc
    q2b_f = (b2 @ inp["ca_q_w"] + inp["ca_q_b"]) * sc
    k2w_f = dup_kv(inp["ca_k_w"])
    k2b_f = dup_kv_b(inp["ca_k_b"])
    w1_f = inp["moe_w1"] * g3[None, :, None]
    b1_f = np.einsum("d,edf->ef", b3, inp["moe_w1"]).astype(f32)
    w3_f = inp["moe_w3"] * g3[None, :, None]
    b3_f = np.einsum("d,edf->ef", b3, inp["moe_w3"]).astype(f32)

    maskT0 = np.ascontiguousarray(mask[:, 0].transpose(0, 2, 1))     # [B,S,S]
    encmaskT0 = np.ascontiguousarray(encmask[:, 0].transpose(0, 2, 1))
    sa_cls = _mask_classes(maskT0[0], SC)
    ca_cls = _mask_classes(encmaskT0[0], KC)
    for b in range(1, B):
        if _mask_classes(maskT0[b], SC) != sa_cls or \
           _mask_classes(encmaskT0[b], KC) != ca_cls:
            sa_cls = tuple(1 for _ in range(SC * QC))
            ca_cls = tuple(1 for _ in range(KC * QC))
            break

    key = (mode, sa_cls, ca_cls)
    if key not in _CACHE:
        _CACHE[key] = _build(mode, sa_cls, ca_cls)
    nc = _CACHE[key]

    def col128(v):
        return np.asarray(v, f32).reshape(-1, 128).T

    def pack_k(w):
        w = np.asarray(w)
        return np.concatenate([w[k * 128:(k + 1) * 128, :]
                               for k in range(w.shape[0] // 128)], axis=1)

    slots = _routing(inp["langs"])
    wqkv = np.concatenate([qw_f, kw_f, vw_f], axis=1).astype(np_dt)
    wca = np.concatenate([q2w_f, k2w_f, inp["ca_v_w"]], axis=1).astype(np_dt)

    bias_common = np.zeros((128, BIAS_W), f32)
    for nm, v in [("qb", col128(qb_f)), ("kb", col128(kb_f)),
                  ("vb", np.broadcast_to(vb_f.astype(f32), (128, KVW))),
                  ("ob", col128(inp["sa_o_b"])),
                  ("q2b", col128(q2b_f)), ("k2b", col128(k2b_f)),
                  ("v2b", np.broadcast_to(inp["ca_v_b"].astype(f32), (128, KVW))),
                  ("o2b", col128(inp["ca_o_b"]))]:
        off, w = _BIAS_COLS[nm]
        bias_common[:, off:off + w] = v

    in_maps = []
    for c in range(8):
        b, j = c // 2, c % 2
        e, coef = slots[b][j]
        xT = pack_k(np.ascontiguousarray(x[b].T))
        # interleave w1/w3 by group: [w1 grp g | w3 grp g] blocks of 512 cols
        gw = GRP * 128
        w13 = np.empty((128, NGRP * DC * 1024), f32)
        for g in range(NGRP):
            for k in range(DC):
                c0 = (g * DC + k) * 1024
                w13[:, c0:c0 + 512] = w1_f[e][k * 128:(k + 1) * 128,
                                              g * 512:(g + 1) * 512]
                w13[:, c0 + 512:c0 + 1024] = w3_f[e][k * 128:(k + 1) * 128,
                                                     g * 512:(g + 1) * 512]
        bt = bias_common.copy()
        for nm, v in [("b1", col128(b1_f[e])), ("b3", col128(b3_f[e]))]:
            off, w = _BIAS_COLS[nm]
            bt[:, off:off + w] = v
        bt[:, _BIAS_COLS["c"][0]] = coef
        m = {
            "xT": xT,
            "encT": pack_k(np.ascontiguousarray(enc[b].T)).astype(np_dt),
            "id128": np.eye(128, dtype=f32).astype(np_dt),
            "ones_col": np.ones((128, 1), f32).astype(np_dt),
            "ones_row": np.ones((1, 128), f32).astype(np_dt),
            "wqkv": pack_k(wqkv), "wca": pack_k(wca),
            "ow": pack_k(inp["sa_o_w"].astype(np_dt)),
            "o2w": pack_k(inp["ca_o_w"].astype(np_dt)),
            "biases": bt,
            "w13": w13.astype(np_dt),
            "w2": pack_k(np.ascontiguousarray(inp["moe_w2"][e])).astype(np_dt),
        }
        if mode != "f32":
            m["xT_st"] = xT.astype(np_dt)
        if any(cc == 1 for cc in sa_cls):
            m["maskT"] = np.exp(maskT0[b]).astype(np_dt)
        if any(cc == 1 for cc in ca_cls):
            m["encmaskT"] = np.exp(encmaskT0[b]).astype(np_dt)
        in_maps.append(m)

    kw = {}
    if _TRACE_DIR:
        kw = dict(trace=True, tmpdir=_TRACE_DIR, trace_cores=[0])
    res = bass_utils.run_bass_kernel_spmd(nc, in_maps, core_ids=list(range(8)), **kw)
    global _LAST_EXEC_NS
    _LAST_EXEC_NS = res.exec_time_ns
    return np.stack([
        res.results[2 * b]["out_res"].T
        + res.results[2 * b]["out_ffn"]
        + res.results[2 * b + 1]["out_ffn"]
        for b in range(B)
    ]).astype(f32)



# revision 22
# speedup vs baseline: 1.0771x; 1.0071x over previous
"""MBart MoE decoder layer on 8 trn2 NeuronCores.

Sharding: 8 cores = 8 (sequence, expert-slot) pairs. Core c handles
sequence b=c//2, expert slot j=c%2 (each sequence is lang-routed to at
most 2 distinct experts; routing is computed on the host from `langs`).
Each core computes the full attention path for its sequence (replicated
across the pair) and one expert FFN over all 256 tokens; the host sums
the pair's partial outputs (expert-sharded combine) and transposes back
to token-major. Expert weights are gathered per-core on the host, so a
core only receives the one expert it needs.

On-device layout is feature-major [D, tokens]: projections take weights
as lhsT (feature-major out) or activations as lhsT (token-major out), so
no activation transposes are needed anywhere. LN gains/biases are folded
into the downstream weights on the host; softmax uses transposed scores
[keys, queries] with the attention mask added via an identity-matmul
into PSUM (host classifies each 128x128 mask block as zero / add / skip,
so causal dead blocks are never computed) and denominators accumulated
via a ones-matmul, then broadcast over partitions with a rank-1 matmul
for one full-lane reciprocal per head pair.
"""

import os
import sys
from contextlib import ExitStack

for _p in ("/opt/trn_rl_repo",):
    if _p not in sys.path:
        sys.path.append(_p)

import numpy as np
import ml_dtypes

import concourse.bass as bass
import concourse.tile as tile
import concourse.mybir as mybir
from concourse import bacc, bass_utils

B, S, SK = 4, 256, 512
D, NH, NKV, HD = 1024, 16, 4, 64
DE, NE = 4096, 8
LN_EPS = 1e-5
REP = NH // NKV
DC = D // 128    # 8 feature chunks
FC = DE // 128   # 32 ffn chunks
SC = S // 128    # 2 self-attn key chunks
KC = SK // 128   # 4 cross-attn key chunks
QC = S // 128    # 2 query halves
KVW = NKV * HD   # 256
GRP = 4          # ffn chunks per MoE weight group
NGRP = FC // GRP

MODE = os.environ.get("KERNEL_MM_DTYPE", "bf16")  # "bf16" | "f32r" | "f32"

_CACHE: dict = {}
_TRACE_DIR = None   # set by test harness for profiling runs
_LAST_EXEC_NS = None

# packed attention-weight column layout: qw | kw(dup) | vw
W_Q, W_K, W_V = 0, D, D + 2 * KVW
WPACK = D + 2 * KVW + KVW  # 1792

# packed per-partition bias column layout
_BIAS_COLS = {}
_off = 0
for _n, _w in [("qb", DC), ("kb", 4), ("vb", KVW), ("ob", DC),
               ("q2b", DC), ("k2b", 4), ("v2b", KVW), ("o2b", DC),
               ("b1", FC), ("b3", FC), ("c", 1)]:
    _BIAS_COLS[_n] = (_off, _w)
    _off += _w
BIAS_W = _off


def _build(mode, sa_cls, ca_cls):
    """sa_cls/ca_cls: block classes per (kc, qhalf): 0=no-mask, 1=mask-add,
    2=fully-masked(skip)."""
    st = {"bf16": mybir.dt.bfloat16, "f32r": mybir.dt.float32r,
          "f32": mybir.dt.float32}[mode]
    f32 = mybir.dt.float32
    same_st = mode == "f32"
    A = mybir.ActivationFunctionType
    OP = mybir.AluOpType

    nc = bacc.Bacc("TRN2", target_bir_lowering=False, debug=False, num_devices=8)
    import os as _os
    _SPLIT = _os.environ.get("KERNEL_DMA_SPLIT", "1") == "1"
    eng_b = nc.scalar if _SPLIT else nc.sync
    eng_s = nc.gpsimd if _SPLIT else nc.sync

    def mm(psum, lhsT, rhs, start, stop):
        nc.tensor.matmul(psum, lhsT, rhs, start=start, stop=stop)

    di = {}

    def din(name, shape, dtype=None):
        di[name] = nc.dram_tensor(name, list(shape), dtype or st, kind="ExternalInput")
        return di[name]

    din("xT", (128, DC * S), f32)
    if not same_st:
        din("xT_st", (128, DC * S))
    din("encT", (128, DC * SK))
    need_samask = any(c == 1 for c in sa_cls)
    need_camask = any(c == 1 for c in ca_cls)
    if need_samask:
        din("maskT", (S, S))
    if need_camask:
        din("encmaskT", (SK, S))
    din("id128", (128, 128))
    din("onehot", (128, 8 * 128))
    din("ones_col", (128, 1))
    din("ones_row", (1, 128))
    din("wqkv", (128, DC * WPACK))
    din("wca", (128, DC * WPACK))
    din("ow", (128, DC * D))
    din("o2w", (128, DC * D))
    din("biases", (128, BIAS_W), f32)
    din("w13", (128, NGRP * DC * 1024))  # per grp, per k: [w1 512 | w3 512]
    din("w2", (128, FC * D))
    out_res = nc.dram_tensor("out_res", [D, S], f32, kind="ExternalOutput")
    out_ffn = nc.dram_tensor("out_ffn", [S, D], f32, kind="ExternalOutput")

    with tile.TileContext(nc) as tc, ExitStack() as ctx:
        cp = ctx.enter_context(tc.tile_pool(name="consts", bufs=1))
        pers = ctx.enter_context(tc.tile_pool(name="pers", bufs=1))

        ones128 = cp.tile([128, 1], st, tag="ones128", name="ones128")
        eng_b.dma_start(ones128[:], di["ones_col"].ap())
        ones1r = cp.tile([1, 128], st, tag="ones1r", name="ones1r")
        eng_b.dma_start(ones1r[:], di["ones_row"].ap())
        eps_t = cp.tile([128, 1], f32, tag="eps_t", name="eps_t")
        nc.vector.memset(eps_t, LN_EPS)
        id128 = cp.tile([128, 128], st, tag="id128", name="id128")
        eng_b.dma_start(id128[:], di["id128"].ap())
        onehot = cp.tile([128, 8 * 128], st, tag="onehot", name="onehot")
        eng_b.dma_start(onehot[:], di["onehot"].ap())
        maskT = encmaskT = None
        if need_samask:
            maskT = cp.tile([128, SC, S], st, tag="maskT", name="maskT")
            for kc in range(SC):
                eng_b.dma_start(maskT[:, kc, :],
                                  di["maskT"].ap()[kc * 128:(kc + 1) * 128, :])
        if need_camask:
            encmaskT = cp.tile([128, KC, S], st, tag="encmaskT", name="encmaskT")
            for kc in range(KC):
                eng_b.dma_start(encmaskT[:, kc, :],
                                  di["encmaskT"].ap()[kc * 128:(kc + 1) * 128, :])

        bias_t = cp.tile([128, BIAS_W], f32, tag="bias_t", name="bias_t")
        eng_b.dma_start(bias_t[:], di["biases"].ap())

        def bias(nm):
            off, w = _BIAS_COLS[nm]
            return bias_t[:, off:off + w]

        def load_chunks(dram, nchunk, width, tag, pool, dtype=st, engine=None):
            t = pool.tile([128, nchunk * width], dtype, tag=tag, name=tag)
            (engine or nc.sync).dma_start(t[:], dram.ap())
            return [t[:, k * width:(k + 1) * width] for k in range(nchunk)]

        def layernorm(src_f32, src_st, out_tag, pool):
            """src: DC chunks [128,S] f32 (+st copies). Returns DC normalized
            chunks [128,S] st (gain/bias folded downstream by host)."""
            with tc.tile_pool(name=f"{out_tag}_lt", bufs=2) as lp, \
                 tc.tile_pool(name=f"{out_tag}_lp", bufs=1, space="PSUM") as sp, \
                 tc.tile_pool(name=f"{out_tag}_lb", bufs=1, space="PSUM") as bp:
                sum_ps = sp.tile([1, S], f32, tag="lnsum", name="lnsum")
                sq_ps = sp.tile([1, S], f32, tag="lnsq", name="lnsq")
                for k in range(DC):
                    sq = lp.tile([128, S], st, tag="lnsqt", name="lnsqt")
                    nc.vector.tensor_tensor(sq[:], src_f32[k][:], src_f32[k][:],
                                            OP.mult)
                    mm(sum_ps[:], ones128[:], src_st[k][:], k == 0, k == DC - 1)
                    mm(sq_ps[:], ones128[:], sq[:], k == 0, k == DC - 1)
                s_sb = lp.tile([1, S], st, tag="ln_ssb", name="ln_ssb")
                nc.vector.tensor_single_scalar(s_sb[:], sum_ps[:], 1.0 / D, OP.mult)
                q_sb = lp.tile([1, S], st, tag="ln_qsb", name="ln_qsb")
                nc.vector.tensor_single_scalar(q_sb[:], sq_ps[:], 1.0 / D, OP.mult)
                s_bc = bp.tile([128, S], f32, tag="ln_sbc", name="ln_sbc")
                q_bc = bp.tile([128, S], f32, tag="ln_qbc", name="ln_qbc")
                mm(s_bc[:], ones1r[:], s_sb[:], True, True)   # mean, bcast
                mm(q_bc[:], ones1r[:], q_sb[:], True, True)   # E[x^2], bcast
                # full-lane stats math on [128,S]
                s_sbuf = lp.tile([128, S], f32, tag="ln_ssbuf", name="ln_ssbuf")
                nc.vector.tensor_copy(s_sbuf[:], s_bc[:])
                var = lp.tile([128, S], f32, tag="ln_var", name="ln_var")
                nc.vector.scalar_tensor_tensor(var[:], s_bc[:], 0.0, s_sbuf[:],
                                               OP.bypass, OP.mult)
                nc.vector.tensor_sub(var[:], q_bc[:], var[:])
                v_t = lp.tile([128, S], f32, tag="ln_vt", name="ln_vt")
                nc.scalar.activation(v_t[:], var[:], A.Abs_reciprocal_sqrt,
                                     bias=eps_t[:])
                u_t = lp.tile([128, S], f32, tag="ln_ut", name="ln_ut")
                nc.vector.tensor_tensor(u_t[:], s_sbuf[:], v_t[:], OP.mult)
                outs = []
                for k in range(DC):
                    o = pool.tile([128, S], st, tag=f"{out_tag}{k}",
                                  name=f"{out_tag}{k}")
                    nc.vector.tensor_tensor(o[:], src_f32[k][:], v_t[:], OP.mult)
                    nc.vector.tensor_sub(o[:], o[:], u_t[:])
                    outs.append(o)
                return outs

        def cast_st(src, tag, pool):
            if same_st:
                return src
            outs = []
            for k, t in enumerate(src):
                o = pool.tile([128, t.shape[-1]], st, tag=f"{tag}{k}",
                              name=f"{tag}{k}")
                nc.vector.tensor_copy(o[:], t[:])
                outs.append(o)
            return outs

        def project_fm(w_slices, rhs_chunks, nout, bias_ap, out_tag, pool,
                       extra=None, out_dt=None, width=None):
            """out^T[dout_chunk] = sum_k w_slices[k][:, m*128:...].T @ rhs[k]."""
            W = width or S
            outs = []
            with tc.tile_pool(name=f"{out_tag}_ps", bufs=3, space="PSUM") as pp:
                for mI in range(nout):
                    ps = pp.tile([128, W], f32, tag="proj", name="proj")
                    for k in range(DC):
                        mm(ps[:], w_slices[k][:, mI * 128:(mI + 1) * 128],
                           rhs_chunks[k][:], k == 0, k == DC - 1)
                    o = pool.tile([128, W], out_dt or st, tag=f"{out_tag}{mI}",
                                  name=f"{out_tag}{mI}")
                    if extra is not None:
                        extra(mI, ps, o)
                    elif bias_ap is not None:
                        nc.vector.tensor_scalar(o[:], ps[:],
                                                bias_ap[:, mI:mI + 1], None,
                                                OP.add)
                    else:
                        nc.vector.tensor_copy(o[:], ps[:])
                    outs.append(o)
            return outs

        def project_tm(act_chunks, w_slices, ntok, bias_bcast, out_tag, pool):
            """token-major V with a ones column appended per kv head:
            out[tok_chunk] = [V_kv | 1] blocks of 65 columns."""
            outs = []
            with tc.tile_pool(name=f"{out_tag}_ps", bufs=3, space="PSUM") as pp:
                for t in range(ntok):
                    ps = pp.tile([128, KVW], f32, tag="projtm", name="projtm")
                    for k in range(DC):
                        mm(ps[:], act_chunks[k][:, t * 128:(t + 1) * 128],
                           w_slices[k][:], k == 0, k == DC - 1)
                    o = pool.tile([128, NKV, HD + 1], st, tag=f"{out_tag}{t}",
                                  name=f"{out_tag}{t}")
                    nc.vector.tensor_add(
                        o[:, :, 0:HD],
                        ps[:].rearrange("p (kv d) -> p kv d", kv=NKV),
                        bias_bcast[:].rearrange("p (kv d) -> p kv d", kv=NKV))
                    for kv in range(NKV):
                        nc.vector.tensor_copy(o[:, kv, HD:HD + 1], ones128[:])
                    outs.append(o)
            return outs

        def attend(qT, kT, vtm, n_kc, mask_tile, cls, out_tag, pool):
            """Baseline-structure attention; mask applied multiplicatively on
            the DVE (host ships exp(mask)) instead of via id128 matmuls."""
            outs = []
            qr = []
            for kc in range(n_kc):
                act = [qh for qh in range(QC) if cls[kc * QC + qh] != 2]
                assert act and act == list(range(act[0], act[-1] + 1))
                qr.append((act[0] * 128, (act[-1] + 1) * 128))
            with tc.tile_pool(name=f"{out_tag}_sp", bufs=3, space="PSUM") as stp, \
                 tc.tile_pool(name=f"{out_tag}_op", bufs=2, space="PSUM") as opp, \
                 tc.tile_pool(name=f"{out_tag}_bp", bufs=1, space="PSUM") as bpp, \
                 tc.tile_pool(name=f"{out_tag}_et", bufs=6) as epool, \
                 tc.tile_pool(name=f"{out_tag}_un", bufs=1) as upool, \
                 tc.tile_pool(name=f"{out_tag}_dt", bufs=1) as dpool:
                # denominators staged at 32-aligned partitions: tile i holds
                # head pairs 4i..4i+3 at rows {0,32,64,96}
                den_t = [dpool.tile([128, 2 * S], f32, tag=f"den_t{i}",
                                    name=f"den_t{i}") for i in range(2)]
                for i in range(2):   # unwritten rows must stay finite
                    nc.vector.memset(den_t[i], 1.0)
                o_un = []
                for c in range(DC):
                    o_ps_h = [opp.tile([65, S], f32, tag=f"oph{hh}",
                                       name=f"oph{hh}") for hh in range(2)]
                    kv = (2 * c) // REP      # same kv head for both of the pair
                    for kc in range(n_kc):
                        q0, q1 = qr[kc]
                        adds = [q for q in range(QC) if cls[kc * QC + q] == 1]
                        st_h = []
                        e_h = []
                        for hh in range(2):
                            qh_ap = qT[c][hh * 64:(hh + 1) * 64, :]
                            kh = kT[kv][hh * 64:(hh + 1) * 64, :]
                            st_ps = stp.tile([128, S], f32, tag="st",
                                             name="st")
                            mm(st_ps[:, q0:q1], kh[:, kc * 128:(kc + 1) * 128],
                               qh_ap[:, q0:q1], True, True)
                            st_h.append(st_ps)
                        for hh in range(2):
                            e = epool.tile([128, S], st, tag="e", name="e")
                            nc.scalar.activation(e[:, q0:q1],
                                                 st_h[hh][:, q0:q1], A.Exp)
                            for q in adds:
                                nc.vector.tensor_tensor(
                                    e[:, q * 128:(q + 1) * 128],
                                    e[:, q * 128:(q + 1) * 128],
                                    mask_tile[:, kc, q * 128:(q + 1) * 128],
                                    OP.mult)
                            e_h.append(e)
                        for hh in range(2):
                            mm(o_ps_h[hh][:, q0:q1],
                               vtm[kc][:, kv, :], e_h[hh][:, q0:q1],
                               kc == 0, kc == n_kc - 1)
                    # evacuate unnormalized O + denominators; frees PSUM fast
                    ou = upool.tile([128, S], st, tag=f"un{c}", name=f"un{c}")
                    row = 32 * (c % 4)
                    for hh in range(2):
                        nc.vector.tensor_copy(ou[hh * 64:(hh + 1) * 64, :],
                                              o_ps_h[hh][0:64, :])
                        nc.vector.tensor_copy(
                            den_t[c // 4][row:row + 1, hh * S:(hh + 1) * S],
                            o_ps_h[hh][64:65, :])
                    o_un.append(ou)
                # ONE rsqrt pass for the whole attention (no ACT table thrash)
                den_sq = []
                for i in range(2):
                    dr = dpool.tile([128, 2 * S], f32, tag=f"den_r{i}",
                                    name=f"den_r{i}")
                    nc.scalar.activation(dr[:], den_t[i][:],
                                         A.Abs_reciprocal_sqrt)
                    ds = dpool.tile([128, 2 * S], st, tag=f"den_sq{i}",
                                    name=f"den_sq{i}")
                    nc.vector.tensor_tensor(ds[:], dr[:], dr[:], OP.mult)
                    den_sq.append(ds)
                for c in range(DC):
                    r_ps = bpp.tile([128, 2 * S], f32, tag="rbc", name="rbc")
                    mm(r_ps[:], onehot[:, c * 128:(c + 1) * 128],
                       den_sq[c // 4][:], True, True)
                    o = pool.tile([128, S], st, tag=f"{out_tag}{c}",
                                  name=f"{out_tag}{c}")
                    for hh in range(2):
                        nc.vector.tensor_tensor(
                            o[hh * 64:(hh + 1) * 64, :],
                            o_un[c][hh * 64:(hh + 1) * 64, :],
                            r_ps[hh * 64:(hh + 1) * 64, hh * S:(hh + 1) * S],
                            OP.mult)
                    outs.append(o)
            return outs

        h1 = [pers.tile([128, S], f32, tag=f"h1T{k}", name=f"h1T{k}")
              for k in range(DC)]
        h2 = [pers.tile([128, S], f32, tag=f"h2T{k}", name=f"h2T{k}")
              for k in range(DC)]

        # w2 fully resident before the MoE starts (4 sliced DMAs on the
        # store queue, issued up front; overlaps the attention phases)
        w2_all = pers.tile([128, FC * D], st, tag="w2_all", name="w2_all")
        for q in range(4):
            w = FC * D // 4
            eng_s.dma_start(w2_all[:, q * w:(q + 1) * w],
                            di["w2"].ap()[:, q * w:(q + 1) * w])

        # ---------------- self attention ----------------
        with tc.tile_pool(name="sa_acts", bufs=1) as sa:
            xT = load_chunks(di["xT"], DC, S, "xT", sa, f32)
            xT_st = xT if same_st else load_chunks(di["xT_st"], DC, S, "xTs", sa)
            n1 = layernorm(xT, xT_st, "n1T", sa)
            with tc.tile_pool(name="wqkvp", bufs=1) as wp:
                wt = load_chunks(di["wqkv"], DC, WPACK, "wqkv", wp)
                qT = project_fm([t[:, W_Q:W_Q + D] for t in wt], n1, DC,
                                bias("qb"), "qT", sa)
                kT = project_fm([t[:, W_K:W_K + 2 * KVW] for t in wt], n1, 4,
                                bias("kb"), "kT", sa)
                v_tm = project_tm(n1, [t[:, W_V:W_V + KVW] for t in wt], SC,
                                  bias("vb"), "v_tm", sa)
            with tc.tile_pool(name="wop", bufs=1) as wp:
                ow_t = load_chunks(di["ow"], DC, D, "ow", wp)
                sa_out = attend(qT, kT, v_tm, SC, maskT, sa_cls, "saT", sa)

                def o_epil(mI, ps, o):
                    nc.vector.scalar_tensor_tensor(o[:], ps[:],
                                                   bias("ob")[:, mI:mI + 1],
                                                   xT[mI][:], OP.add, OP.add)
                project_fm(ow_t, sa_out, DC, None, "h1w", _FixedPool(h1),
                           extra=o_epil, out_dt=f32)

        # ---------------- cross attention ----------------
        with tc.tile_pool(name="ca_acts", bufs=1) as ca:
            encT = load_chunks(di["encT"], DC, SK, "encT", ca, engine=eng_b)
            h1_st = cast_st(h1, "h1s", ca)
            with tc.tile_pool(name="wcap", bufs=1) as wp:
                wt = load_chunks(di["wca"], DC, WPACK, "wca", wp, engine=eng_b)
                k2T = project_fm([t[:, W_K:W_K + 2 * KVW] for t in wt], encT, 4,
                                 bias("k2b"), "k2T", ca, width=SK)
                v2_tm = project_tm(encT, [t[:, W_V:W_V + KVW] for t in wt], KC,
                                   bias("v2b"), "v2_tm", ca)
                n2 = layernorm(h1, h1_st, "n2T", ca)
                q2T = project_fm([t[:, W_Q:W_Q + D] for t in wt], n2, DC,
                                 bias("q2b"), "q2T", ca)
            with tc.tile_pool(name="wo2p", bufs=1) as wp:
                o2w_t = load_chunks(di["o2w"], DC, D, "o2w", wp, engine=eng_b)
                ca_out = attend(q2T, k2T, v2_tm, KC, encmaskT, ca_cls, "caT", ca)

                def o2_epil(mI, ps, o):
                    nc.vector.scalar_tensor_tensor(o[:], ps[:],
                                                   bias("o2b")[:, mI:mI + 1],
                                                   h1[mI][:], OP.add, OP.add)
                project_fm(o2w_t, ca_out, DC, None, "h2w", _FixedPool(h2),
                           extra=o2_epil, out_dt=f32)

        # residual output (host: out_b = res.T + ffn_j0 + ffn_j1)
        for k in range(DC):
            eng_s.dma_start(out_res.ap()[k * 128:(k + 1) * 128, :], h2[k][:])

        # ---------------- MoE expert ----------------
        with tc.tile_pool(name="moe_acts", bufs=1) as mo:
            h2_st = cast_st(h2, "h2s", mo)
            n3 = layernorm(h2, h2_st, "n3T", mo)

            with tc.tile_pool(name="w13p", bufs=3) as wp, \
                 tc.tile_pool(name="mTp", bufs=6) as mp, \
                 tc.tile_pool(name="gh_ps", bufs=3, space="PSUM") as gp, \
                 tc.tile_pool(name="y_ps", bufs=1, space="PSUM") as yp, \
                 tc.tile_pool(name="gelu_t", bufs=3) as gt, \
                 tc.tile_pool(name="outp", bufs=2) as op_:
                y_ps = [[yp.tile([128, 512], f32, tag=f"y{t}{n}", name=f"y{t}{n}")
                         for n in range(2)] for t in range(QC)]
                gw = GRP * 128
                GW = DC * 1024
                for g in range(NGRP):
                    wgt = wp.tile([128, GW], st, tag="w13g", name="w13g")
                    nc.sync.dma_start(wgt[:], di["w13"].ap()[:, g * GW:(g + 1) * GW])
                    wg = [wgt[:, k * 1024:(k + 1) * 1024] for k in range(DC)]
                    for mi in range(GRP):
                        mI = g * GRP + mi
                        # one PSUM bank: gelu-arg in [0:S], mult-arg in [S:2S]
                        gh = gp.tile([128, 2 * S], f32, tag="gh", name="gh")
                        for k in range(DC):
                            mm(gh[:, 0:S], wg[k][:, mi * 128:(mi + 1) * 128],
                               n3[k][:], k == 0, k == DC - 1)
                        for k in range(DC):
                            mm(gh[:, S:2 * S],
                               wg[k][:, gw + mi * 128:gw + (mi + 1) * 128],
                               n3[k][:], k == 0, k == DC - 1)
                        ge = gt.tile([128, S], f32, tag="ge", name="ge")
                        nc.scalar.activation(ge[:], gh[:, 0:S], A.Gelu,
                                             bias=bias("b1")[:, mI:mI + 1])
                        mT = mp.tile([128, S], st, tag="mT", name="mT")
                        nc.vector.scalar_tensor_tensor(mT[:], gh[:, S:2 * S],
                                                       bias("b3")[:, mI:mI + 1],
                                                       ge[:], OP.add, OP.mult)
                        # fused down-projection: w2 already resident
                        for t in range(QC):
                            for n in range(2):
                                mm(y_ps[t][n][:], mT[:, t * 128:(t + 1) * 128],
                                   w2_all[:, mI * D + n * 512:
                                          mI * D + (n + 1) * 512],
                                   mI == 0, mI == FC - 1)
                for t in range(QC):
                    for n in range(2):
                        o = op_.tile([128, 512], f32, tag="o_out", name="o_out")
                        nc.vector.tensor_scalar_mul(o[:], y_ps[t][n][:],
                                                    bias("c")[:, 0:1])
                        eng_s.dma_start(
                            out_ffn.ap()[t * 128:(t + 1) * 128,
                                         n * 512:(n + 1) * 512], o[:])

    nc.compile()
    return nc


class _FixedPool:
    """Adapter letting project_fm write into pre-allocated tiles."""

    def __init__(self, tiles):
        self._tiles = list(tiles)
        self._i = 0

    def tile(self, shape, dtype, tag=None, name=None):
        t = self._tiles[self._i]
        self._i += 1
        return t


def _routing(langs):
    """Per-sequence expert slots [(expert_idx, coef) x2], matching the
    reference: coef[e,b] = any(langs[b]==4+e) * (1/count(langs[b]>3))."""
    langs = np.asarray(langs)
    slots = []
    for b in range(langs.shape[0]):
        row = [int(v) for v in langs[b]]
        cnt = sum(1 for v in row if v > 3)
        rw = 1.0 if cnt == 0 else 1.0 / cnt
        seen = []
        for v in row:
            if v > 3 and 0 <= v - 4 < NE and (v - 4) not in seen:
                seen.append(v - 4)
        sl = [(e, rw) for e in seen]
        while len(sl) < 2:
            sl.append((0, 0.0))
        slots.append(sl[:2])
    return slots


def _mask_classes(maskT, n_kc):
    """Classify each [128 keys x 128 queries] block of a transposed mask:
    0 all-zero (no add), 1 general (add), 2 fully masked (skip compute).
    Keeps at least one active key block per query and contiguous active
    ranges per key chunk."""
    cls = []
    for kc in range(n_kc):
        for qh in range(QC):
            blk = maskT[kc * 128:(kc + 1) * 128, qh * 128:(qh + 1) * 128]
            if np.all(blk == 0):
                cls.append(0)
            elif np.all(blk <= -1e8):
                cls.append(2)
            else:
                cls.append(1)
    for qh in range(QC):
        if all(cls[kc * QC + qh] == 2 for kc in range(n_kc)):
            for kc in range(n_kc):
                cls[kc * QC + qh] = 1
    for kc in range(n_kc):
        act = [q for q in range(QC) if cls[kc * QC + q] != 2]
        if not act or act != list(range(act[0], act[-1] + 1)):
            for q in range(QC):
                if cls[kc * QC + q] == 2:
                    cls[kc * QC + q] = 1
    return tuple(cls)


def kernel(**inputs):
    mode = MODE
    np_dt = ml_dtypes.bfloat16 if mode == "bf16" else np.float32
    f32 = np.float32

    inp = {k: np.asarray(v) for k, v in inputs.items()}
    x = inp["hidden_states"].astype(f32)
    enc = inp["encoder_hidden_states"].astype(f32)
    mask = inp["attention_mask"].astype(f32)
    encmask = inp["encoder_attention_mask"].astype(f32)
    g1, b1 = inp["ln1_g"].astype(f32), inp["ln1_b"].astype(f32)
    g2, b2 = inp["ln2_g"].astype(f32), inp["ln2_b"].astype(f32)
    g3, b3 = inp["ln3_g"].astype(f32), inp["ln3_b"].astype(f32)

    def dup_kv(w):
        return np.concatenate([np.tile(w[:, 64 * j:64 * (j + 1)], (1, 2))
                               for j in range(NKV)], axis=1)

    def dup_kv_b(v):
        return np.concatenate([np.tile(v[64 * j:64 * (j + 1)], 2)
                               for j in range(NKV)])

    sc = HD ** -0.5
    qw_f = g1[:, None] * inp["sa_q_w"] * sc
    qb_f = (b1 @ inp["sa_q_w"] + inp["sa_q_b"]) * sc
    kw_f = dup_kv(g1[:, None] * inp["sa_k_w"])
    kb_f = dup_kv_b(b1 @ inp["sa_k_w"] + inp["sa_k_b"])
    vw_f = g1[:, None] * inp["sa_v_w"]
    vb_f = b1 @ inp["sa_v_w"] + inp["sa_v_b"]
    q2w_f = g2[:, None] * inp["ca_q_w"] * sc
    q2b_f = (b2 @ inp["ca_q_w"] + inp["ca_q_b"]) * sc
    k2w_f = dup_kv(inp["ca_k_w"])
    k2b_f = dup_kv_b(inp["ca_k_b"])
    w1_f = inp["moe_w1"] * g3[None, :, None]
    b1_f = np.einsum("d,edf->ef", b3, inp["moe_w1"]).astype(f32)
    w3_f = inp["moe_w3"] * g3[None, :, None]
    b3_f = np.einsum("d,edf->ef", b3, inp["moe_w3"]).astype(f32)

    maskT0 = np.ascontiguousarray(mask[:, 0].transpose(0, 2, 1))     # [B,S,S]
    encmaskT0 = np.ascontiguousarray(encmask[:, 0].transpose(0, 2, 1))
    sa_cls = _mask_classes(maskT0[0], SC)
    ca_cls = _mask_classes(encmaskT0[0], KC)
    for b in range(1, B):
        if _mask_classes(maskT0[b], SC) != sa_cls or \
           _mask_classes(encmaskT0[b], KC) != ca_cls:
            sa_cls = tuple(1 for _ in range(SC * QC))
            ca_cls = tuple(1 for _ in range(KC * QC))
            break

    key = (mode, sa_cls, ca_cls)
    if key not in _CACHE:
        _CACHE[key] = _build(mode, sa_cls, ca_cls)
    nc = _CACHE[key]

    def col128(v):
        return np.asarray(v, f32).reshape(-1, 128).T

    def pack_k(w):
        w = np.asarray(w)
        return np.concatenate([w[k * 128:(k + 1) * 128, :]
                               for k in range(w.shape[0] // 128)], axis=1)

    slots = _routing(inp["langs"])
    # block c selects den row 32*(c%4) of den tile c//4
    onehot = np.zeros((128, 8 * 128), f32)
    for c in range(8):
        onehot[32 * (c % 4), c * 128:(c + 1) * 128] = 1.0
    wqkv = np.concatenate([qw_f, kw_f, vw_f], axis=1).astype(np_dt)
    wca = np.concatenate([q2w_f, k2w_f, inp["ca_v_w"]], axis=1).astype(np_dt)

    bias_common = np.zeros((128, BIAS_W), f32)
    for nm, v in [("qb", col128(qb_f)), ("kb", col128(kb_f)),
                  ("vb", np.broadcast_to(vb_f.astype(f32), (128, KVW))),
                  ("ob", col128(inp["sa_o_b"])),
                  ("q2b", col128(q2b_f)), ("k2b", col128(k2b_f)),
                  ("v2b", np.broadcast_to(inp["ca_v_b"].astype(f32), (128, KVW))),
                  ("o2b", col128(inp["ca_o_b"]))]:
        off, w = _BIAS_COLS[nm]
        bias_common[:, off:off + w] = v

    in_maps = []
    for c in range(8):
        b, j = c // 2, c % 2
        e, coef = slots[b][j]
        xT = pack_k(np.ascontiguousarray(x[b].T))
        # interleave w1/w3 by group: [w1 grp g | w3 grp g] blocks of 512 cols
        gw = GRP * 128
        w13 = np.empty((128, NGRP * DC * 1024), f32)
        for g in range(NGRP):
            for k in range(DC):
                c0 = (g * DC + k) * 1024
                w13[:, c0:c0 + 512] = w1_f[e][k * 128:(k + 1) * 128,
                                              g * 512:(g + 1) * 512]
                w13[:, c0 + 512:c0 + 1024] = w3_f[e][k * 128:(k + 1) * 128,
                                                     g * 512:(g + 1) * 512]
        bt = bias_common.copy()
        for nm, v in [("b1", col128(b1_f[e])), ("b3", col128(b3_f[e]))]:
            off, w = _BIAS_COLS[nm]
            bt[:, off:off + w] = v
        bt[:, _BIAS_COLS["c"][0]] = coef
        m = {
            "xT": xT,
            "encT": pack_k(np.ascontiguousarray(enc[b].T)).astype(np_dt),
            "id128": np.eye(128, dtype=f32).astype(np_dt),
            "onehot": onehot.astype(np_dt),
            "ones_col": np.ones((128, 1), f32).astype(np_dt),
            "ones_row": np.ones((1, 128), f32).astype(np_dt),
            "wqkv": pack_k(wqkv), "wca": pack_k(wca),
            "ow": pack_k(inp["sa_o_w"].astype(np_dt)),
            "o2w": pack_k(inp["ca_o_w"].astype(np_dt)),
            "biases": bt,
            "w13": w13.astype(np_dt),
            "w2": pack_k(np.ascontiguousarray(inp["moe_w2"][e])).astype(np_dt),
        }
        if mode != "f32":
            m["xT_st"] = xT.astype(np_dt)
        if any(cc == 1 for cc in sa_cls):
            m["maskT"] = np.exp(maskT0[b]).astype(np_dt)
        if any(cc == 1 for cc in ca_cls):
            m["encmaskT"] = np.exp(encmaskT0[b]).astype(np_dt)
        in_maps.append(m)

    kw = {}
    if _TRACE_DIR:
        kw = dict(trace=True, tmpdir=_TRACE_DIR, trace_cores=[0])
    res = bass_utils.run_bass_kernel_spmd(nc, in_maps, core_ids=list(range(8)), **kw)
    global _LAST_EXEC_NS
    _LAST_EXEC_NS = res.exec_time_ns
    return np.stack([
        res.results[2 * b]["out_res"].T
        + res.results[2 * b]["out_ffn"]
        + res.results[2 * b + 1]["out_ffn"]
        for b in range(B)
    ]).astype(f32)



# revision 26
# speedup vs baseline: 1.1261x; 1.0455x over previous
"""MBart MoE decoder layer on 8 trn2 NeuronCores.

Sharding: 8 cores = 8 (sequence, expert-slot) pairs. Core c handles
sequence b=c//2, expert slot j=c%2 (each sequence is lang-routed to at
most 2 distinct experts; routing is computed on the host from `langs`).
Each core computes the full attention path for its sequence (replicated
across the pair) and one expert FFN over all 256 tokens; the host sums
the pair's partial outputs (expert-sharded combine) and transposes back
to token-major. Expert weights are gathered per-core on the host, so a
core only receives the one expert it needs.

On-device layout is feature-major [D, tokens]: projections take weights
as lhsT (feature-major out) or activations as lhsT (token-major out), so
no activation transposes are needed anywhere. LN gains/biases are folded
into the downstream weights on the host; softmax uses transposed scores
[keys, queries] with the attention mask added via an identity-matmul
into PSUM (host classifies each 128x128 mask block as zero / add / skip,
so causal dead blocks are never computed) and denominators accumulated
via a ones-matmul, then broadcast over partitions with a rank-1 matmul
for one full-lane reciprocal per head pair.
"""

import os
import sys
from contextlib import ExitStack

for _p in ("/opt/trn_rl_repo",):
    if _p not in sys.path:
        sys.path.append(_p)

import numpy as np
import ml_dtypes

import concourse.bass as bass
import concourse.tile as tile
import concourse.mybir as mybir
from concourse import bacc, bass_utils

B, S, SK = 4, 256, 512
D, NH, NKV, HD = 1024, 16, 4, 64
DE, NE = 4096, 8
LN_EPS = 1e-5
REP = NH // NKV
DC = D // 128    # 8 feature chunks
FC = DE // 128   # 32 ffn chunks
SC = S // 128    # 2 self-attn key chunks
KC = SK // 128   # 4 cross-attn key chunks
QC = S // 128    # 2 query halves
KVW = NKV * HD   # 256
GRP = 4          # ffn chunks per MoE weight group
NGRP = FC // GRP

MODE = os.environ.get("KERNEL_MM_DTYPE", "bf16")  # "bf16" | "f32r" | "f32"

_CACHE: dict = {}
_TRACE_DIR = None   # set by test harness for profiling runs
_LAST_EXEC_NS = None

# packed attention-weight column layout: qw | kw(dup) | vw
W_Q, W_K, W_V = 0, D, D + 2 * KVW
WPACK = D + 2 * KVW + KVW  # 1792

# packed per-partition bias column layout
_BIAS_COLS = {}
_off = 0
for _n, _w in [("qb", DC), ("kb", 4), ("vb", KVW), ("ob", DC),
               ("q2b", DC), ("k2b", 4), ("v2b", KVW), ("o2b", DC),
               ("b1", FC), ("b3", FC), ("c", 1)]:
    _BIAS_COLS[_n] = (_off, _w)
    _off += _w
BIAS_W = _off


def _build(mode, sa_cls, ca_cls):
    """sa_cls/ca_cls: block classes per (kc, qhalf): 0=no-mask, 1=mask-add,
    2=fully-masked(skip)."""
    st = {"bf16": mybir.dt.bfloat16, "f32r": mybir.dt.float32r,
          "f32": mybir.dt.float32}[mode]
    f32 = mybir.dt.float32
    same_st = mode == "f32"
    A = mybir.ActivationFunctionType
    OP = mybir.AluOpType

    nc = bacc.Bacc("TRN2", target_bir_lowering=False, debug=False, num_devices=8)
    import os as _os
    _SPLIT = _os.environ.get("KERNEL_DMA_SPLIT", "1") == "1"
    eng_b = nc.scalar if _SPLIT else nc.sync
    eng_s = nc.gpsimd if _SPLIT else nc.sync

    def mm(psum, lhsT, rhs, start, stop):
        nc.tensor.matmul(psum, lhsT, rhs, start=start, stop=stop)

    di = {}

    def din(name, shape, dtype=None):
        di[name] = nc.dram_tensor(name, list(shape), dtype or st, kind="ExternalInput")
        return di[name]

    din("xT", (128, DC * S), f32)
    if not same_st:
        din("xT_st", (128, DC * S))
    din("encT", (128, DC * SK))
    need_samask = any(c == 1 for c in sa_cls)
    need_camask = any(c == 1 for c in ca_cls)
    if need_samask:
        din("maskT", (S, S))
    if need_camask:
        din("encmaskT", (SK, S))
    din("id128", (128, 128))
    din("onehot", (128, 8 * 128))
    din("ones_col", (128, 1))
    din("ones_row", (1, 128))
    din("wqkv", (128, DC * WPACK))
    din("wca", (128, DC * WPACK))
    din("ow", (128, DC * D))
    din("o2w", (128, DC * D))
    din("biases", (128, BIAS_W), f32)
    din("w13", (128, NGRP * DC * 1024))  # per grp, per k: [w1 512 | w3 512]
    din("w2", (128, FC * D))
    out_res = nc.dram_tensor("out_res", [D, S], f32, kind="ExternalOutput")
    out_ffn = nc.dram_tensor("out_ffn", [S, D], f32, kind="ExternalOutput")

    with tile.TileContext(nc) as tc, ExitStack() as ctx:
        cp = ctx.enter_context(tc.tile_pool(name="consts", bufs=1))
        pers = ctx.enter_context(tc.tile_pool(name="pers", bufs=1))

        ones128 = cp.tile([128, 1], st, tag="ones128", name="ones128")
        eng_b.dma_start(ones128[:], di["ones_col"].ap())
        ones1r = cp.tile([1, 128], st, tag="ones1r", name="ones1r")
        eng_b.dma_start(ones1r[:], di["ones_row"].ap())
        eps_t = cp.tile([128, 1], f32, tag="eps_t", name="eps_t")
        nc.vector.memset(eps_t, LN_EPS)
        id128 = cp.tile([128, 128], st, tag="id128", name="id128")
        eng_b.dma_start(id128[:], di["id128"].ap())
        onehot = cp.tile([128, 8 * 128], st, tag="onehot", name="onehot")
        eng_b.dma_start(onehot[:], di["onehot"].ap())
        maskT = encmaskT = None
        if need_samask:
            maskT = cp.tile([128, SC, S], st, tag="maskT", name="maskT")
            for kc in range(SC):
                eng_b.dma_start(maskT[:, kc, :],
                                  di["maskT"].ap()[kc * 128:(kc + 1) * 128, :])
        if need_camask:
            encmaskT = cp.tile([128, KC, S], st, tag="encmaskT", name="encmaskT")
            for kc in range(KC):
                eng_b.dma_start(encmaskT[:, kc, :],
                                  di["encmaskT"].ap()[kc * 128:(kc + 1) * 128, :])

        bias_t = cp.tile([128, BIAS_W], f32, tag="bias_t", name="bias_t")
        eng_b.dma_start(bias_t[:], di["biases"].ap())

        def bias(nm):
            off, w = _BIAS_COLS[nm]
            return bias_t[:, off:off + w]

        def load_chunks(dram, nchunk, width, tag, pool, dtype=st, engine=None,
                        after=None, inst_out=None):
            t = pool.tile([128, nchunk * width], dtype, tag=tag, name=tag)
            inst = (engine or nc.sync).dma_start(t[:], dram.ap())
            if after is not None:
                tile.add_dep_helper(inst.ins, after, sync=True,
                                    reason="dma priority order")
            if inst_out is not None:
                inst_out.append(inst.ins)
            return [t[:, k * width:(k + 1) * width] for k in range(nchunk)]

        def layernorm(src_f32, src_st, out_tag, pool):
            """src: DC chunks [128,S] f32 (+st copies). Returns DC normalized
            chunks [128,S] st (gain/bias folded downstream by host)."""
            with tc.tile_pool(name=f"{out_tag}_lt", bufs=2) as lp, \
                 tc.tile_pool(name=f"{out_tag}_lp", bufs=1, space="PSUM") as sp, \
                 tc.tile_pool(name=f"{out_tag}_lb", bufs=1, space="PSUM") as bp:
                sum_ps = sp.tile([1, S], f32, tag="lnsum", name="lnsum")
                sq_ps = sp.tile([1, S], f32, tag="lnsq", name="lnsq")
                for k in range(DC):
                    sq = lp.tile([128, S], st, tag="lnsqt", name="lnsqt")
                    nc.vector.tensor_tensor(sq[:], src_st[k][:], src_st[k][:],
                                            OP.mult)
                    mm(sum_ps[:], ones128[:], src_st[k][:], k == 0, k == DC - 1)
                    mm(sq_ps[:], ones128[:], sq[:], k == 0, k == DC - 1)
                s_sb = lp.tile([1, S], st, tag="ln_ssb", name="ln_ssb")
                nc.vector.tensor_single_scalar(s_sb[:], sum_ps[:], 1.0 / D, OP.mult)
                q_sb = lp.tile([1, S], st, tag="ln_qsb", name="ln_qsb")
                nc.vector.tensor_single_scalar(q_sb[:], sq_ps[:], 1.0 / D, OP.mult)
                s_bc = bp.tile([128, S], f32, tag="ln_sbc", name="ln_sbc")
                q_bc = bp.tile([128, S], f32, tag="ln_qbc", name="ln_qbc")
                mm(s_bc[:], ones1r[:], s_sb[:], True, True)   # mean, bcast
                mm(q_bc[:], ones1r[:], q_sb[:], True, True)   # E[x^2], bcast
                # var = E[x^2] - mean^2; rstd/mean*rstd in st so the
                # per-chunk normalize runs in the DVE 16-bit fast mode
                m2 = lp.tile([128, S], f32, tag="ln_m2", name="ln_m2")
                nc.scalar.activation(m2[:], s_bc[:], A.Square)
                var = lp.tile([128, S], f32, tag="ln_var", name="ln_var")
                nc.vector.tensor_sub(var[:], q_bc[:], m2[:])
                v_t = lp.tile([128, S], st, tag="ln_vt", name="ln_vt")
                nc.scalar.activation(v_t[:], var[:], A.Abs_reciprocal_sqrt,
                                     bias=eps_t[:])
                u_t = lp.tile([128, S], st, tag="ln_ut", name="ln_ut")
                nc.vector.tensor_tensor(u_t[:], s_bc[:], v_t[:], OP.mult)
                outs = []
                for k in range(DC):
                    o = pool.tile([128, S], st, tag=f"{out_tag}{k}",
                                  name=f"{out_tag}{k}")
                    nc.vector.tensor_tensor(o[:], src_st[k][:], v_t[:], OP.mult)
                    nc.vector.tensor_sub(o[:], o[:], u_t[:])
                    outs.append(o)
                return outs

        def cast_st(src, tag, pool):
            if same_st:
                return src
            outs = []
            for k, t in enumerate(src):
                o = pool.tile([128, t.shape[-1]], st, tag=f"{tag}{k}",
                              name=f"{tag}{k}")
                nc.vector.tensor_copy(o[:], t[:])
                outs.append(o)
            return outs

        def project_fm(w_slices, rhs_chunks, nout, bias_ap, out_tag, pool,
                       extra=None, out_dt=None, width=None):
            """out^T[dout_chunk] = sum_k w_slices[k][:, m*128:...].T @ rhs[k]."""
            W = width or S
            outs = []
            with tc.tile_pool(name=f"{out_tag}_ps", bufs=3, space="PSUM") as pp:
                for mI in range(nout):
                    ps = pp.tile([128, W], f32, tag="proj", name="proj")
                    for k in range(DC):
                        mm(ps[:], w_slices[k][:, mI * 128:(mI + 1) * 128],
                           rhs_chunks[k][:], k == 0, k == DC - 1)
                    o = pool.tile([128, W], out_dt or st, tag=f"{out_tag}{mI}",
                                  name=f"{out_tag}{mI}")
                    if extra is not None:
                        extra(mI, ps, o)
                    elif bias_ap is not None:
                        nc.vector.tensor_scalar(o[:], ps[:],
                                                bias_ap[:, mI:mI + 1], None,
                                                OP.add)
                    else:
                        nc.vector.tensor_copy(o[:], ps[:])
                    outs.append(o)
            return outs

        def project_tm(act_chunks, w_slices, ntok, bias_bcast, out_tag, pool):
            """token-major V with a ones column appended per kv head:
            out[tok_chunk] = [V_kv | 1] blocks of 65 columns."""
            outs = []
            with tc.tile_pool(name=f"{out_tag}_ps", bufs=3, space="PSUM") as pp:
                for t in range(ntok):
                    ps = pp.tile([128, KVW], f32, tag="projtm", name="projtm")
                    for k in range(DC):
                        mm(ps[:], act_chunks[k][:, t * 128:(t + 1) * 128],
                           w_slices[k][:], k == 0, k == DC - 1)
                    o = pool.tile([128, NKV, HD + 1], st, tag=f"{out_tag}{t}",
                                  name=f"{out_tag}{t}")
                    nc.vector.tensor_add(
                        o[:, :, 0:HD],
                        ps[:].rearrange("p (kv d) -> p kv d", kv=NKV),
                        bias_bcast[:].rearrange("p (kv d) -> p kv d", kv=NKV))
                    for kv in range(NKV):
                        nc.vector.tensor_copy(o[:, kv, HD:HD + 1], ones128[:])
                    outs.append(o)
            return outs

        def attend(qT, kT, vtm, n_kc, mask_tile, cls, out_tag, pool):
            """Baseline-structure attention; mask applied multiplicatively on
            the DVE (host ships exp(mask)) instead of via id128 matmuls."""
            outs = []
            qr = []
            for kc in range(n_kc):
                act = [qh for qh in range(QC) if cls[kc * QC + qh] != 2]
                assert act and act == list(range(act[0], act[-1] + 1))
                qr.append((act[0] * 128, (act[-1] + 1) * 128))
            with tc.tile_pool(name=f"{out_tag}_sp", bufs=3, space="PSUM") as stp, \
                 tc.tile_pool(name=f"{out_tag}_op", bufs=2, space="PSUM") as opp, \
                 tc.tile_pool(name=f"{out_tag}_bp", bufs=1, space="PSUM") as bpp, \
                 tc.tile_pool(name=f"{out_tag}_et", bufs=6) as epool, \
                 tc.tile_pool(name=f"{out_tag}_un", bufs=1) as upool, \
                 tc.tile_pool(name=f"{out_tag}_dt", bufs=1) as dpool:
                # denominators staged at 32-aligned partitions: tile i holds
                # head pairs 4i..4i+3 at rows {0,32,64,96}
                den_t = [dpool.tile([128, 2 * S], f32, tag=f"den_t{i}",
                                    name=f"den_t{i}") for i in range(2)]
                for i in range(2):   # unwritten rows must stay finite
                    nc.vector.memset(den_t[i], 1.0)
                o_un = []
                for c in range(DC):
                    o_ps_h = [opp.tile([65, S], f32, tag=f"oph{hh}",
                                       name=f"oph{hh}") for hh in range(2)]
                    kv = (2 * c) // REP      # same kv head for both of the pair
                    for kc in range(n_kc):
                        q0, q1 = qr[kc]
                        adds = [q for q in range(QC) if cls[kc * QC + q] == 1]
                        st_h = []
                        e_h = []
                        for hh in range(2):
                            qh_ap = qT[c][hh * 64:(hh + 1) * 64, :]
                            kh = kT[kv][hh * 64:(hh + 1) * 64, :]
                            st_ps = stp.tile([128, S], f32, tag="st",
                                             name="st")
                            mm(st_ps[:, q0:q1], kh[:, kc * 128:(kc + 1) * 128],
                               qh_ap[:, q0:q1], True, True)
                            st_h.append(st_ps)
                        for hh in range(2):
                            e = epool.tile([128, S], st, tag="e", name="e")
                            nc.scalar.activation(e[:, q0:q1],
                                                 st_h[hh][:, q0:q1], A.Exp)
                            for q in adds:
                                nc.vector.tensor_tensor(
                                    e[:, q * 128:(q + 1) * 128],
                                    e[:, q * 128:(q + 1) * 128],
                                    mask_tile[:, kc, q * 128:(q + 1) * 128],
                                    OP.mult)
                            e_h.append(e)
                        for hh in range(2):
                            mm(o_ps_h[hh][:, q0:q1],
                               vtm[kc][:, kv, :], e_h[hh][:, q0:q1],
                               kc == 0, kc == n_kc - 1)
                    # evacuate unnormalized O + denominators; frees PSUM fast
                    ou = upool.tile([128, S], st, tag=f"un{c}", name=f"un{c}")
                    row = 32 * (c % 4)
                    for hh in range(2):
                        nc.vector.tensor_copy(ou[hh * 64:(hh + 1) * 64, :],
                                              o_ps_h[hh][0:64, :])
                        nc.vector.tensor_copy(
                            den_t[c // 4][row:row + 1, hh * S:(hh + 1) * S],
                            o_ps_h[hh][64:65, :])
                    o_un.append(ou)
                # ONE rsqrt pass for the whole attention (no ACT table thrash)
                den_sq = []
                for i in range(2):
                    dr = dpool.tile([128, 2 * S], f32, tag=f"den_r{i}",
                                    name=f"den_r{i}")
                    nc.scalar.activation(dr[:], den_t[i][:],
                                         A.Abs_reciprocal_sqrt)
                    ds = dpool.tile([128, 2 * S], st, tag=f"den_sq{i}",
                                    name=f"den_sq{i}")
                    nc.vector.tensor_tensor(ds[:], dr[:], dr[:], OP.mult)
                    den_sq.append(ds)
                for c in range(DC):
                    r_ps = bpp.tile([128, 2 * S], f32, tag="rbc", name="rbc")
                    mm(r_ps[:], onehot[:, c * 128:(c + 1) * 128],
                       den_sq[c // 4][:], True, True)
                    o = pool.tile([128, S], st, tag=f"{out_tag}{c}",
                                  name=f"{out_tag}{c}")
                    for hh in range(2):
                        nc.vector.tensor_tensor(
                            o[hh * 64:(hh + 1) * 64, :],
                            o_un[c][hh * 64:(hh + 1) * 64, :],
                            r_ps[hh * 64:(hh + 1) * 64, hh * S:(hh + 1) * S],
                            OP.mult)
                    outs.append(o)
            return outs

        h1 = [pers.tile([128, S], f32, tag=f"h1T{k}", name=f"h1T{k}")
              for k in range(DC)]
        h2 = [pers.tile([128, S], f32, tag=f"h2T{k}", name=f"h2T{k}")
              for k in range(DC)]

        # w2 fully resident before the MoE starts (4 sliced DMAs on the
        # store queue); gated behind the attention weight loads so the
        # prefetch never starves first-needed transfers
        w2_all = pers.tile([128, FC * D], st, tag="w2_all", name="w2_all")
        w2_gate = []
        enc_gate = []
        wca_i = []

        def _issue_w2():
            for q in range(4):
                w = FC * D // 4
                inst = eng_s.dma_start(w2_all[:, q * w:(q + 1) * w],
                                       di["w2"].ap()[:, q * w:(q + 1) * w])
                if q == 0 and wca_i:
                    tile.add_dep_helper(inst.ins, wca_i[0], sync=True,
                                        reason="w2 prefetch after attn weights")

        # ---------------- self attention ----------------
        with tc.tile_pool(name="sa_acts", bufs=1) as sa:
            xT = load_chunks(di["xT"], DC, S, "xT", sa, f32)
            xT_st = xT if same_st else load_chunks(di["xT_st"], DC, S, "xTs", sa)
            n1 = layernorm(xT, xT_st, "n1T", sa)
            with tc.tile_pool(name="wqkvp", bufs=1) as wp:
                wqkv_i = []
                wt = load_chunks(di["wqkv"], DC, WPACK, "wqkv", wp,
                                 inst_out=wqkv_i)
                enc_gate.extend(wqkv_i)
                qT = project_fm([t[:, W_Q:W_Q + D] for t in wt], n1, DC,
                                bias("qb"), "qT", sa)
                kT = project_fm([t[:, W_K:W_K + 2 * KVW] for t in wt], n1, 4,
                                bias("kb"), "kT", sa)
                v_tm = project_tm(n1, [t[:, W_V:W_V + KVW] for t in wt], SC,
                                  bias("vb"), "v_tm", sa)
            with tc.tile_pool(name="wop", bufs=1) as wp:
                ow_t = load_chunks(di["ow"], DC, D, "ow", wp)
                sa_out = attend(qT, kT, v_tm, SC, maskT, sa_cls, "saT", sa)

                def o_epil(mI, ps, o):
                    nc.vector.scalar_tensor_tensor(o[:], ps[:],
                                                   bias("ob")[:, mI:mI + 1],
                                                   xT[mI][:], OP.add, OP.add)
                project_fm(ow_t, sa_out, DC, None, "h1w", _FixedPool(h1),
                           extra=o_epil, out_dt=f32)

        # ---------------- cross attention ----------------
        with tc.tile_pool(name="ca_acts", bufs=1) as ca:
            encT = load_chunks(di["encT"], DC, SK, "encT", ca, engine=eng_b,
                               after=enc_gate[0])
            h1_st = cast_st(h1, "h1s", ca)
            with tc.tile_pool(name="wcap", bufs=1) as wp:
                wt = load_chunks(di["wca"], DC, WPACK, "wca", wp, engine=eng_b,
                                 inst_out=wca_i)
                _issue_w2()
                k2T = project_fm([t[:, W_K:W_K + 2 * KVW] for t in wt], encT, 4,
                                 bias("k2b"), "k2T", ca, width=SK)
                v2_tm = project_tm(encT, [t[:, W_V:W_V + KVW] for t in wt], KC,
                                   bias("v2b"), "v2_tm", ca)
                n2 = layernorm(h1, h1_st, "n2T", ca)
                q2T = project_fm([t[:, W_Q:W_Q + D] for t in wt], n2, DC,
                                 bias("q2b"), "q2T", ca)
            with tc.tile_pool(name="wo2p", bufs=1) as wp:
                o2w_t = load_chunks(di["o2w"], DC, D, "o2w", wp, engine=eng_b)
                ca_out = attend(q2T, k2T, v2_tm, KC, encmaskT, ca_cls, "caT", ca)

                def o2_epil(mI, ps, o):
                    nc.vector.scalar_tensor_tensor(o[:], ps[:],
                                                   bias("o2b")[:, mI:mI + 1],
                                                   h1[mI][:], OP.add, OP.add)
                project_fm(o2w_t, ca_out, DC, None, "h2w", _FixedPool(h2),
                           extra=o2_epil, out_dt=f32)

        # residual output (host: out_b = res.T + ffn_j0 + ffn_j1)
        for k in range(DC):
            eng_s.dma_start(out_res.ap()[k * 128:(k + 1) * 128, :], h2[k][:])

        # ---------------- MoE expert ----------------
        with tc.tile_pool(name="moe_acts", bufs=1) as mo:
            h2_st = cast_st(h2, "h2s", mo)
            n3 = layernorm(h2, h2_st, "n3T", mo)

            with tc.tile_pool(name="w13p", bufs=3) as wp, \
                 tc.tile_pool(name="mTp", bufs=6) as mp, \
                 tc.tile_pool(name="gh_ps", bufs=3, space="PSUM") as gp, \
                 tc.tile_pool(name="y_ps", bufs=1, space="PSUM") as yp, \
                 tc.tile_pool(name="gelu_t", bufs=3) as gt, \
                 tc.tile_pool(name="outp", bufs=2) as op_:
                y_ps = [[yp.tile([128, 512], f32, tag=f"y{t}{n}", name=f"y{t}{n}")
                         for n in range(2)] for t in range(QC)]
                gw = GRP * 128
                GW = DC * 1024
                for g in range(NGRP):
                    wgt = wp.tile([128, GW], st, tag="w13g", name="w13g")
                    nc.sync.dma_start(wgt[:], di["w13"].ap()[:, g * GW:(g + 1) * GW])
                    wg = [wgt[:, k * 1024:(k + 1) * 1024] for k in range(DC)]
                    for mi in range(GRP):
                        mI = g * GRP + mi
                        # one PSUM bank: gelu-arg in [0:S], mult-arg in [S:2S]
                        gh = gp.tile([128, 2 * S], f32, tag="gh", name="gh")
                        for k in range(DC):
                            mm(gh[:, 0:S], wg[k][:, mi * 128:(mi + 1) * 128],
                               n3[k][:], k == 0, k == DC - 1)
                        for k in range(DC):
                            mm(gh[:, S:2 * S],
                               wg[k][:, gw + mi * 128:gw + (mi + 1) * 128],
                               n3[k][:], k == 0, k == DC - 1)
                        ge = gt.tile([128, S], f32, tag="ge", name="ge")
                        nc.scalar.activation(ge[:], gh[:, 0:S], A.Gelu,
                                             bias=bias("b1")[:, mI:mI + 1])
                        mT = mp.tile([128, S], st, tag="mT", name="mT")
                        nc.vector.scalar_tensor_tensor(mT[:], gh[:, S:2 * S],
                                                       bias("b3")[:, mI:mI + 1],
                                                       ge[:], OP.add, OP.mult)
                        # fused down-projection: w2 already resident
                        for t in range(QC):
                            for n in range(2):
                                mm(y_ps[t][n][:], mT[:, t * 128:(t + 1) * 128],
                                   w2_all[:, mI * D + n * 512:
                                          mI * D + (n + 1) * 512],
                                   mI == 0, mI == FC - 1)
                for t in range(QC):
                    for n in range(2):
                        o = op_.tile([128, 512], f32, tag="o_out", name="o_out")
                        nc.vector.tensor_scalar_mul(o[:], y_ps[t][n][:],
                                                    bias("c")[:, 0:1])
                        eng_s.dma_start(
                            out_ffn.ap()[t * 128:(t + 1) * 128,
                                         n * 512:(n + 1) * 512], o[:])

    nc.compile()
    return nc


class _FixedPool:
    """Adapter letting project_fm write into pre-allocated tiles."""

    def __init__(self, tiles):
        self._tiles = list(tiles)
        self._i = 0

    def tile(self, shape, dtype, tag=None, name=None):
        t = self._tiles[self._i]
        self._i += 1
        return t


def _routing(langs):
    """Per-sequence expert slots [(expert_idx, coef) x2], matching the
    reference: coef[e,b] = any(langs[b]==4+e) * (1/count(langs[b]>3))."""
    langs = np.asarray(langs)
    slots = []
    for b in range(langs.shape[0]):
        row = [int(v) for v in langs[b]]
        cnt = sum(1 for v in row if v > 3)
        rw = 1.0 if cnt == 0 else 1.0 / cnt
        seen = []
        for v in row:
            if v > 3 and 0 <= v - 4 < NE and (v - 4) not in seen:
                seen.append(v - 4)
        sl = [(e, rw) for e in seen]
        while len(sl) < 2:
            sl.append((0, 0.0))
        slots.append(sl[:2])
    return slots


def _mask_classes(maskT, n_kc):
    """Classify each [128 keys x 128 queries] block of a transposed mask:
    0 all-zero (no add), 1 general (add), 2 fully masked (skip compute).
    Keeps at least one active key block per query and contiguous active
    ranges per key chunk."""
    cls = []
    for kc in range(n_kc):
        for qh in range(QC):
            blk = maskT[kc * 128:(kc + 1) * 128, qh * 128:(qh + 1) * 128]
            if np.all(blk == 0):
                cls.append(0)
            elif np.all(blk <= -1e8):
                cls.append(2)
            else:
                cls.append(1)
    for qh in range(QC):
        if all(cls[kc * QC + qh] == 2 for kc in range(n_kc)):
            for kc in range(n_kc):
                cls[kc * QC + qh] = 1
    for kc in range(n_kc):
        act = [q for q in range(QC) if cls[kc * QC + q] != 2]
        if not act or act != list(range(act[0], act[-1] + 1)):
            for q in range(QC):
                if cls[kc * QC + q] == 2:
                    cls[kc * QC + q] = 1
    return tuple(cls)


def kernel(**inputs):
    mode = MODE
    np_dt = ml_dtypes.bfloat16 if mode == "bf16" else np.float32
    f32 = np.float32

    inp = {k: np.asarray(v) for k, v in inputs.items()}
    x = inp["hidden_states"].astype(f32)
    enc = inp["encoder_hidden_states"].astype(f32)
    mask = inp["attention_mask"].astype(f32)
    encmask = inp["encoder_attention_mask"].astype(f32)
    g1, b1 = inp["ln1_g"].astype(f32), inp["ln1_b"].astype(f32)
    g2, b2 = inp["ln2_g"].astype(f32), inp["ln2_b"].astype(f32)
    g3, b3 = inp["ln3_g"].astype(f32), inp["ln3_b"].astype(f32)

    def dup_kv(w):
        return np.concatenate([np.tile(w[:, 64 * j:64 * (j + 1)], (1, 2))
                               for j in range(NKV)], axis=1)

    def dup_kv_b(v):
        return np.concatenate([np.tile(v[64 * j:64 * (j + 1)], 2)
                               for j in range(NKV)])

    sc = HD ** -0.5
    qw_f = g1[:, None] * inp["sa_q_w"] * sc
    qb_f = (b1 @ inp["sa_q_w"] + inp["sa_q_b"]) * sc
    kw_f = dup_kv(g1[:, None] * inp["sa_k_w"])
    kb_f = dup_kv_b(b1 @ inp["sa_k_w"] + inp["sa_k_b"])
    vw_f = g1[:, None] * inp["sa_v_w"]
    vb_f = b1 @ inp["sa_v_w"] + inp["sa_v_b"]
    q2w_f = g2[:, None] * inp["ca_q_w"] * sc
    q2b_f = (b2 @ inp["ca_q_w"] + inp["ca_q_b"]) * sc
    k2w_f = dup_kv(inp["ca_k_w"])
    k2b_f = dup_kv_b(inp["ca_k_b"])
    w1_f = inp["moe_w1"] * g3[None, :, None]
    b1_f = np.einsum("d,edf->ef", b3, inp["moe_w1"]).astype(f32)
    w3_f = inp["moe_w3"] * g3[None, :, None]
    b3_f = np.einsum("d,edf->ef", b3, inp["moe_w3"]).astype(f32)

    maskT0 = np.ascontiguousarray(mask[:, 0].transpose(0, 2, 1))     # [B,S,S]
    encmaskT0 = np.ascontiguousarray(encmask[:, 0].transpose(0, 2, 1))
    sa_cls = _mask_classes(maskT0[0], SC)
    ca_cls = _mask_classes(encmaskT0[0], KC)
    for b in range(1, B):
        if _mask_classes(maskT0[b], SC) != sa_cls or \
           _mask_classes(encmaskT0[b], KC) != ca_cls:
            sa_cls = tuple(1 for _ in range(SC * QC))
            ca_cls = tuple(1 for _ in range(KC * QC))
            break

    key = (mode, sa_cls, ca_cls)
    if key not in _CACHE:
        _CACHE[key] = _build(mode, sa_cls, ca_cls)
    nc = _CACHE[key]

    def col128(v):
        return np.asarray(v, f32).reshape(-1, 128).T

    def pack_k(w):
        w = np.asarray(w)
        return np.concatenate([w[k * 128:(k + 1) * 128, :]
                               for k in range(w.shape[0] // 128)], axis=1)

    slots = _routing(inp["langs"])
    # block c selects den row 32*(c%4) of den tile c//4
    onehot = np.zeros((128, 8 * 128), f32)
    for c in range(8):
        onehot[32 * (c % 4), c * 128:(c + 1) * 128] = 1.0
    wqkv = np.concatenate([qw_f, kw_f, vw_f], axis=1).astype(np_dt)
    wca = np.concatenate([q2w_f, k2w_f, inp["ca_v_w"]], axis=1).astype(np_dt)

    bias_common = np.zeros((128, BIAS_W), f32)
    for nm, v in [("qb", col128(qb_f)), ("kb", col128(kb_f)),
                  ("vb", np.broadcast_to(vb_f.astype(f32), (128, KVW))),
                  ("ob", col128(inp["sa_o_b"])),
                  ("q2b", col128(q2b_f)), ("k2b", col128(k2b_f)),
                  ("v2b", np.broadcast_to(inp["ca_v_b"].astype(f32), (128, KVW))),
                  ("o2b", col128(inp["ca_o_b"]))]:
        off, w = _BIAS_COLS[nm]
        bias_common[:, off:off + w] = v

    in_maps = []
    for c in range(8):
        b, j = c // 2, c % 2
        e, coef = slots[b][j]
        xT = pack_k(np.ascontiguousarray(x[b].T))
        # interleave w1/w3 by group: [w1 grp g | w3 grp g] blocks of 512 cols
        gw = GRP * 128
        w13 = np.empty((128, NGRP * DC * 1024), f32)
        for g in range(NGRP):
            for k in range(DC):
                c0 = (g * DC + k) * 1024
                w13[:, c0:c0 + 512] = w1_f[e][k * 128:(k + 1) * 128,
                                              g * 512:(g + 1) * 512]
                w13[:, c0 + 512:c0 + 1024] = w3_f[e][k * 128:(k + 1) * 128,
                                                     g * 512:(g + 1) * 512]
        bt = bias_common.copy()
        for nm, v in [("b1", col128(b1_f[e])), ("b3", col128(b3_f[e]))]:
            off, w = _BIAS_COLS[nm]
            bt[:, off:off + w] = v
        bt[:, _BIAS_COLS["c"][0]] = coef
        m = {
            "xT": xT,
            "encT": pack_k(np.ascontiguousarray(enc[b].T)).astype(np_dt),
            "id128": np.eye(128, dtype=f32).astype(np_dt),
            "onehot": onehot.astype(np_dt),
            "ones_col": np.ones((128, 1), f32).astype(np_dt),
            "ones_row": np.ones((1, 128), f32).astype(np_dt),
            "wqkv": pack_k(wqkv), "wca": pack_k(wca),
            "ow": pack_k(inp["sa_o_w"].astype(np_dt)),
            "o2w": pack_k(inp["ca_o_w"].astype(np_dt)),
            "biases": bt,
            "w13": w13.astype(np_dt),
            "w2": pack_k(np.ascontiguousarray(inp["moe_w2"][e])).astype(np_dt),
        }
        if mode != "f32":
            m["xT_st"] = xT.astype(np_dt)
        if any(cc == 1 for cc in sa_cls):
            m["maskT"] = np.exp(maskT0[b]).astype(np_dt)
        if any(cc == 1 for cc in ca_cls):
            m["encmaskT"] = np.exp(encmaskT0[b]).astype(np_dt)
        in_maps.append(m)

    kw = {}
    if _TRACE_DIR:
        kw = dict(trace=True, tmpdir=_TRACE_DIR, trace_cores=[0])
    res = bass_utils.run_bass_kernel_spmd(nc, in_maps, core_ids=list(range(8)), **kw)
    global _LAST_EXEC_NS
    _LAST_EXEC_NS = res.exec_time_ns
    return np.stack([
        res.results[2 * b]["out_res"].T
        + res.results[2 * b]["out_ffn"]
        + res.results[2 * b + 1]["out_ffn"]
        for b in range(B)
    ]).astype(f32)



# revision 27
# speedup vs baseline: 1.1773x; 1.0455x over previous
"""MBart MoE decoder layer on 8 trn2 NeuronCores.

Sharding: 8 cores = 8 (sequence, expert-slot) pairs. Core c handles
sequence b=c//2, expert slot j=c%2 (each sequence is lang-routed to at
most 2 distinct experts; routing is computed on the host from `langs`).
Each core computes the full attention path for its sequence (replicated
across the pair) and one expert FFN over all 256 tokens; the host sums
the pair's partial outputs (expert-sharded combine) and transposes back
to token-major. Expert weights are gathered per-core on the host, so a
core only receives the one expert it needs.

On-device layout is feature-major [D, tokens]: projections take weights
as lhsT (feature-major out) or activations as lhsT (token-major out), so
no activation transposes are needed anywhere. LN gains/biases are folded
into the downstream weights on the host; softmax uses transposed scores
[keys, queries] with the attention mask added via an identity-matmul
into PSUM (host classifies each 128x128 mask block as zero / add / skip,
so causal dead blocks are never computed) and denominators accumulated
via a ones-matmul, then broadcast over partitions with a rank-1 matmul
for one full-lane reciprocal per head pair.
"""

import os
import sys
from contextlib import ExitStack

for _p in ("/opt/trn_rl_repo",):
    if _p not in sys.path:
        sys.path.append(_p)

import numpy as np
import ml_dtypes

import concourse.bass as bass
import concourse.tile as tile
import concourse.mybir as mybir
from concourse import bacc, bass_utils

B, S, SK = 4, 256, 512
D, NH, NKV, HD = 1024, 16, 4, 64
DE, NE = 4096, 8
LN_EPS = 1e-5
REP = NH // NKV
DC = D // 128    # 8 feature chunks
FC = DE // 128   # 32 ffn chunks
SC = S // 128    # 2 self-attn key chunks
KC = SK // 128   # 4 cross-attn key chunks
QC = S // 128    # 2 query halves
KVW = NKV * HD   # 256
GRP = 4          # ffn chunks per MoE weight group
NGRP = FC // GRP

MODE = os.environ.get("KERNEL_MM_DTYPE", "bf16")  # "bf16" | "f32r" | "f32"

_CACHE: dict = {}
_TRACE_DIR = None   # set by test harness for profiling runs
_LAST_EXEC_NS = None

# packed attention-weight column layout: qw | kw(dup) | vw
W_Q, W_K, W_V = 0, D, D + 2 * KVW
WPACK = D + 2 * KVW + KVW  # 1792

# packed per-partition bias column layout
_BIAS_COLS = {}
_off = 0
for _n, _w in [("qb", DC), ("kb", 4), ("vb", KVW), ("ob", DC),
               ("q2b", DC), ("k2b", 4), ("v2b", KVW), ("o2b", DC),
               ("b1", FC), ("b3", FC), ("c", 1)]:
    _BIAS_COLS[_n] = (_off, _w)
    _off += _w
BIAS_W = _off


def _build(mode, sa_cls, ca_cls):
    """sa_cls/ca_cls: block classes per (kc, qhalf): 0=no-mask, 1=mask-add,
    2=fully-masked(skip)."""
    st = {"bf16": mybir.dt.bfloat16, "f32r": mybir.dt.float32r,
          "f32": mybir.dt.float32}[mode]
    f32 = mybir.dt.float32
    same_st = mode == "f32"
    A = mybir.ActivationFunctionType
    OP = mybir.AluOpType

    nc = bacc.Bacc("TRN2", target_bir_lowering=False, debug=False, num_devices=8)
    import os as _os
    _SPLIT = _os.environ.get("KERNEL_DMA_SPLIT", "1") == "1"
    eng_b = nc.scalar if _SPLIT else nc.sync
    eng_s = nc.gpsimd if _SPLIT else nc.sync

    def mm(psum, lhsT, rhs, start, stop):
        nc.tensor.matmul(psum, lhsT, rhs, start=start, stop=stop)

    di = {}

    def din(name, shape, dtype=None):
        di[name] = nc.dram_tensor(name, list(shape), dtype or st, kind="ExternalInput")
        return di[name]

    din("xT", (128, DC * S), f32)
    if not same_st:
        din("xT_st", (128, DC * S))
    din("encT", (128, DC * SK))
    need_samask = any(c == 1 for c in sa_cls)
    need_camask = any(c == 1 for c in ca_cls)
    if need_samask:
        din("maskT", (S, S))
    if need_camask:
        din("encmaskT", (SK, S))
    din("id128", (128, 128))
    din("onehot", (128, 8 * 128))
    din("ones_col", (128, 1))
    din("ones_row", (1, 128))
    din("wqkv", (128, DC * WPACK))
    din("wca", (128, DC * WPACK))
    din("ow", (128, DC * D))
    din("o2w", (128, DC * D))
    din("biases", (128, BIAS_W), f32)
    din("w13", (128, NGRP * DC * 1024))  # per grp, per k: [w1 512 | w3 512]
    din("w2", (128, FC * D))
    out_res = nc.dram_tensor("out_res", [D, S], f32, kind="ExternalOutput")
    out_ffn = nc.dram_tensor("out_ffn", [S, D], f32, kind="ExternalOutput")

    with tile.TileContext(nc) as tc, ExitStack() as ctx:
        cp = ctx.enter_context(tc.tile_pool(name="consts", bufs=1))
        pers = ctx.enter_context(tc.tile_pool(name="pers", bufs=1))

        ones128 = cp.tile([128, 1], st, tag="ones128", name="ones128")
        eng_b.dma_start(ones128[:], di["ones_col"].ap())
        ones1r = cp.tile([1, 128], st, tag="ones1r", name="ones1r")
        eng_b.dma_start(ones1r[:], di["ones_row"].ap())
        eps_t = cp.tile([128, 1], f32, tag="eps_t", name="eps_t")
        nc.vector.memset(eps_t, LN_EPS)
        id128 = cp.tile([128, 128], st, tag="id128", name="id128")
        eng_b.dma_start(id128[:], di["id128"].ap())
        onehot = cp.tile([128, 8 * 128], st, tag="onehot", name="onehot")
        eng_b.dma_start(onehot[:], di["onehot"].ap())
        maskT = encmaskT = None
        if need_samask:
            maskT = cp.tile([128, SC, S], st, tag="maskT", name="maskT")
            for kc in range(SC):
                eng_b.dma_start(maskT[:, kc, :],
                                  di["maskT"].ap()[kc * 128:(kc + 1) * 128, :])
        if need_camask:
            encmaskT = cp.tile([128, KC, S], st, tag="encmaskT", name="encmaskT")
            for kc in range(KC):
                eng_b.dma_start(encmaskT[:, kc, :],
                                  di["encmaskT"].ap()[kc * 128:(kc + 1) * 128, :])

        bias_t = cp.tile([128, BIAS_W], f32, tag="bias_t", name="bias_t")
        eng_b.dma_start(bias_t[:], di["biases"].ap())

        def bias(nm):
            off, w = _BIAS_COLS[nm]
            return bias_t[:, off:off + w]

        def load_chunks(dram, nchunk, width, tag, pool, dtype=st, engine=None,
                        after=None, inst_out=None):
            t = pool.tile([128, nchunk * width], dtype, tag=tag, name=tag)
            inst = (engine or nc.sync).dma_start(t[:], dram.ap())
            if after is not None:
                tile.add_dep_helper(inst.ins, after, sync=True,
                                    reason="dma priority order")
            if inst_out is not None:
                inst_out.append(inst.ins)
            return [t[:, k * width:(k + 1) * width] for k in range(nchunk)]

        def layernorm(src_f32, src_st, out_tag, pool):
            """src: DC chunks [128,S] f32 (+st copies). Returns DC normalized
            chunks [128,S] st (gain/bias folded downstream by host)."""
            with tc.tile_pool(name=f"{out_tag}_lt", bufs=2) as lp, \
                 tc.tile_pool(name=f"{out_tag}_lp", bufs=1, space="PSUM") as sp, \
                 tc.tile_pool(name=f"{out_tag}_lb", bufs=1, space="PSUM") as bp:
                sum_ps = sp.tile([1, S], f32, tag="lnsum", name="lnsum")
                sq_ps = sp.tile([1, S], f32, tag="lnsq", name="lnsq")
                for k in range(DC):
                    sq = lp.tile([128, S], st, tag="lnsqt", name="lnsqt")
                    nc.vector.tensor_tensor(sq[:], src_st[k][:], src_st[k][:],
                                            OP.mult)
                    mm(sum_ps[:], ones128[:], src_st[k][:], k == 0, k == DC - 1)
                    mm(sq_ps[:], ones128[:], sq[:], k == 0, k == DC - 1)
                s_sb = lp.tile([1, S], st, tag="ln_ssb", name="ln_ssb")
                nc.vector.tensor_single_scalar(s_sb[:], sum_ps[:], 1.0 / D, OP.mult)
                q_sb = lp.tile([1, S], st, tag="ln_qsb", name="ln_qsb")
                nc.vector.tensor_single_scalar(q_sb[:], sq_ps[:], 1.0 / D, OP.mult)
                s_bc = bp.tile([128, S], f32, tag="ln_sbc", name="ln_sbc")
                q_bc = bp.tile([128, S], f32, tag="ln_qbc", name="ln_qbc")
                mm(s_bc[:], ones1r[:], s_sb[:], True, True)   # mean, bcast
                mm(q_bc[:], ones1r[:], q_sb[:], True, True)   # E[x^2], bcast
                # var = E[x^2] - mean^2; rstd/mean*rstd in st so the
                # per-chunk normalize runs in the DVE 16-bit fast mode
                m2 = lp.tile([128, S], f32, tag="ln_m2", name="ln_m2")
                nc.scalar.activation(m2[:], s_bc[:], A.Square)
                var = lp.tile([128, S], f32, tag="ln_var", name="ln_var")
                nc.vector.tensor_sub(var[:], q_bc[:], m2[:])
                v_t = lp.tile([128, S], st, tag="ln_vt", name="ln_vt")
                nc.scalar.activation(v_t[:], var[:], A.Abs_reciprocal_sqrt,
                                     bias=eps_t[:])
                u_t = lp.tile([128, S], st, tag="ln_ut", name="ln_ut")
                nc.vector.tensor_tensor(u_t[:], s_bc[:], v_t[:], OP.mult)
                outs = []
                for k in range(DC):
                    o = pool.tile([128, S], st, tag=f"{out_tag}{k}",
                                  name=f"{out_tag}{k}")
                    nc.vector.tensor_tensor(o[:], src_st[k][:], v_t[:], OP.mult)
                    nc.vector.tensor_sub(o[:], o[:], u_t[:])
                    outs.append(o)
                return outs

        def cast_st(src, tag, pool):
            if same_st:
                return src
            outs = []
            for k, t in enumerate(src):
                o = pool.tile([128, t.shape[-1]], st, tag=f"{tag}{k}",
                              name=f"{tag}{k}")
                nc.vector.tensor_copy(o[:], t[:])
                outs.append(o)
            return outs

        def project_fm(w_slices, rhs_chunks, nout, bias_ap, out_tag, pool,
                       extra=None, out_dt=None, width=None):
            """out^T[dout_chunk] = sum_k w_slices[k][:, m*128:...].T @ rhs[k]."""
            W = width or S
            outs = []
            with tc.tile_pool(name=f"{out_tag}_ps", bufs=3, space="PSUM") as pp:
                for mI in range(nout):
                    ps = pp.tile([128, W], f32, tag="proj", name="proj")
                    for k in range(DC):
                        mm(ps[:], w_slices[k][:, mI * 128:(mI + 1) * 128],
                           rhs_chunks[k][:], k == 0, k == DC - 1)
                    o = pool.tile([128, W], out_dt or st, tag=f"{out_tag}{mI}",
                                  name=f"{out_tag}{mI}")
                    if extra is not None:
                        extra(mI, ps, o)
                    elif bias_ap is not None:
                        nc.vector.tensor_scalar(o[:], ps[:],
                                                bias_ap[:, mI:mI + 1], None,
                                                OP.add)
                    else:
                        nc.vector.tensor_copy(o[:], ps[:])
                    outs.append(o)
            return outs

        def project_tm(act_chunks, w_slices, ntok, bias_bcast, out_tag, pool):
            """token-major V with a ones column appended per kv head:
            out[tok_chunk] = [V_kv | 1] blocks of 65 columns."""
            outs = []
            with tc.tile_pool(name=f"{out_tag}_ps", bufs=3, space="PSUM") as pp:
                for t in range(ntok):
                    ps = pp.tile([128, KVW], f32, tag="projtm", name="projtm")
                    for k in range(DC):
                        mm(ps[:], act_chunks[k][:, t * 128:(t + 1) * 128],
                           w_slices[k][:], k == 0, k == DC - 1)
                    o = pool.tile([128, NKV, HD + 1], st, tag=f"{out_tag}{t}",
                                  name=f"{out_tag}{t}")
                    nc.vector.tensor_add(
                        o[:, :, 0:HD],
                        ps[:].rearrange("p (kv d) -> p kv d", kv=NKV),
                        bias_bcast[:].rearrange("p (kv d) -> p kv d", kv=NKV))
                    for kv in range(NKV):
                        nc.vector.tensor_copy(o[:, kv, HD:HD + 1], ones128[:])
                    outs.append(o)
            return outs

        def attend(qT, kT, vtm, n_kc, mask_tile, cls, out_tag, pool):
            """Baseline-structure attention; mask applied multiplicatively on
            the DVE (host ships exp(mask)) instead of via id128 matmuls."""
            outs = []
            qr = []
            for kc in range(n_kc):
                act = [qh for qh in range(QC) if cls[kc * QC + qh] != 2]
                assert act and act == list(range(act[0], act[-1] + 1))
                qr.append((act[0] * 128, (act[-1] + 1) * 128))
            with tc.tile_pool(name=f"{out_tag}_sp", bufs=3, space="PSUM") as stp, \
                 tc.tile_pool(name=f"{out_tag}_op", bufs=2, space="PSUM") as opp, \
                 tc.tile_pool(name=f"{out_tag}_bp", bufs=1, space="PSUM") as bpp, \
                 tc.tile_pool(name=f"{out_tag}_et", bufs=6) as epool, \
                 tc.tile_pool(name=f"{out_tag}_un", bufs=1) as upool, \
                 tc.tile_pool(name=f"{out_tag}_dt", bufs=1) as dpool:
                # denominators staged at 32-aligned partitions: tile i holds
                # head pairs 4i..4i+3 at rows {0,32,64,96}
                den_t = [dpool.tile([128, 2 * S], f32, tag=f"den_t{i}",
                                    name=f"den_t{i}") for i in range(2)]
                for i in range(2):   # unwritten rows must stay finite
                    nc.vector.memset(den_t[i], 1.0)
                o_un = []
                for c in range(DC):
                    o_ps_h = [opp.tile([65, S], f32, tag=f"oph{hh}",
                                       name=f"oph{hh}") for hh in range(2)]
                    kv = (2 * c) // REP      # same kv head for both of the pair
                    for kc in range(n_kc):
                        q0, q1 = qr[kc]
                        adds = [q for q in range(QC) if cls[kc * QC + q] == 1]
                        st_h = []
                        e_h = []
                        for hh in range(2):
                            qh_ap = qT[c][hh * 64:(hh + 1) * 64, :]
                            kh = kT[kv][hh * 64:(hh + 1) * 64, :]
                            st_ps = stp.tile([128, S], f32, tag="st",
                                             name="st")
                            mm(st_ps[:, q0:q1], kh[:, kc * 128:(kc + 1) * 128],
                               qh_ap[:, q0:q1], True, True)
                            st_h.append(st_ps)
                        for hh in range(2):
                            e = epool.tile([128, S], st, tag="e", name="e")
                            nc.scalar.activation(e[:, q0:q1],
                                                 st_h[hh][:, q0:q1], A.Exp)
                            for q in adds:
                                nc.vector.tensor_tensor(
                                    e[:, q * 128:(q + 1) * 128],
                                    e[:, q * 128:(q + 1) * 128],
                                    mask_tile[:, kc, q * 128:(q + 1) * 128],
                                    OP.mult)
                            e_h.append(e)
                        for hh in range(2):
                            mm(o_ps_h[hh][:, q0:q1],
                               vtm[kc][:, kv, :], e_h[hh][:, q0:q1],
                               kc == 0, kc == n_kc - 1)
                    # evacuate unnormalized O + denominators; frees PSUM fast
                    ou = upool.tile([128, S], st, tag=f"un{c}", name=f"un{c}")
                    row = 32 * (c % 4)
                    for hh in range(2):
                        nc.vector.tensor_copy(ou[hh * 64:(hh + 1) * 64, :],
                                              o_ps_h[hh][0:64, :])
                        nc.vector.tensor_copy(
                            den_t[c // 4][row:row + 1, hh * S:(hh + 1) * S],
                            o_ps_h[hh][64:65, :])
                    o_un.append(ou)
                # ONE rsqrt pass for the whole attention (no ACT table thrash)
                den_sq = []
                for i in range(2):
                    dr = dpool.tile([128, 2 * S], f32, tag=f"den_r{i}",
                                    name=f"den_r{i}")
                    nc.scalar.activation(dr[:], den_t[i][:],
                                         A.Abs_reciprocal_sqrt)
                    ds = dpool.tile([128, 2 * S], st, tag=f"den_sq{i}",
                                    name=f"den_sq{i}")
                    nc.vector.tensor_tensor(ds[:], dr[:], dr[:], OP.mult)
                    den_sq.append(ds)
                for c in range(DC):
                    r_ps = bpp.tile([128, 2 * S], f32, tag="rbc", name="rbc")
                    mm(r_ps[:], onehot[:, c * 128:(c + 1) * 128],
                       den_sq[c // 4][:], True, True)
                    o = pool.tile([128, S], st, tag=f"{out_tag}{c}",
                                  name=f"{out_tag}{c}")
                    for hh in range(2):
                        nc.vector.tensor_tensor(
                            o[hh * 64:(hh + 1) * 64, :],
                            o_un[c][hh * 64:(hh + 1) * 64, :],
                            r_ps[hh * 64:(hh + 1) * 64, hh * S:(hh + 1) * S],
                            OP.mult)
                    outs.append(o)
            return outs

        h1 = [pers.tile([128, S], f32, tag=f"h1T{k}", name=f"h1T{k}")
              for k in range(DC)]
        h2 = [pers.tile([128, S], f32, tag=f"h2T{k}", name=f"h2T{k}")
              for k in range(DC)]

        # w2 fully resident before the MoE starts (4 sliced DMAs on the
        # store queue); gated behind the attention weight loads so the
        # prefetch never starves first-needed transfers
        w2_all = pers.tile([128, FC * D], st, tag="w2_all", name="w2_all")
        GW13 = DC * 1024
        w13e = [pers.tile([128, GW13], st, tag=f"w13e{g}", name=f"w13e{g}")
                for g in range(2)]

        def _issue_w2():
            # sync-queue FIFO after the attention weights: fills the DMA-idle
            # attention window without starving first-needed loads
            for q in range(4):
                w = FC * D // 4
                nc.sync.dma_start(w2_all[:, q * w:(q + 1) * w],
                                  di["w2"].ap()[:, q * w:(q + 1) * w])
            for g in range(2):
                nc.sync.dma_start(w13e[g][:],
                                  di["w13"].ap()[:, g * GW13:(g + 1) * GW13])

        # ---------------- self attention ----------------
        with tc.tile_pool(name="sa_acts", bufs=1) as sa:
            xT = load_chunks(di["xT"], DC, S, "xT", sa, f32)
            xT_st = xT if same_st else load_chunks(di["xT_st"], DC, S, "xTs", sa)
            n1 = layernorm(xT, xT_st, "n1T", sa)
            with tc.tile_pool(name="wqkvp", bufs=1) as wp:
                wt = load_chunks(di["wqkv"], DC, WPACK, "wqkv", wp)
                qT = project_fm([t[:, W_Q:W_Q + D] for t in wt], n1, DC,
                                bias("qb"), "qT", sa)
                kT = project_fm([t[:, W_K:W_K + 2 * KVW] for t in wt], n1, 4,
                                bias("kb"), "kT", sa)
                v_tm = project_tm(n1, [t[:, W_V:W_V + KVW] for t in wt], SC,
                                  bias("vb"), "v_tm", sa)
            with tc.tile_pool(name="wop", bufs=1) as wp:
                ow_t = load_chunks(di["ow"], DC, D, "ow", wp)
                sa_out = attend(qT, kT, v_tm, SC, maskT, sa_cls, "saT", sa)

                def o_epil(mI, ps, o):
                    nc.vector.scalar_tensor_tensor(o[:], ps[:],
                                                   bias("ob")[:, mI:mI + 1],
                                                   xT[mI][:], OP.add, OP.add)
                project_fm(ow_t, sa_out, DC, None, "h1w", _FixedPool(h1),
                           extra=o_epil, out_dt=f32)

        # ---------------- cross attention ----------------
        with tc.tile_pool(name="ca_acts", bufs=1) as ca:
            encT = load_chunks(di["encT"], DC, SK, "encT", ca)
            h1_st = cast_st(h1, "h1s", ca)
            with tc.tile_pool(name="wcap", bufs=1) as wp:
                wt = load_chunks(di["wca"], DC, WPACK, "wca", wp)
                _issue_w2()
                k2T = project_fm([t[:, W_K:W_K + 2 * KVW] for t in wt], encT, 4,
                                 bias("k2b"), "k2T", ca, width=SK)
                v2_tm = project_tm(encT, [t[:, W_V:W_V + KVW] for t in wt], KC,
                                   bias("v2b"), "v2_tm", ca)
                n2 = layernorm(h1, h1_st, "n2T", ca)
                q2T = project_fm([t[:, W_Q:W_Q + D] for t in wt], n2, DC,
                                 bias("q2b"), "q2T", ca)
            with tc.tile_pool(name="wo2p", bufs=1) as wp:
                o2w_t = load_chunks(di["o2w"], DC, D, "o2w", wp)
                ca_out = attend(q2T, k2T, v2_tm, KC, encmaskT, ca_cls, "caT", ca)

                def o2_epil(mI, ps, o):
                    nc.vector.scalar_tensor_tensor(o[:], ps[:],
                                                   bias("o2b")[:, mI:mI + 1],
                                                   h1[mI][:], OP.add, OP.add)
                project_fm(o2w_t, ca_out, DC, None, "h2w", _FixedPool(h2),
                           extra=o2_epil, out_dt=f32)

        # residual output (host: out_b = res.T + ffn_j0 + ffn_j1)
        for k in range(DC):
            eng_s.dma_start(out_res.ap()[k * 128:(k + 1) * 128, :], h2[k][:])

        # ---------------- MoE expert ----------------
        with tc.tile_pool(name="moe_acts", bufs=1) as mo:
            h2_st = cast_st(h2, "h2s", mo)
            n3 = layernorm(h2, h2_st, "n3T", mo)

            with tc.tile_pool(name="w13p", bufs=3) as wp, \
                 tc.tile_pool(name="mTp", bufs=6) as mp, \
                 tc.tile_pool(name="gh_ps", bufs=3, space="PSUM") as gp, \
                 tc.tile_pool(name="y_ps", bufs=1, space="PSUM") as yp, \
                 tc.tile_pool(name="gelu_t", bufs=3) as gt, \
                 tc.tile_pool(name="outp", bufs=2) as op_:
                y_ps = [[yp.tile([128, 512], f32, tag=f"y{t}{n}", name=f"y{t}{n}")
                         for n in range(2)] for t in range(QC)]
                gw = GRP * 128
                GW = DC * 1024
                for g in range(NGRP):
                    if g < 2:
                        wgt = w13e[g]
                    else:
                        wgt = wp.tile([128, GW], st, tag="w13g", name="w13g")
                        nc.sync.dma_start(wgt[:],
                                          di["w13"].ap()[:, g * GW:(g + 1) * GW])
                    wg = [wgt[:, k * 1024:(k + 1) * 1024] for k in range(DC)]
                    for mi in range(GRP):
                        mI = g * GRP + mi
                        # one PSUM bank: gelu-arg in [0:S], mult-arg in [S:2S]
                        gh = gp.tile([128, 2 * S], f32, tag="gh", name="gh")
                        for k in range(DC):
                            mm(gh[:, 0:S], wg[k][:, mi * 128:(mi + 1) * 128],
                               n3[k][:], k == 0, k == DC - 1)
                        for k in range(DC):
                            mm(gh[:, S:2 * S],
                               wg[k][:, gw + mi * 128:gw + (mi + 1) * 128],
                               n3[k][:], k == 0, k == DC - 1)
                        ge = gt.tile([128, S], f32, tag="ge", name="ge")
                        nc.scalar.activation(ge[:], gh[:, 0:S], A.Gelu,
                                             bias=bias("b1")[:, mI:mI + 1])
                        mT = mp.tile([128, S], st, tag="mT", name="mT")
                        nc.vector.scalar_tensor_tensor(mT[:], gh[:, S:2 * S],
                                                       bias("b3")[:, mI:mI + 1],
                                                       ge[:], OP.add, OP.mult)
                        # fused down-projection: w2 already resident
                        for t in range(QC):
                            for n in range(2):
                                mm(y_ps[t][n][:], mT[:, t * 128:(t + 1) * 128],
                                   w2_all[:, mI * D + n * 512:
                                          mI * D + (n + 1) * 512],
                                   mI == 0, mI == FC - 1)
                for t in range(QC):
                    for n in range(2):
                        o = op_.tile([128, 512], f32, tag="o_out", name="o_out")
                        nc.vector.tensor_scalar_mul(o[:], y_ps[t][n][:],
                                                    bias("c")[:, 0:1])
                        eng_s.dma_start(
                            out_ffn.ap()[t * 128:(t + 1) * 128,
                                         n * 512:(n + 1) * 512], o[:])

    nc.compile()
    return nc


class _FixedPool:
    """Adapter letting project_fm write into pre-allocated tiles."""

    def __init__(self, tiles):
        self._tiles = list(tiles)
        self._i = 0

    def tile(self, shape, dtype, tag=None, name=None):
        t = self._tiles[self._i]
        self._i += 1
        return t


def _routing(langs):
    """Per-sequence expert slots [(expert_idx, coef) x2], matching the
    reference: coef[e,b] = any(langs[b]==4+e) * (1/count(langs[b]>3))."""
    langs = np.asarray(langs)
    slots = []
    for b in range(langs.shape[0]):
        row = [int(v) for v in langs[b]]
        cnt = sum(1 for v in row if v > 3)
        rw = 1.0 if cnt == 0 else 1.0 / cnt
        seen = []
        for v in row:
            if v > 3 and 0 <= v - 4 < NE and (v - 4) not in seen:
                seen.append(v - 4)
        sl = [(e, rw) for e in seen]
        while len(sl) < 2:
            sl.append((0, 0.0))
        slots.append(sl[:2])
    return slots


def _mask_classes(maskT, n_kc):
    """Classify each [128 keys x 128 queries] block of a transposed mask:
    0 all-zero (no add), 1 general (add), 2 fully masked (skip compute).
    Keeps at least one active key block per query and contiguous active
    ranges per key chunk."""
    cls = []
    for kc in range(n_kc):
        for qh in range(QC):
            blk = maskT[kc * 128:(kc + 1) * 128, qh * 128:(qh + 1) * 128]
            if np.all(blk == 0):
                cls.append(0)
            elif np.all(blk <= -1e8):
                cls.append(2)
            else:
                cls.append(1)
    for qh in range(QC):
        if all(cls[kc * QC + qh] == 2 for kc in range(n_kc)):
            for kc in range(n_kc):
                cls[kc * QC + qh] = 1
    for kc in range(n_kc):
        act = [q for q in range(QC) if cls[kc * QC + q] != 2]
        if not act or act != list(range(act[0], act[-1] + 1)):
            for q in range(QC):
                if cls[kc * QC + q] == 2:
                    cls[kc * QC + q] = 1
    return tuple(cls)


def kernel(**inputs):
    mode = MODE
    np_dt = ml_dtypes.bfloat16 if mode == "bf16" else np.float32
    f32 = np.float32

    inp = {k: np.asarray(v) for k, v in inputs.items()}
    x = inp["hidden_states"].astype(f32)
    enc = inp["encoder_hidden_states"].astype(f32)
    mask = inp["attention_mask"].astype(f32)
    encmask = inp["encoder_attention_mask"].astype(f32)
    g1, b1 = inp["ln1_g"].astype(f32), inp["ln1_b"].astype(f32)
    g2, b2 = inp["ln2_g"].astype(f32), inp["ln2_b"].astype(f32)
    g3, b3 = inp["ln3_g"].astype(f32), inp["ln3_b"].astype(f32)

    def dup_kv(w):
        return np.concatenate([np.tile(w[:, 64 * j:64 * (j + 1)], (1, 2))
                               for j in range(NKV)], axis=1)

    def dup_kv_b(v):
        return np.concatenate([np.tile(v[64 * j:64 * (j + 1)], 2)
                               for j in range(NKV)])

    sc = HD ** -0.5
    qw_f = g1[:, None] * inp["sa_q_w"] * sc
    qb_f = (b1 @ inp["sa_q_w"] + inp["sa_q_b"]) * sc
    kw_f = dup_kv(g1[:, None] * inp["sa_k_w"])
    kb_f = dup_kv_b(b1 @ inp["sa_k_w"] + inp["sa_k_b"])
    vw_f = g1[:, None] * inp["sa_v_w"]
    vb_f = b1 @ inp["sa_v_w"] + inp["sa_v_b"]
    q2w_f = g2[:, None] * inp["ca_q_w"] * sc
    q2b_f = (b2 @ inp["ca_q_w"] + inp["ca_q_b"]) * sc
    k2w_f = dup_kv(inp["ca_k_w"])
    k2b_f = dup_kv_b(inp["ca_k_b"])
    w1_f = inp["moe_w1"] * g3[None, :, None]
    b1_f = np.einsum("d,edf->ef", b3, inp["moe_w1"]).astype(f32)
    w3_f = inp["moe_w3"] * g3[None, :, None]
    b3_f = np.einsum("d,edf->ef", b3, inp["moe_w3"]).astype(f32)

    maskT0 = np.ascontiguousarray(mask[:, 0].transpose(0, 2, 1))     # [B,S,S]
    encmaskT0 = np.ascontiguousarray(encmask[:, 0].transpose(0, 2, 1))
    sa_cls = _mask_classes(maskT0[0], SC)
    ca_cls = _mask_classes(encmaskT0[0], KC)
    for b in range(1, B):
        if _mask_classes(maskT0[b], SC) != sa_cls or \
           _mask_classes(encmaskT0[b], KC) != ca_cls:
            sa_cls = tuple(1 for _ in range(SC * QC))
            ca_cls = tuple(1 for _ in range(KC * QC))
            break

    key = (mode, sa_cls, ca_cls)
    if key not in _CACHE:
        _CACHE[key] = _build(mode, sa_cls, ca_cls)
    nc = _CACHE[key]

    def col128(v):
        return np.asarray(v, f32).reshape(-1, 128).T

    def pack_k(w):
        w = np.asarray(w)
        return np.concatenate([w[k * 128:(k + 1) * 128, :]
                               for k in range(w.shape[0] // 128)], axis=1)

    slots = _routing(inp["langs"])
    # block c selects den row 32*(c%4) of den tile c//4
    onehot = np.zeros((128, 8 * 128), f32)
    for c in range(8):
        onehot[32 * (c % 4), c * 128:(c + 1) * 128] = 1.0
    wqkv = np.concatenate([qw_f, kw_f, vw_f], axis=1).astype(np_dt)
    wca = np.concatenate([q2w_f, k2w_f, inp["ca_v_w"]], axis=1).astype(np_dt)

    bias_common = np.zeros((128, BIAS_W), f32)
    for nm, v in [("qb", col128(qb_f)), ("kb", col128(kb_f)),
                  ("vb", np.broadcast_to(vb_f.astype(f32), (128, KVW))),
                  ("ob", col128(inp["sa_o_b"])),
                  ("q2b", col128(q2b_f)), ("k2b", col128(k2b_f)),
                  ("v2b", np.broadcast_to(inp["ca_v_b"].astype(f32), (128, KVW))),
                  ("o2b", col128(inp["ca_o_b"]))]:
        off, w = _BIAS_COLS[nm]
        bias_common[:, off:off + w] = v

    in_maps = []
    for c in range(8):
        b, j = c // 2, c % 2
        e, coef = slots[b][j]
        xT = pack_k(np.ascontiguousarray(x[b].T))
        # interleave w1/w3 by group: [w1 grp g | w3 grp g] blocks of 512 cols
        gw = GRP * 128
        w13 = np.empty((128, NGRP * DC * 1024), f32)
        for g in range(NGRP):
            for k in range(DC):
                c0 = (g * DC + k) * 1024
                w13[:, c0:c0 + 512] = w1_f[e][k * 128:(k + 1) * 128,
                                              g * 512:(g + 1) * 512]
                w13[:, c0 + 512:c0 + 1024] = w3_f[e][k * 128:(k + 1) * 128,
                                                     g * 512:(g + 1) * 512]
        bt = bias_common.copy()
        for nm, v in [("b1", col128(b1_f[e])), ("b3", col128(b3_f[e]))]:
            off, w = _BIAS_COLS[nm]
            bt[:, off:off + w] = v
        bt[:, _BIAS_COLS["c"][0]] = coef
        m = {
            "xT": xT,
            "encT": pack_k(np.ascontiguousarray(enc[b].T)).astype(np_dt),
            "id128": np.eye(128, dtype=f32).astype(np_dt),
            "onehot": onehot.astype(np_dt),
            "ones_col": np.ones((128, 1), f32).astype(np_dt),
            "ones_row": np.ones((1, 128), f32).astype(np_dt),
            "wqkv": pack_k(wqkv), "wca": pack_k(wca),
            "ow": pack_k(inp["sa_o_w"].astype(np_dt)),
            "o2w": pack_k(inp["ca_o_w"].astype(np_dt)),
            "biases": bt,
            "w13": w13.astype(np_dt),
            "w2": pack_k(np.ascontiguousarray(inp["moe_w2"][e])).astype(np_dt),
        }
        if mode != "f32":
            m["xT_st"] = xT.astype(np_dt)
        if any(cc == 1 for cc in sa_cls):
            m["maskT"] = np.exp(maskT0[b]).astype(np_dt)
        if any(cc == 1 for cc in ca_cls):
            m["encmaskT"] = np.exp(encmaskT0[b]).astype(np_dt)
        in_maps.append(m)

    kw = {}
    if _TRACE_DIR:
        kw = dict(trace=True, tmpdir=_TRACE_DIR, trace_cores=[0])
    res = bass_utils.run_bass_kernel_spmd(nc, in_maps, core_ids=list(range(8)), **kw)
    global _LAST_EXEC_NS
    _LAST_EXEC_NS = res.exec_time_ns
    return np.stack([
        res.results[2 * b]["out_res"].T
        + res.results[2 * b]["out_ffn"]
        + res.results[2 * b + 1]["out_ffn"]
        for b in range(B)
    ]).astype(f32)



# revision 33
# speedup vs baseline: 1.2320x; 1.0465x over previous
"""MBart MoE decoder layer on 8 trn2 NeuronCores.

Sharding: 8 cores = 8 (sequence, expert-slot) pairs. Core c handles
sequence b=c//2, expert slot j=c%2 (each sequence is lang-routed to at
most 2 distinct experts; routing is computed on the host from `langs`).
Each core computes the full attention path for its sequence (replicated
across the pair) and one expert FFN over all 256 tokens; the host sums
the pair's partial outputs (expert-sharded combine) and transposes back
to token-major. Expert weights are gathered per-core on the host, so a
core only receives the one expert it needs.

On-device layout is feature-major [D, tokens]: projections take weights
as lhsT (feature-major out) or activations as lhsT (token-major out), so
no activation transposes are needed anywhere. LN gains/biases are folded
into the downstream weights on the host; softmax uses transposed scores
[keys, queries] with the attention mask added via an identity-matmul
into PSUM (host classifies each 128x128 mask block as zero / add / skip,
so causal dead blocks are never computed) and denominators accumulated
via a ones-matmul, then broadcast over partitions with a rank-1 matmul
for one full-lane reciprocal per head pair.
"""

import os
import sys
from contextlib import ExitStack

for _p in ("/opt/trn_rl_repo",):
    if _p not in sys.path:
        sys.path.append(_p)

import numpy as np
import ml_dtypes

import concourse.bass as bass
import concourse.tile as tile
import concourse.mybir as mybir
from concourse import bacc, bass_utils

B, S, SK = 4, 256, 512
D, NH, NKV, HD = 1024, 16, 4, 64
DE, NE = 4096, 8
LN_EPS = 1e-5
REP = NH // NKV
DC = D // 128    # 8 feature chunks
FC = DE // 128   # 32 ffn chunks
SC = S // 128    # 2 self-attn key chunks
KC = SK // 128   # 4 cross-attn key chunks
QC = S // 128    # 2 query halves
KVW = NKV * HD   # 256
GRP = 4          # ffn chunks per MoE weight group
NGRP = FC // GRP

MODE = os.environ.get("KERNEL_MM_DTYPE", "bf16")  # "bf16" | "f32r" | "f32"

_CACHE: dict = {}
_TRACE_DIR = None   # set by test harness for profiling runs
_LAST_EXEC_NS = None

# packed attention-weight column layout: qw | kw(dup) | vw
W_Q, W_K, W_V = 0, D, D + 2 * KVW
WPACK = D + 2 * KVW + KVW  # 1792

# packed per-partition bias column layout
_BIAS_COLS = {}
_off = 0
for _n, _w in [("qb", DC), ("kb", 4), ("vb", KVW), ("ob", DC),
               ("q2b", DC), ("k2b", 4), ("v2b", KVW), ("o2b", DC),
               ("b1", FC), ("b3", FC), ("c", 1)]:
    _BIAS_COLS[_n] = (_off, _w)
    _off += _w
BIAS_W = _off


def _build(mode, sa_cls, ca_cls):
    """sa_cls/ca_cls: block classes per (kc, qhalf): 0=no-mask, 1=mask-add,
    2=fully-masked(skip)."""
    st = {"bf16": mybir.dt.bfloat16, "f32r": mybir.dt.float32r,
          "f32": mybir.dt.float32}[mode]
    f32 = mybir.dt.float32
    same_st = mode == "f32"
    A = mybir.ActivationFunctionType
    OP = mybir.AluOpType

    nc = bacc.Bacc("TRN2", target_bir_lowering=False, debug=False, num_devices=8)
    import os as _os
    _SPLIT = _os.environ.get("KERNEL_DMA_SPLIT", "1") == "1"
    eng_b = nc.scalar if _SPLIT else nc.sync
    eng_s = nc.gpsimd if _SPLIT else nc.sync

    def mm(psum, lhsT, rhs, start, stop):
        nc.tensor.matmul(psum, lhsT, rhs, start=start, stop=stop)

    di = {}

    def din(name, shape, dtype=None):
        di[name] = nc.dram_tensor(name, list(shape), dtype or st, kind="ExternalInput")
        return di[name]

    din("xT_st", (128, DC * S))
    din("encT", (128, DC * SK))
    need_samask = any(c == 1 for c in sa_cls)
    need_camask = any(c == 1 for c in ca_cls)
    if need_samask:
        din("maskT", (S, S))
    if need_camask:
        din("encmaskT", (SK, S))
    din("id128", (128, 128))
    din("onehot", (128, 8 * 128))
    din("ones_col", (128, 1))
    din("ones_row", (1, 128))
    din("wqkv", (128, DC * WPACK))
    din("wca", (128, DC * WPACK))
    din("ow", (128, DC * D))
    din("o2w", (128, DC * D))
    din("biases", (128, BIAS_W), f32)
    din("w13", (128, NGRP * DC * 1024))  # per grp, per k: [w1 512 | w3 512]
    din("w2", (128, FC * D))
    out_res = nc.dram_tensor("out_res", [D, S], f32, kind="ExternalOutput")
    out_ffn = nc.dram_tensor("out_ffn", [S, D], f32, kind="ExternalOutput")

    with tile.TileContext(nc) as tc, ExitStack() as ctx:
        cp = ctx.enter_context(tc.tile_pool(name="consts", bufs=1))
        pers = ctx.enter_context(tc.tile_pool(name="pers", bufs=1))
        xp = ctx.enter_context(tc.tile_pool(name="xpool", bufs=1))

        ones128 = cp.tile([128, 1], st, tag="ones128", name="ones128")
        eng_b.dma_start(ones128[:], di["ones_col"].ap())
        ones1r = cp.tile([1, 128], st, tag="ones1r", name="ones1r")
        eng_b.dma_start(ones1r[:], di["ones_row"].ap())
        eps_t = cp.tile([128, 1], f32, tag="eps_t", name="eps_t")
        nc.vector.memset(eps_t, LN_EPS)
        id128 = cp.tile([128, 128], st, tag="id128", name="id128")
        eng_b.dma_start(id128[:], di["id128"].ap())
        onehot = cp.tile([128, 8 * 128], st, tag="onehot", name="onehot")
        eng_b.dma_start(onehot[:], di["onehot"].ap())
        maskT = encmaskT = None
        if need_samask:
            maskT = cp.tile([128, SC, S], st, tag="maskT", name="maskT")
            for kc in range(SC):
                eng_b.dma_start(maskT[:, kc, :],
                                  di["maskT"].ap()[kc * 128:(kc + 1) * 128, :])
        if need_camask:
            encmaskT = cp.tile([128, KC, S], st, tag="encmaskT", name="encmaskT")
            for kc in range(KC):
                eng_b.dma_start(encmaskT[:, kc, :],
                                  di["encmaskT"].ap()[kc * 128:(kc + 1) * 128, :])

        bias_t = cp.tile([128, BIAS_W], f32, tag="bias_t", name="bias_t")
        eng_b.dma_start(bias_t[:], di["biases"].ap())

        def bias(nm):
            off, w = _BIAS_COLS[nm]
            return bias_t[:, off:off + w]

        def load_chunks(dram, nchunk, width, tag, pool, dtype=st, engine=None,
                        after=None, inst_out=None):
            t = pool.tile([128, nchunk * width], dtype, tag=tag, name=tag)
            inst = (engine or nc.sync).dma_start(t[:], dram.ap())
            if after is not None:
                tile.add_dep_helper(inst.ins, after, sync=True,
                                    reason="dma priority order")
            if inst_out is not None:
                inst_out.append(inst.ins)
            return [t[:, k * width:(k + 1) * width] for k in range(nchunk)]

        def layernorm(src_f32, src_st, out_tag, pool):
            """src: DC chunks [128,S] f32 (+st copies). Returns DC normalized
            chunks [128,S] st (gain/bias folded downstream by host)."""
            with tc.tile_pool(name=f"{out_tag}_lt", bufs=2) as lp, \
                 tc.tile_pool(name=f"{out_tag}_lp", bufs=1, space="PSUM") as sp, \
                 tc.tile_pool(name=f"{out_tag}_lb", bufs=1, space="PSUM") as bp:
                sum_ps = sp.tile([1, S], f32, tag="lnsum", name="lnsum")
                sq_ps = sp.tile([1, S], f32, tag="lnsq", name="lnsq")
                for k in range(DC):
                    sq = lp.tile([128, S], st, tag="lnsqt", name="lnsqt")
                    nc.vector.tensor_tensor(sq[:], src_st[k][:], src_st[k][:],
                                            OP.mult)
                    mm(sum_ps[:], ones128[:], src_st[k][:], k == 0, k == DC - 1)
                    mm(sq_ps[:], ones128[:], sq[:], k == 0, k == DC - 1)
                s_sb = lp.tile([1, S], st, tag="ln_ssb", name="ln_ssb")
                nc.vector.tensor_single_scalar(s_sb[:], sum_ps[:], 1.0 / D, OP.mult)
                q_sb = lp.tile([1, S], st, tag="ln_qsb", name="ln_qsb")
                nc.vector.tensor_single_scalar(q_sb[:], sq_ps[:], 1.0 / D, OP.mult)
                s_bc = bp.tile([128, S], f32, tag="ln_sbc", name="ln_sbc")
                q_bc = bp.tile([128, S], f32, tag="ln_qbc", name="ln_qbc")
                mm(s_bc[:], ones1r[:], s_sb[:], True, True)   # mean, bcast
                mm(q_bc[:], ones1r[:], q_sb[:], True, True)   # E[x^2], bcast
                # var = E[x^2] - mean^2; rstd/mean*rstd in st so the
                # per-chunk normalize runs in the DVE 16-bit fast mode
                m2 = lp.tile([128, S], f32, tag="ln_m2", name="ln_m2")
                nc.scalar.activation(m2[:], s_bc[:], A.Square)
                var = lp.tile([128, S], f32, tag="ln_var", name="ln_var")
                nc.vector.tensor_sub(var[:], q_bc[:], m2[:])
                v_t = lp.tile([128, S], st, tag="ln_vt", name="ln_vt")
                nc.scalar.activation(v_t[:], var[:], A.Abs_reciprocal_sqrt,
                                     bias=eps_t[:])
                u_t = lp.tile([128, S], st, tag="ln_ut", name="ln_ut")
                nc.vector.tensor_tensor(u_t[:], s_bc[:], v_t[:], OP.mult)
                outs = []
                for k in range(DC):
                    o = pool.tile([128, S], st, tag=f"{out_tag}{k}",
                                  name=f"{out_tag}{k}")
                    nc.vector.tensor_tensor(o[:], src_st[k][:], v_t[:], OP.mult)
                    nc.vector.tensor_sub(o[:], o[:], u_t[:])
                    outs.append(o)
                return outs

        def cast_st(src, tag, pool):
            if same_st:
                return src
            outs = []
            for k, t in enumerate(src):
                o = pool.tile([128, t.shape[-1]], st, tag=f"{tag}{k}",
                              name=f"{tag}{k}")
                nc.vector.tensor_copy(o[:], t[:])
                outs.append(o)
            return outs

        def project_fm(w_slices, rhs_chunks, nout, bias_ap, out_tag, pool,
                       extra=None, out_dt=None, width=None):
            """out^T[dout_chunk] = sum_k w_slices[k][:, m*128:...].T @ rhs[k]."""
            W = width or S
            outs = []
            with tc.tile_pool(name=f"{out_tag}_ps", bufs=3, space="PSUM") as pp:
                for mI in range(nout):
                    ps = pp.tile([128, W], f32, tag="proj", name="proj")
                    for k in range(DC):
                        mm(ps[:], w_slices[k][:, mI * 128:(mI + 1) * 128],
                           rhs_chunks[k][:], k == 0, k == DC - 1)
                    o = pool.tile([128, W], out_dt or st, tag=f"{out_tag}{mI}",
                                  name=f"{out_tag}{mI}")
                    if extra is not None:
                        extra(mI, ps, o)
                    elif bias_ap is not None:
                        nc.vector.tensor_scalar(o[:], ps[:],
                                                bias_ap[:, mI:mI + 1], None,
                                                OP.add)
                    else:
                        nc.vector.tensor_copy(o[:], ps[:])
                    outs.append(o)
            return outs

        def project_tm(act_chunks, w_slices, ntok, bias_bcast, out_tag, pool):
            """token-major V with a ones column appended per kv head:
            out[tok_chunk] = [V_kv | 1] blocks of 65 columns."""
            outs = []
            with tc.tile_pool(name=f"{out_tag}_ps", bufs=3, space="PSUM") as pp:
                for t in range(ntok):
                    ps = pp.tile([128, KVW], f32, tag="projtm", name="projtm")
                    for k in range(DC):
                        mm(ps[:], act_chunks[k][:, t * 128:(t + 1) * 128],
                           w_slices[k][:], k == 0, k == DC - 1)
                    o = pool.tile([128, NKV, HD + 1], st, tag=f"{out_tag}{t}",
                                  name=f"{out_tag}{t}")
                    nc.vector.tensor_add(
                        o[:, :, 0:HD],
                        ps[:].rearrange("p (kv d) -> p kv d", kv=NKV),
                        bias_bcast[:].rearrange("p (kv d) -> p kv d", kv=NKV))
                    for kv in range(NKV):
                        nc.vector.tensor_copy(o[:, kv, HD:HD + 1], ones128[:])
                    outs.append(o)
            return outs

        def attend(qT, kT, vtm, n_kc, mask_tile, cls, out_tag, pool):
            """Baseline-structure attention; mask applied multiplicatively on
            the DVE (host ships exp(mask)) instead of via id128 matmuls."""
            outs = []
            qr = []
            for kc in range(n_kc):
                act = [qh for qh in range(QC) if cls[kc * QC + qh] != 2]
                assert act and act == list(range(act[0], act[-1] + 1))
                qr.append((act[0] * 128, (act[-1] + 1) * 128))
            with tc.tile_pool(name=f"{out_tag}_sp", bufs=3, space="PSUM") as stp, \
                 tc.tile_pool(name=f"{out_tag}_op", bufs=2, space="PSUM") as opp, \
                 tc.tile_pool(name=f"{out_tag}_bp", bufs=1, space="PSUM") as bpp, \
                 tc.tile_pool(name=f"{out_tag}_et", bufs=5) as epool, \
                 tc.tile_pool(name=f"{out_tag}_un", bufs=1) as upool, \
                 tc.tile_pool(name=f"{out_tag}_dt", bufs=1) as dpool:
                # denominators staged at 32-aligned partitions: tile i holds
                # head pairs 4i..4i+3 at rows {0,32,64,96}
                den_t = [dpool.tile([128, 2 * S], st, tag=f"den_t{i}",
                                    name=f"den_t{i}") for i in range(2)]
                for i in range(2):   # unwritten rows must stay finite
                    nc.vector.memset(den_t[i], 1.0)
                o_un = []
                for c in range(DC):
                    o_ps_h = [opp.tile([65, S], f32, tag=f"oph{hh}",
                                       name=f"oph{hh}") for hh in range(2)]
                    kv = (2 * c) // REP      # same kv head for both of the pair
                    for kc in range(n_kc):
                        q0, q1 = qr[kc]
                        adds = [q for q in range(QC) if cls[kc * QC + q] == 1]
                        st_h = []
                        e_h = []
                        for hh in range(2):
                            qh_ap = qT[c][hh * 64:(hh + 1) * 64, :]
                            kh = kT[kv][hh * 64:(hh + 1) * 64, :]
                            st_ps = stp.tile([128, S], f32, tag="st",
                                             name="st")
                            mm(st_ps[:, q0:q1], kh[:, kc * 128:(kc + 1) * 128],
                               qh_ap[:, q0:q1], True, True)
                            st_h.append(st_ps)
                        for hh in range(2):
                            e = epool.tile([128, S], st, tag="e", name="e")
                            nc.scalar.activation(e[:, q0:q1],
                                                 st_h[hh][:, q0:q1], A.Exp)
                            for q in adds:
                                nc.vector.tensor_tensor(
                                    e[:, q * 128:(q + 1) * 128],
                                    e[:, q * 128:(q + 1) * 128],
                                    mask_tile[:, kc, q * 128:(q + 1) * 128],
                                    OP.mult)
                            e_h.append(e)
                        for hh in range(2):
                            mm(o_ps_h[hh][:, q0:q1],
                               vtm[kc][:, kv, :], e_h[hh][:, q0:q1],
                               kc == 0, kc == n_kc - 1)
                    # evacuate unnormalized O + denominators; frees PSUM fast
                    ou = upool.tile([128, S], st, tag=f"un{c}", name=f"un{c}")
                    row = 32 * (c % 4)
                    for hh in range(2):
                        nc.vector.tensor_copy(ou[hh * 64:(hh + 1) * 64, :],
                                              o_ps_h[hh][0:64, :])
                        nc.vector.tensor_copy(
                            den_t[c // 4][row:row + 1, hh * S:(hh + 1) * S],
                            o_ps_h[hh][64:65, :])
                    o_un.append(ou)
                # ONE rsqrt pass for the whole attention (no ACT table thrash)
                den_sq = den_t
                for i in range(2):
                    nc.scalar.activation(den_t[i][:], den_t[i][:],
                                         A.Abs_reciprocal_sqrt)
                    nc.vector.tensor_tensor(den_t[i][:], den_t[i][:],
                                            den_t[i][:], OP.mult)
                for c in range(DC):
                    r_ps = bpp.tile([128, 2 * S], f32, tag="rbc", name="rbc")
                    mm(r_ps[:], onehot[:, c * 128:(c + 1) * 128],
                       den_sq[c // 4][:], True, True)
                    o = pool.tile([128, S], st, tag=f"{out_tag}{c}",
                                  name=f"{out_tag}{c}")
                    for hh in range(2):
                        nc.vector.tensor_tensor(
                            o[hh * 64:(hh + 1) * 64, :],
                            o_un[c][hh * 64:(hh + 1) * 64, :],
                            r_ps[hh * 64:(hh + 1) * 64, hh * S:(hh + 1) * S],
                            OP.mult)
                    outs.append(o)
            return outs

        h1 = [pers.tile([128, S], f32, tag=f"h1T{k}", name=f"h1T{k}")
              for k in range(DC)]
        h2 = [pers.tile([128, S], f32, tag=f"h2T{k}", name=f"h2T{k}")
              for k in range(DC)]

        # w2 fully resident before the MoE starts (4 sliced DMAs on the
        # store queue); gated behind the attention weight loads so the
        # prefetch never starves first-needed transfers
        w2_all = pers.tile([128, FC * D], st, tag="w2_all", name="w2_all")
        GW13 = DC * 1024
        w13e = [pers.tile([128, GW13], st, tag=f"w13e{g}", name=f"w13e{g}")
                for g in range(1)]

        def _issue_w2():
            # sync-queue FIFO after the attention weights: fills the DMA-idle
            # attention window without starving first-needed loads
            for q in range(4):
                w = FC * D // 4
                nc.sync.dma_start(w2_all[:, q * w:(q + 1) * w],
                                  di["w2"].ap()[:, q * w:(q + 1) * w])
            for g in range(1):
                nc.sync.dma_start(w13e[g][:],
                                  di["w13"].ap()[:, g * GW13:(g + 1) * GW13])

        # ---------------- self attention ----------------
        # sync-queue DMA order == need order: consumers' DMA waits are
        # coarsened to later completions on the same lane, so any transfer
        # queued out of need-order delays every later consumer.
        with tc.tile_pool(name="sa_acts", bufs=1) as sa:
            xT_st = load_chunks(di["xT_st"], DC, S, "xTs", sa)
            n1 = layernorm(xT_st, xT_st, "n1T", sa)
            with tc.tile_pool(name="wqkvp", bufs=1) as wp:
                wt = load_chunks(di["wqkv"], DC, WPACK, "wqkv", wp)
                qT = project_fm([t[:, W_Q:W_Q + D] for t in wt], n1, DC,
                                bias("qb"), "qT", sa)
                kT = project_fm([t[:, W_K:W_K + 2 * KVW] for t in wt], n1, 4,
                                bias("kb"), "kT", sa)
                v_tm = project_tm(n1, [t[:, W_V:W_V + KVW] for t in wt], SC,
                                  bias("vb"), "v_tm", sa)
            # cross-attn K/V only need encT — loaded and computed here so the
            # PE has filler work during self-attend softmax stalls and the
            # cross weights are queued right after wqkv
            encT = load_chunks(di["encT"], DC, SK, "encT", pers)
            wca = load_chunks(di["wca"], DC, WPACK, "wca", pers)
            k2T = project_fm([t[:, W_K:W_K + 2 * KVW] for t in wca], encT, 4,
                             bias("k2b"), "k2T", pers, width=SK)
            v2_tm = project_tm(encT, [t[:, W_V:W_V + KVW] for t in wca], KC,
                               bias("v2b"), "v2_tm", pers)
            with tc.tile_pool(name="wop", bufs=1) as wp:
                ow_t = load_chunks(di["ow"], DC, D, "ow", wp)
                sa_out = attend(qT, kT, v_tm, SC, maskT, sa_cls, "saT", sa)

                def o_epil(mI, ps, o):
                    nc.vector.scalar_tensor_tensor(o[:], ps[:],
                                                   bias("ob")[:, mI:mI + 1],
                                                   xT_st[mI][:], OP.add, OP.add)
                project_fm(ow_t, sa_out, DC, None, "h1w", _FixedPool(h1),
                           extra=o_epil, out_dt=f32)

        # ---------------- cross attention ----------------
        with tc.tile_pool(name="ca_acts", bufs=1) as ca:
            _issue_w2()   # MoE prefetch: queued after all attention weights
            h1_st = cast_st(h1, "h1s", ca)
            n2 = layernorm(h1, h1_st, "n2T", ca)
            q2T = project_fm([t[:, W_Q:W_Q + D] for t in wca], n2, DC,
                             bias("q2b"), "q2T", ca)
            with tc.tile_pool(name="wo2p", bufs=1) as wp:
                o2w_t = load_chunks(di["o2w"], DC, D, "o2w", wp)
                ca_out = attend(q2T, k2T, v2_tm, KC, encmaskT, ca_cls, "caT", ca)

                def o2_epil(mI, ps, o):
                    nc.vector.scalar_tensor_tensor(o[:], ps[:],
                                                   bias("o2b")[:, mI:mI + 1],
                                                   h1[mI][:], OP.add, OP.add)
                project_fm(o2w_t, ca_out, DC, None, "h2w", _FixedPool(h2),
                           extra=o2_epil, out_dt=f32)

        # residual output (host: out_b = res.T + ffn_j0 + ffn_j1)
        for k in range(DC):
            eng_s.dma_start(out_res.ap()[k * 128:(k + 1) * 128, :], h2[k][:])

        # ---------------- MoE expert ----------------
        with tc.tile_pool(name="moe_acts", bufs=1) as mo:
            h2_st = cast_st(h2, "h2s", mo)
            n3 = layernorm(h2, h2_st, "n3T", mo)

            with tc.tile_pool(name="w13p", bufs=2) as wp, \
                 tc.tile_pool(name="mTp", bufs=4) as mp, \
                 tc.tile_pool(name="gh_ps", bufs=3, space="PSUM") as gp, \
                 tc.tile_pool(name="y_ps", bufs=1, space="PSUM") as yp, \
                 tc.tile_pool(name="gelu_t", bufs=3) as gt, \
                 tc.tile_pool(name="outp", bufs=2) as op_:
                y_ps = [[yp.tile([128, 512], f32, tag=f"y{t}{n}", name=f"y{t}{n}")
                         for n in range(2)] for t in range(QC)]
                gw = GRP * 128
                GW = DC * 1024
                for g in range(NGRP):
                    if g < 1:
                        wgt = w13e[g]
                    else:
                        wgt = wp.tile([128, GW], st, tag="w13g", name="w13g")
                        nc.sync.dma_start(wgt[:],
                                          di["w13"].ap()[:, g * GW:(g + 1) * GW])
                    wg = [wgt[:, k * 1024:(k + 1) * 1024] for k in range(DC)]
                    for mi in range(GRP):
                        mI = g * GRP + mi
                        # one PSUM bank: gelu-arg in [0:S], mult-arg in [S:2S]
                        gh = gp.tile([128, 2 * S], f32, tag="gh", name="gh")
                        for k in range(DC):
                            mm(gh[:, 0:S], wg[k][:, mi * 128:(mi + 1) * 128],
                               n3[k][:], k == 0, k == DC - 1)
                        for k in range(DC):
                            mm(gh[:, S:2 * S],
                               wg[k][:, gw + mi * 128:gw + (mi + 1) * 128],
                               n3[k][:], k == 0, k == DC - 1)
                        ge = gt.tile([128, S], st, tag="ge", name="ge")
                        nc.scalar.activation(ge[:], gh[:, 0:S], A.Gelu,
                                             bias=bias("b1")[:, mI:mI + 1])
                        mT = mp.tile([128, S], st, tag="mT", name="mT")
                        nc.vector.scalar_tensor_tensor(mT[:], gh[:, S:2 * S],
                                                       bias("b3")[:, mI:mI + 1],
                                                       ge[:], OP.add, OP.mult)
                        # fused down-projection: w2 already resident
                        for t in range(QC):
                            for n in range(2):
                                mm(y_ps[t][n][:], mT[:, t * 128:(t + 1) * 128],
                                   w2_all[:, mI * D + n * 512:
                                          mI * D + (n + 1) * 512],
                                   mI == 0, mI == FC - 1)
                for t in range(QC):
                    for n in range(2):
                        o = op_.tile([128, 512], f32, tag="o_out", name="o_out")
                        nc.vector.tensor_scalar_mul(o[:], y_ps[t][n][:],
                                                    bias("c")[:, 0:1])
                        eng_s.dma_start(
                            out_ffn.ap()[t * 128:(t + 1) * 128,
                                         n * 512:(n + 1) * 512], o[:])

    nc.compile()
    return nc


class _FixedPool:
    """Adapter letting project_fm write into pre-allocated tiles."""

    def __init__(self, tiles):
        self._tiles = list(tiles)
        self._i = 0

    def tile(self, shape, dtype, tag=None, name=None):
        t = self._tiles[self._i]
        self._i += 1
        return t


def _routing(langs):
    """Per-sequence expert slots [(expert_idx, coef) x2], matching the
    reference: coef[e,b] = any(langs[b]==4+e) * (1/count(langs[b]>3))."""
    langs = np.asarray(langs)
    slots = []
    for b in range(langs.shape[0]):
        row = [int(v) for v in langs[b]]
        cnt = sum(1 for v in row if v > 3)
        rw = 1.0 if cnt == 0 else 1.0 / cnt
        seen = []
        for v in row:
            if v > 3 and 0 <= v - 4 < NE and (v - 4) not in seen:
                seen.append(v - 4)
        sl = [(e, rw) for e in seen]
        while len(sl) < 2:
            sl.append((0, 0.0))
        slots.append(sl[:2])
    return slots


def _mask_classes(maskT, n_kc):
    """Classify each [128 keys x 128 queries] block of a transposed mask:
    0 all-zero (no add), 1 general (add), 2 fully masked (skip compute).
    Keeps at least one active key block per query and contiguous active
    ranges per key chunk."""
    cls = []
    for kc in range(n_kc):
        for qh in range(QC):
            blk = maskT[kc * 128:(kc + 1) * 128, qh * 128:(qh + 1) * 128]
            if np.all(blk == 0):
                cls.append(0)
            elif np.all(blk <= -1e8):
                cls.append(2)
            else:
                cls.append(1)
    for qh in range(QC):
        if all(cls[kc * QC + qh] == 2 for kc in range(n_kc)):
            for kc in range(n_kc):
                cls[kc * QC + qh] = 1
    for kc in range(n_kc):
        act = [q for q in range(QC) if cls[kc * QC + q] != 2]
        if not act or act != list(range(act[0], act[-1] + 1)):
            for q in range(QC):
                if cls[kc * QC + q] == 2:
                    cls[kc * QC + q] = 1
    return tuple(cls)


def kernel(**inputs):
    mode = MODE
    np_dt = ml_dtypes.bfloat16 if mode == "bf16" else np.float32
    f32 = np.float32

    inp = {k: np.asarray(v) for k, v in inputs.items()}
    x = inp["hidden_states"].astype(f32)
    enc = inp["encoder_hidden_states"].astype(f32)
    mask = inp["attention_mask"].astype(f32)
    encmask = inp["encoder_attention_mask"].astype(f32)
    g1, b1 = inp["ln1_g"].astype(f32), inp["ln1_b"].astype(f32)
    g2, b2 = inp["ln2_g"].astype(f32), inp["ln2_b"].astype(f32)
    g3, b3 = inp["ln3_g"].astype(f32), inp["ln3_b"].astype(f32)

    def dup_kv(w):
        return np.concatenate([np.tile(w[:, 64 * j:64 * (j + 1)], (1, 2))
                               for j in range(NKV)], axis=1)

    def dup_kv_b(v):
        return np.concatenate([np.tile(v[64 * j:64 * (j + 1)], 2)
                               for j in range(NKV)])

    sc = HD ** -0.5
    qw_f = g1[:, None] * inp["sa_q_w"] * sc
    qb_f = (b1 @ inp["sa_q_w"] + inp["sa_q_b"]) * sc
    kw_f = dup_kv(g1[:, None] * inp["sa_k_w"])
    kb_f = dup_kv_b(b1 @ inp["sa_k_w"] + inp["sa_k_b"])
    vw_f = g1[:, None] * inp["sa_v_w"]
    vb_f = b1 @ inp["sa_v_w"] + inp["sa_v_b"]
    q2w_f = g2[:, None] * inp["ca_q_w"] * sc
    q2b_f = (b2 @ inp["ca_q_w"] + inp["ca_q_b"]) * sc
    k2w_f = dup_kv(inp["ca_k_w"])
    k2b_f = dup_kv_b(inp["ca_k_b"])
    w1_f = inp["moe_w1"] * g3[None, :, None]
    b1_f = np.einsum("d,edf->ef", b3, inp["moe_w1"]).astype(f32)
    w3_f = inp["moe_w3"] * g3[None, :, None]
    b3_f = np.einsum("d,edf->ef", b3, inp["moe_w3"]).astype(f32)

    maskT0 = np.ascontiguousarray(mask[:, 0].transpose(0, 2, 1))     # [B,S,S]
    encmaskT0 = np.ascontiguousarray(encmask[:, 0].transpose(0, 2, 1))
    sa_cls = _mask_classes(maskT0[0], SC)
    ca_cls = _mask_classes(encmaskT0[0], KC)
    for b in range(1, B):
        if _mask_classes(maskT0[b], SC) != sa_cls or \
           _mask_classes(encmaskT0[b], KC) != ca_cls:
            sa_cls = tuple(1 for _ in range(SC * QC))
            ca_cls = tuple(1 for _ in range(KC * QC))
            break

    key = (mode, sa_cls, ca_cls)
    if key not in _CACHE:
        _CACHE[key] = _build(mode, sa_cls, ca_cls)
    nc = _CACHE[key]

    def col128(v):
        return np.asarray(v, f32).reshape(-1, 128).T

    def pack_k(w):
        w = np.asarray(w)
        return np.concatenate([w[k * 128:(k + 1) * 128, :]
                               for k in range(w.shape[0] // 128)], axis=1)

    slots = _routing(inp["langs"])
    # block c selects den row 32*(c%4) of den tile c//4
    onehot = np.zeros((128, 8 * 128), f32)
    for c in range(8):
        onehot[32 * (c % 4), c * 128:(c + 1) * 128] = 1.0
    wqkv = np.concatenate([qw_f, kw_f, vw_f], axis=1).astype(np_dt)
    wca = np.concatenate([q2w_f, k2w_f, inp["ca_v_w"]], axis=1).astype(np_dt)

    bias_common = np.zeros((128, BIAS_W), f32)
    for nm, v in [("qb", col128(qb_f)), ("kb", col128(kb_f)),
                  ("vb", np.broadcast_to(vb_f.astype(f32), (128, KVW))),
                  ("ob", col128(inp["sa_o_b"])),
                  ("q2b", col128(q2b_f)), ("k2b", col128(k2b_f)),
                  ("v2b", np.broadcast_to(inp["ca_v_b"].astype(f32), (128, KVW))),
                  ("o2b", col128(inp["ca_o_b"]))]:
        off, w = _BIAS_COLS[nm]
        bias_common[:, off:off + w] = v

    in_maps = []
    for c in range(8):
        b, j = c // 2, c % 2
        e, coef = slots[b][j]
        xTp = pack_k(np.ascontiguousarray(x[b].T))
        # interleave w1/w3 by group: [w1 grp g | w3 grp g] blocks of 512 cols
        gw = GRP * 128
        w13 = np.empty((128, NGRP * DC * 1024), f32)
        for g in range(NGRP):
            for k in range(DC):
                c0 = (g * DC + k) * 1024
                w13[:, c0:c0 + 512] = w1_f[e][k * 128:(k + 1) * 128,
                                              g * 512:(g + 1) * 512]
                w13[:, c0 + 512:c0 + 1024] = w3_f[e][k * 128:(k + 1) * 128,
                                                     g * 512:(g + 1) * 512]
        bt = bias_common.copy()
        for nm, v in [("b1", col128(b1_f[e])), ("b3", col128(b3_f[e]))]:
            off, w = _BIAS_COLS[nm]
            bt[:, off:off + w] = v
        bt[:, _BIAS_COLS["c"][0]] = coef
        m = {
            "encT": pack_k(np.ascontiguousarray(enc[b].T)).astype(np_dt),
            "id128": np.eye(128, dtype=f32).astype(np_dt),
            "onehot": onehot.astype(np_dt),
            "ones_col": np.ones((128, 1), f32).astype(np_dt),
            "ones_row": np.ones((1, 128), f32).astype(np_dt),
            "wqkv": pack_k(wqkv), "wca": pack_k(wca),
            "ow": pack_k(inp["sa_o_w"].astype(np_dt)),
            "o2w": pack_k(inp["ca_o_w"].astype(np_dt)),
            "biases": bt,
            "w13": w13.astype(np_dt),
            "w2": pack_k(np.ascontiguousarray(inp["moe_w2"][e])).astype(np_dt),
        }
        m["xT_st"] = xTp.astype(np_dt)
        if any(cc == 1 for cc in sa_cls):
            m["maskT"] = np.exp(maskT0[b]).astype(np_dt)
        if any(cc == 1 for cc in ca_cls):
            m["encmaskT"] = np.exp(encmaskT0[b]).astype(np_dt)
        in_maps.append(m)

    kw = {}
    if _TRACE_DIR:
        kw = dict(trace=True, tmpdir=_TRACE_DIR, trace_cores=[0])
    res = bass_utils.run_bass_kernel_spmd(nc, in_maps, core_ids=list(range(8)), **kw)
    global _LAST_EXEC_NS
    _LAST_EXEC_NS = res.exec_time_ns
    return np.stack([
        res.results[2 * b]["out_res"].T
        + res.results[2 * b]["out_ffn"]
        + res.results[2 * b + 1]["out_ffn"]
        for b in range(B)
    ]).astype(f32)



# revision 34
# speedup vs baseline: 1.2518x; 1.0160x over previous
"""MBart MoE decoder layer on 8 trn2 NeuronCores.

Sharding: 8 cores = 8 (sequence, expert-slot) pairs. Core c handles
sequence b=c//2, expert slot j=c%2 (each sequence is lang-routed to at
most 2 distinct experts; routing is computed on the host from `langs`).
Each core computes the full attention path for its sequence (replicated
across the pair) and one expert FFN over all 256 tokens; the host sums
the pair's partial outputs (expert-sharded combine) and transposes back
to token-major. Expert weights are gathered per-core on the host, so a
core only receives the one expert it needs.

On-device layout is feature-major [D, tokens]: projections take weights
as lhsT (feature-major out) or activations as lhsT (token-major out), so
no activation transposes are needed anywhere. LN gains/biases are folded
into the downstream weights on the host; softmax uses transposed scores
[keys, queries] with the attention mask added via an identity-matmul
into PSUM (host classifies each 128x128 mask block as zero / add / skip,
so causal dead blocks are never computed) and denominators accumulated
via a ones-matmul, then broadcast over partitions with a rank-1 matmul
for one full-lane reciprocal per head pair.
"""

import os
import sys
from contextlib import ExitStack

for _p in ("/opt/trn_rl_repo",):
    if _p not in sys.path:
        sys.path.append(_p)

import numpy as np
import ml_dtypes

import concourse.bass as bass
import concourse.tile as tile
import concourse.mybir as mybir
from concourse import bacc, bass_utils

B, S, SK = 4, 256, 512
D, NH, NKV, HD = 1024, 16, 4, 64
DE, NE = 4096, 8
LN_EPS = 1e-5
REP = NH // NKV
DC = D // 128    # 8 feature chunks
FC = DE // 128   # 32 ffn chunks
SC = S // 128    # 2 self-attn key chunks
KC = SK // 128   # 4 cross-attn key chunks
QC = S // 128    # 2 query halves
KVW = NKV * HD   # 256
GRP = 4          # ffn chunks per MoE weight group
NGRP = FC // GRP

MODE = os.environ.get("KERNEL_MM_DTYPE", "bf16")  # "bf16" | "f32r" | "f32"

_CACHE: dict = {}
_TRACE_DIR = None   # set by test harness for profiling runs
_LAST_EXEC_NS = None

# packed attention-weight column layout: qw | kw(dup) | vw
W_Q, W_K, W_V = 0, D, D + 2 * KVW
WPACK = D + 2 * KVW + KVW  # 1792

# packed per-partition bias column layout
_BIAS_COLS = {}
_off = 0
for _n, _w in [("qb", DC), ("kb", 4), ("vb", KVW), ("ob", DC),
               ("q2b", DC), ("k2b", 4), ("v2b", KVW), ("o2b", DC),
               ("b1", FC), ("b3", FC), ("c", 1)]:
    _BIAS_COLS[_n] = (_off, _w)
    _off += _w
BIAS_W = _off


def _build(mode, sa_cls, ca_cls):
    """sa_cls/ca_cls: block classes per (kc, qhalf): 0=no-mask, 1=mask-add,
    2=fully-masked(skip)."""
    st = {"bf16": mybir.dt.bfloat16, "f32r": mybir.dt.float32r,
          "f32": mybir.dt.float32}[mode]
    f32 = mybir.dt.float32
    same_st = mode == "f32"
    A = mybir.ActivationFunctionType
    OP = mybir.AluOpType

    nc = bacc.Bacc("TRN2", target_bir_lowering=False, debug=False, num_devices=8)
    import os as _os
    _SPLIT = _os.environ.get("KERNEL_DMA_SPLIT", "1") == "1"
    eng_b = nc.scalar if _SPLIT else nc.sync
    eng_s = nc.gpsimd if _SPLIT else nc.sync

    def mm(psum, lhsT, rhs, start, stop):
        nc.tensor.matmul(psum, lhsT, rhs, start=start, stop=stop)

    di = {}

    def din(name, shape, dtype=None):
        di[name] = nc.dram_tensor(name, list(shape), dtype or st, kind="ExternalInput")
        return di[name]

    din("xT_st", (128, DC * S))
    din("encT", (128, DC * SK))
    need_samask = any(c == 1 for c in sa_cls)
    need_camask = any(c == 1 for c in ca_cls)
    if need_samask:
        din("maskT", (S, S))
    if need_camask:
        din("encmaskT", (SK, S))
    din("id128", (128, 128))
    din("onehot", (128, 8 * 128))
    din("ones_col", (128, 1))
    din("ones_row", (1, 128))
    din("wq", (128, DC * D))
    din("wkv", (128, DC * (WPACK - D)))
    din("wca", (128, DC * WPACK))
    din("ow", (128, DC * D))
    din("o2w", (128, DC * D))
    din("biases", (128, BIAS_W), f32)
    din("w13", (128, NGRP * DC * 1024))  # per grp, per k: [w1 512 | w3 512]
    din("w2", (128, FC * D))
    out_res = nc.dram_tensor("out_res", [D, S], f32, kind="ExternalOutput")
    out_ffn = nc.dram_tensor("out_ffn", [S, D], st, kind="ExternalOutput")

    with tile.TileContext(nc) as tc, ExitStack() as ctx:
        cp = ctx.enter_context(tc.tile_pool(name="consts", bufs=1))
        pers = ctx.enter_context(tc.tile_pool(name="pers", bufs=1))
        xp = ctx.enter_context(tc.tile_pool(name="xpool", bufs=1))

        ones128 = cp.tile([128, 1], st, tag="ones128", name="ones128")
        eng_b.dma_start(ones128[:], di["ones_col"].ap())
        ones1r = cp.tile([1, 128], st, tag="ones1r", name="ones1r")
        eng_b.dma_start(ones1r[:], di["ones_row"].ap())
        eps_t = cp.tile([128, 1], f32, tag="eps_t", name="eps_t")
        nc.vector.memset(eps_t, LN_EPS)
        id128 = cp.tile([128, 128], st, tag="id128", name="id128")
        eng_b.dma_start(id128[:], di["id128"].ap())
        onehot = cp.tile([128, 8 * 128], st, tag="onehot", name="onehot")
        eng_b.dma_start(onehot[:], di["onehot"].ap())
        maskT = encmaskT = None
        if need_samask:
            maskT = cp.tile([128, SC, S], st, tag="maskT", name="maskT")
            for kc in range(SC):
                eng_b.dma_start(maskT[:, kc, :],
                                  di["maskT"].ap()[kc * 128:(kc + 1) * 128, :])
        if need_camask:
            encmaskT = cp.tile([128, KC, S], st, tag="encmaskT", name="encmaskT")
            for kc in range(KC):
                eng_b.dma_start(encmaskT[:, kc, :],
                                  di["encmaskT"].ap()[kc * 128:(kc + 1) * 128, :])

        bias_t = cp.tile([128, BIAS_W], f32, tag="bias_t", name="bias_t")
        eng_b.dma_start(bias_t[:], di["biases"].ap())

        def bias(nm):
            off, w = _BIAS_COLS[nm]
            return bias_t[:, off:off + w]

        def load_chunks(dram, nchunk, width, tag, pool, dtype=st, engine=None,
                        after=None, inst_out=None):
            t = pool.tile([128, nchunk * width], dtype, tag=tag, name=tag)
            inst = (engine or nc.sync).dma_start(t[:], dram.ap())
            if after is not None:
                tile.add_dep_helper(inst.ins, after, sync=True,
                                    reason="dma priority order")
            if inst_out is not None:
                inst_out.append(inst.ins)
            return [t[:, k * width:(k + 1) * width] for k in range(nchunk)]

        def layernorm(src_f32, src_st, out_tag, pool):
            """src: DC chunks [128,S] f32 (+st copies). Returns DC normalized
            chunks [128,S] st (gain/bias folded downstream by host)."""
            with tc.tile_pool(name=f"{out_tag}_lt", bufs=2) as lp, \
                 tc.tile_pool(name=f"{out_tag}_lp", bufs=1, space="PSUM") as sp, \
                 tc.tile_pool(name=f"{out_tag}_lb", bufs=1, space="PSUM") as bp:
                sum_ps = sp.tile([1, S], f32, tag="lnsum", name="lnsum")
                sq_ps = sp.tile([1, S], f32, tag="lnsq", name="lnsq")
                for k in range(DC):
                    sq = lp.tile([128, S], st, tag="lnsqt", name="lnsqt")
                    nc.vector.tensor_tensor(sq[:], src_st[k][:], src_st[k][:],
                                            OP.mult)
                    mm(sum_ps[:], ones128[:], src_st[k][:], k == 0, k == DC - 1)
                    mm(sq_ps[:], ones128[:], sq[:], k == 0, k == DC - 1)
                s_sb = lp.tile([1, S], st, tag="ln_ssb", name="ln_ssb")
                nc.vector.tensor_single_scalar(s_sb[:], sum_ps[:], 1.0 / D, OP.mult)
                q_sb = lp.tile([1, S], st, tag="ln_qsb", name="ln_qsb")
                nc.vector.tensor_single_scalar(q_sb[:], sq_ps[:], 1.0 / D, OP.mult)
                s_bc = bp.tile([128, S], f32, tag="ln_sbc", name="ln_sbc")
                q_bc = bp.tile([128, S], f32, tag="ln_qbc", name="ln_qbc")
                mm(s_bc[:], ones1r[:], s_sb[:], True, True)   # mean, bcast
                mm(q_bc[:], ones1r[:], q_sb[:], True, True)   # E[x^2], bcast
                # var = E[x^2] - mean^2; rstd/mean*rstd in st so the
                # per-chunk normalize runs in the DVE 16-bit fast mode
                m2 = lp.tile([128, S], f32, tag="ln_m2", name="ln_m2")
                nc.scalar.activation(m2[:], s_bc[:], A.Square)
                var = lp.tile([128, S], f32, tag="ln_var", name="ln_var")
                nc.vector.tensor_sub(var[:], q_bc[:], m2[:])
                v_t = lp.tile([128, S], st, tag="ln_vt", name="ln_vt")
                nc.scalar.activation(v_t[:], var[:], A.Abs_reciprocal_sqrt,
                                     bias=eps_t[:])
                u_t = lp.tile([128, S], st, tag="ln_ut", name="ln_ut")
                nc.vector.tensor_tensor(u_t[:], s_bc[:], v_t[:], OP.mult)
                outs = []
                for k in range(DC):
                    o = pool.tile([128, S], st, tag=f"{out_tag}{k}",
                                  name=f"{out_tag}{k}")
                    nc.vector.tensor_tensor(o[:], src_st[k][:], v_t[:], OP.mult)
                    nc.vector.tensor_sub(o[:], o[:], u_t[:])
                    outs.append(o)
                return outs

        def cast_st(src, tag, pool):
            if same_st:
                return src
            outs = []
            for k, t in enumerate(src):
                o = pool.tile([128, t.shape[-1]], st, tag=f"{tag}{k}",
                              name=f"{tag}{k}")
                nc.vector.tensor_copy(o[:], t[:])
                outs.append(o)
            return outs

        def project_fm(w_slices, rhs_chunks, nout, bias_ap, out_tag, pool,
                       extra=None, out_dt=None, width=None):
            """out^T[dout_chunk] = sum_k w_slices[k][:, m*128:...].T @ rhs[k]."""
            W = width or S
            outs = []
            with tc.tile_pool(name=f"{out_tag}_ps", bufs=3, space="PSUM") as pp:
                for mI in range(nout):
                    ps = pp.tile([128, W], f32, tag="proj", name="proj")
                    for k in range(DC):
                        mm(ps[:], w_slices[k][:, mI * 128:(mI + 1) * 128],
                           rhs_chunks[k][:], k == 0, k == DC - 1)
                    o = pool.tile([128, W], out_dt or st, tag=f"{out_tag}{mI}",
                                  name=f"{out_tag}{mI}")
                    if extra is not None:
                        extra(mI, ps, o)
                    elif bias_ap is not None:
                        nc.vector.tensor_scalar(o[:], ps[:],
                                                bias_ap[:, mI:mI + 1], None,
                                                OP.add)
                    else:
                        nc.vector.tensor_copy(o[:], ps[:])
                    outs.append(o)
            return outs

        def project_tm(act_chunks, w_slices, ntok, bias_bcast, out_tag, pool):
            """token-major V with a ones column appended per kv head:
            out[tok_chunk] = [V_kv | 1] blocks of 65 columns."""
            outs = []
            with tc.tile_pool(name=f"{out_tag}_ps", bufs=3, space="PSUM") as pp:
                for t in range(ntok):
                    ps = pp.tile([128, KVW], f32, tag="projtm", name="projtm")
                    for k in range(DC):
                        mm(ps[:], act_chunks[k][:, t * 128:(t + 1) * 128],
                           w_slices[k][:], k == 0, k == DC - 1)
                    o = pool.tile([128, NKV, HD + 1], st, tag=f"{out_tag}{t}",
                                  name=f"{out_tag}{t}")
                    nc.vector.tensor_add(
                        o[:, :, 0:HD],
                        ps[:].rearrange("p (kv d) -> p kv d", kv=NKV),
                        bias_bcast[:].rearrange("p (kv d) -> p kv d", kv=NKV))
                    for kv in range(NKV):
                        nc.vector.tensor_copy(o[:, kv, HD:HD + 1], ones128[:])
                    outs.append(o)
            return outs

        def attend(qT, kT, vtm, n_kc, mask_tile, cls, out_tag, pool):
            """Baseline-structure attention; mask applied multiplicatively on
            the DVE (host ships exp(mask)) instead of via id128 matmuls."""
            outs = []
            qr = []
            for kc in range(n_kc):
                act = [qh for qh in range(QC) if cls[kc * QC + qh] != 2]
                assert act and act == list(range(act[0], act[-1] + 1))
                qr.append((act[0] * 128, (act[-1] + 1) * 128))
            with tc.tile_pool(name=f"{out_tag}_sp", bufs=3, space="PSUM") as stp, \
                 tc.tile_pool(name=f"{out_tag}_op", bufs=2, space="PSUM") as opp, \
                 tc.tile_pool(name=f"{out_tag}_bp", bufs=1, space="PSUM") as bpp, \
                 tc.tile_pool(name=f"{out_tag}_et", bufs=5) as epool, \
                 tc.tile_pool(name=f"{out_tag}_un", bufs=1) as upool, \
                 tc.tile_pool(name=f"{out_tag}_dt", bufs=1) as dpool:
                # denominators staged at 32-aligned partitions: tile i holds
                # head pairs 4i..4i+3 at rows {0,32,64,96}
                den_t = [dpool.tile([128, 2 * S], st, tag=f"den_t{i}",
                                    name=f"den_t{i}") for i in range(2)]
                for i in range(2):   # unwritten rows must stay finite
                    nc.vector.memset(den_t[i], 1.0)
                o_un = []
                for c in range(DC):
                    o_ps_h = [opp.tile([65, S], f32, tag=f"oph{hh}",
                                       name=f"oph{hh}") for hh in range(2)]
                    kv = (2 * c) // REP      # same kv head for both of the pair
                    for kc in range(n_kc):
                        q0, q1 = qr[kc]
                        adds = [q for q in range(QC) if cls[kc * QC + q] == 1]
                        st_h = []
                        e_h = []
                        for hh in range(2):
                            qh_ap = qT[c][hh * 64:(hh + 1) * 64, :]
                            kh = kT[kv][hh * 64:(hh + 1) * 64, :]
                            st_ps = stp.tile([128, S], f32, tag="st",
                                             name="st")
                            mm(st_ps[:, q0:q1], kh[:, kc * 128:(kc + 1) * 128],
                               qh_ap[:, q0:q1], True, True)
                            st_h.append(st_ps)
                        for hh in range(2):
                            e = epool.tile([128, S], st, tag="e", name="e")
                            nc.scalar.activation(e[:, q0:q1],
                                                 st_h[hh][:, q0:q1], A.Exp)
                            for q in adds:
                                nc.vector.tensor_tensor(
                                    e[:, q * 128:(q + 1) * 128],
                                    e[:, q * 128:(q + 1) * 128],
                                    mask_tile[:, kc, q * 128:(q + 1) * 128],
                                    OP.mult)
                            e_h.append(e)
                        for hh in range(2):
                            mm(o_ps_h[hh][:, q0:q1],
                               vtm[kc][:, kv, :], e_h[hh][:, q0:q1],
                               kc == 0, kc == n_kc - 1)
                    # evacuate unnormalized O + denominators; frees PSUM fast
                    ou = upool.tile([128, S], st, tag=f"un{c}", name=f"un{c}")
                    row = 32 * (c % 4)
                    for hh in range(2):
                        nc.vector.tensor_copy(ou[hh * 64:(hh + 1) * 64, :],
                                              o_ps_h[hh][0:64, :])
                        nc.vector.tensor_copy(
                            den_t[c // 4][row:row + 1, hh * S:(hh + 1) * S],
                            o_ps_h[hh][64:65, :])
                    o_un.append(ou)
                # ONE rsqrt pass for the whole attention (no ACT table thrash)
                den_sq = den_t
                for i in range(2):
                    nc.scalar.activation(den_t[i][:], den_t[i][:],
                                         A.Abs_reciprocal_sqrt)
                    nc.vector.tensor_tensor(den_t[i][:], den_t[i][:],
                                            den_t[i][:], OP.mult)
                for c in range(DC):
                    r_ps = bpp.tile([128, 2 * S], f32, tag="rbc", name="rbc")
                    mm(r_ps[:], onehot[:, c * 128:(c + 1) * 128],
                       den_sq[c // 4][:], True, True)
                    o = pool.tile([128, S], st, tag=f"{out_tag}{c}",
                                  name=f"{out_tag}{c}")
                    for hh in range(2):
                        nc.vector.tensor_tensor(
                            o[hh * 64:(hh + 1) * 64, :],
                            o_un[c][hh * 64:(hh + 1) * 64, :],
                            r_ps[hh * 64:(hh + 1) * 64, hh * S:(hh + 1) * S],
                            OP.mult)
                    outs.append(o)
            return outs

        h1 = [pers.tile([128, S], f32, tag=f"h1T{k}", name=f"h1T{k}")
              for k in range(DC)]
        h2 = [pers.tile([128, S], f32, tag=f"h2T{k}", name=f"h2T{k}")
              for k in range(DC)]

        # w2 fully resident before the MoE starts (4 sliced DMAs on the
        # store queue); gated behind the attention weight loads so the
        # prefetch never starves first-needed transfers
        w2_all = pers.tile([128, FC * D], st, tag="w2_all", name="w2_all")
        GW13 = DC * 1024
        w13e = [pers.tile([128, GW13], st, tag=f"w13e{g}", name=f"w13e{g}")
                for g in range(1)]

        def _issue_w2():
            # sync-queue FIFO after the attention weights: fills the DMA-idle
            # attention window without starving first-needed loads
            for q in range(4):
                w = FC * D // 4
                nc.sync.dma_start(w2_all[:, q * w:(q + 1) * w],
                                  di["w2"].ap()[:, q * w:(q + 1) * w])
            for g in range(1):
                nc.sync.dma_start(w13e[g][:],
                                  di["w13"].ap()[:, g * GW13:(g + 1) * GW13])

        # ---------------- self attention ----------------
        # sync-queue DMA order == need order: consumers' DMA waits are
        # coarsened to later completions on the same lane, so any transfer
        # queued out of need-order delays every later consumer.
        with tc.tile_pool(name="sa_acts", bufs=1) as sa:
            xT_st = load_chunks(di["xT_st"], DC, S, "xTs", sa)
            n1 = layernorm(xT_st, xT_st, "n1T", sa)
            with tc.tile_pool(name="wqkvp", bufs=1) as wp:
                wq_t = load_chunks(di["wq"], DC, D, "wq", wp)
                wkv_t = load_chunks(di["wkv"], DC, WPACK - D, "wkv", wp)
                qT = project_fm(wq_t, n1, DC, bias("qb"), "qT", sa)
                kT = project_fm([t[:, 0:2 * KVW] for t in wkv_t], n1, 4,
                                bias("kb"), "kT", sa)
                v_tm = project_tm(n1, [t[:, 2 * KVW:3 * KVW] for t in wkv_t], SC,
                                  bias("vb"), "v_tm", sa)
            # cross-attn K/V only need encT — loaded and computed here so the
            # PE has filler work during self-attend softmax stalls and the
            # cross weights are queued right after wqkv
            encT = load_chunks(di["encT"], DC, SK, "encT", pers)
            wca = load_chunks(di["wca"], DC, WPACK, "wca", pers)
            k2T = project_fm([t[:, W_K:W_K + 2 * KVW] for t in wca], encT, 4,
                             bias("k2b"), "k2T", pers, width=SK)
            v2_tm = project_tm(encT, [t[:, W_V:W_V + KVW] for t in wca], KC,
                               bias("v2b"), "v2_tm", pers)
            with tc.tile_pool(name="wop", bufs=1) as wp:
                ow_t = load_chunks(di["ow"], DC, D, "ow", wp)
                sa_out = attend(qT, kT, v_tm, SC, maskT, sa_cls, "saT", sa)

                def o_epil(mI, ps, o):
                    nc.vector.scalar_tensor_tensor(o[:], ps[:],
                                                   bias("ob")[:, mI:mI + 1],
                                                   xT_st[mI][:], OP.add, OP.add)
                project_fm(ow_t, sa_out, DC, None, "h1w", _FixedPool(h1),
                           extra=o_epil, out_dt=f32)

        # ---------------- cross attention ----------------
        with tc.tile_pool(name="ca_acts", bufs=1) as ca:
            _issue_w2()   # MoE prefetch: queued after all attention weights
            h1_st = cast_st(h1, "h1s", ca)
            n2 = layernorm(h1, h1_st, "n2T", ca)
            q2T = project_fm([t[:, W_Q:W_Q + D] for t in wca], n2, DC,
                             bias("q2b"), "q2T", ca)
            with tc.tile_pool(name="wo2p", bufs=1) as wp:
                o2w_t = load_chunks(di["o2w"], DC, D, "o2w", wp)
                ca_out = attend(q2T, k2T, v2_tm, KC, encmaskT, ca_cls, "caT", ca)

                def o2_epil(mI, ps, o):
                    nc.vector.scalar_tensor_tensor(o[:], ps[:],
                                                   bias("o2b")[:, mI:mI + 1],
                                                   h1[mI][:], OP.add, OP.add)
                project_fm(o2w_t, ca_out, DC, None, "h2w", _FixedPool(h2),
                           extra=o2_epil, out_dt=f32)

        # residual output (host: out_b = res.T + ffn_j0 + ffn_j1)
        for k in range(DC):
            eng_s.dma_start(out_res.ap()[k * 128:(k + 1) * 128, :], h2[k][:])

        # ---------------- MoE expert ----------------
        with tc.tile_pool(name="moe_acts", bufs=1) as mo:
            h2_st = cast_st(h2, "h2s", mo)
            n3 = layernorm(h2, h2_st, "n3T", mo)

            with tc.tile_pool(name="w13p", bufs=2) as wp, \
                 tc.tile_pool(name="mTp", bufs=4) as mp, \
                 tc.tile_pool(name="gh_ps", bufs=3, space="PSUM") as gp, \
                 tc.tile_pool(name="y_ps", bufs=1, space="PSUM") as yp, \
                 tc.tile_pool(name="gelu_t", bufs=3) as gt, \
                 tc.tile_pool(name="outp", bufs=2) as op_:
                y_ps = [[yp.tile([128, 512], f32, tag=f"y{t}{n}", name=f"y{t}{n}")
                         for n in range(2)] for t in range(QC)]
                gw = GRP * 128
                GW = DC * 1024
                for g in range(NGRP):
                    if g < 1:
                        wgt = w13e[g]
                    else:
                        wgt = wp.tile([128, GW], st, tag="w13g", name="w13g")
                        nc.sync.dma_start(wgt[:],
                                          di["w13"].ap()[:, g * GW:(g + 1) * GW])
                    wg = [wgt[:, k * 1024:(k + 1) * 1024] for k in range(DC)]
                    for mi in range(GRP):
                        mI = g * GRP + mi
                        # one PSUM bank: gelu-arg in [0:S], mult-arg in [S:2S]
                        gh = gp.tile([128, 2 * S], f32, tag="gh", name="gh")
                        for k in range(DC):
                            mm(gh[:, 0:S], wg[k][:, mi * 128:(mi + 1) * 128],
                               n3[k][:], k == 0, k == DC - 1)
                        for k in range(DC):
                            mm(gh[:, S:2 * S],
                               wg[k][:, gw + mi * 128:gw + (mi + 1) * 128],
                               n3[k][:], k == 0, k == DC - 1)
                        ge = gt.tile([128, S], st, tag="ge", name="ge")
                        nc.scalar.activation(ge[:], gh[:, 0:S], A.Gelu,
                                             bias=bias("b1")[:, mI:mI + 1])
                        mT = mp.tile([128, S], st, tag="mT", name="mT")
                        nc.vector.scalar_tensor_tensor(mT[:], gh[:, S:2 * S],
                                                       bias("b3")[:, mI:mI + 1],
                                                       ge[:], OP.add, OP.mult)
                        # fused down-projection: w2 already resident
                        for t in range(QC):
                            for n in range(2):
                                mm(y_ps[t][n][:], mT[:, t * 128:(t + 1) * 128],
                                   w2_all[:, mI * D + n * 512:
                                          mI * D + (n + 1) * 512],
                                   mI == 0, mI == FC - 1)
                for t in range(QC):
                    for n in range(2):
                        o = op_.tile([128, 512], st, tag="o_out", name="o_out")
                        nc.vector.tensor_scalar_mul(o[:], y_ps[t][n][:],
                                                    bias("c")[:, 0:1])
                        eng_s.dma_start(
                            out_ffn.ap()[t * 128:(t + 1) * 128,
                                         n * 512:(n + 1) * 512], o[:])

    nc.compile()
    return nc


class _FixedPool:
    """Adapter letting project_fm write into pre-allocated tiles."""

    def __init__(self, tiles):
        self._tiles = list(tiles)
        self._i = 0

    def tile(self, shape, dtype, tag=None, name=None):
        t = self._tiles[self._i]
        self._i += 1
        return t


def _routing(langs):
    """Per-sequence expert slots [(expert_idx, coef) x2], matching the
    reference: coef[e,b] = any(langs[b]==4+e) * (1/count(langs[b]>3))."""
    langs = np.asarray(langs)
    slots = []
    for b in range(langs.shape[0]):
        row = [int(v) for v in langs[b]]
        cnt = sum(1 for v in row if v > 3)
        rw = 1.0 if cnt == 0 else 1.0 / cnt
        seen = []
        for v in row:
            if v > 3 and 0 <= v - 4 < NE and (v - 4) not in seen:
                seen.append(v - 4)
        sl = [(e, rw) for e in seen]
        while len(sl) < 2:
            sl.append((0, 0.0))
        slots.append(sl[:2])
    return slots


def _mask_classes(maskT, n_kc):
    """Classify each [128 keys x 128 queries] block of a transposed mask:
    0 all-zero (no add), 1 general (add), 2 fully masked (skip compute).
    Keeps at least one active key block per query and contiguous active
    ranges per key chunk."""
    cls = []
    for kc in range(n_kc):
        for qh in range(QC):
            blk = maskT[kc * 128:(kc + 1) * 128, qh * 128:(qh + 1) * 128]
            if np.all(blk == 0):
                cls.append(0)
            elif np.all(blk <= -1e8):
                cls.append(2)
            else:
                cls.append(1)
    for qh in range(QC):
        if all(cls[kc * QC + qh] == 2 for kc in range(n_kc)):
            for kc in range(n_kc):
                cls[kc * QC + qh] = 1
    for kc in range(n_kc):
        act = [q for q in range(QC) if cls[kc * QC + q] != 2]
        if not act or act != list(range(act[0], act[-1] + 1)):
            for q in range(QC):
                if cls[kc * QC + q] == 2:
                    cls[kc * QC + q] = 1
    return tuple(cls)


def kernel(**inputs):
    mode = MODE
    np_dt = ml_dtypes.bfloat16 if mode == "bf16" else np.float32
    f32 = np.float32

    inp = {k: np.asarray(v) for k, v in inputs.items()}
    x = inp["hidden_states"].astype(f32)
    enc = inp["encoder_hidden_states"].astype(f32)
    mask = inp["attention_mask"].astype(f32)
    encmask = inp["encoder_attention_mask"].astype(f32)
    g1, b1 = inp["ln1_g"].astype(f32), inp["ln1_b"].astype(f32)
    g2, b2 = inp["ln2_g"].astype(f32), inp["ln2_b"].astype(f32)
    g3, b3 = inp["ln3_g"].astype(f32), inp["ln3_b"].astype(f32)

    def dup_kv(w):
        return np.concatenate([np.tile(w[:, 64 * j:64 * (j + 1)], (1, 2))
                               for j in range(NKV)], axis=1)

    def dup_kv_b(v):
        return np.concatenate([np.tile(v[64 * j:64 * (j + 1)], 2)
                               for j in range(NKV)])

    sc = HD ** -0.5
    qw_f = g1[:, None] * inp["sa_q_w"] * sc
    qb_f = (b1 @ inp["sa_q_w"] + inp["sa_q_b"]) * sc
    kw_f = dup_kv(g1[:, None] * inp["sa_k_w"])
    kb_f = dup_kv_b(b1 @ inp["sa_k_w"] + inp["sa_k_b"])
    vw_f = g1[:, None] * inp["sa_v_w"]
    vb_f = b1 @ inp["sa_v_w"] + inp["sa_v_b"]
    q2w_f = g2[:, None] * inp["ca_q_w"] * sc
    q2b_f = (b2 @ inp["ca_q_w"] + inp["ca_q_b"]) * sc
    k2w_f = dup_kv(inp["ca_k_w"])
    k2b_f = dup_kv_b(inp["ca_k_b"])
    w1_f = inp["moe_w1"] * g3[None, :, None]
    b1_f = np.einsum("d,edf->ef", b3, inp["moe_w1"]).astype(f32)
    w3_f = inp["moe_w3"] * g3[None, :, None]
    b3_f = np.einsum("d,edf->ef", b3, inp["moe_w3"]).astype(f32)

    maskT0 = np.ascontiguousarray(mask[:, 0].transpose(0, 2, 1))     # [B,S,S]
    encmaskT0 = np.ascontiguousarray(encmask[:, 0].transpose(0, 2, 1))
    sa_cls = _mask_classes(maskT0[0], SC)
    ca_cls = _mask_classes(encmaskT0[0], KC)
    for b in range(1, B):
        if _mask_classes(maskT0[b], SC) != sa_cls or \
           _mask_classes(encmaskT0[b], KC) != ca_cls:
            sa_cls = tuple(1 for _ in range(SC * QC))
            ca_cls = tuple(1 for _ in range(KC * QC))
            break

    key = (mode, sa_cls, ca_cls)
    if key not in _CACHE:
        _CACHE[key] = _build(mode, sa_cls, ca_cls)
    nc = _CACHE[key]

    def col128(v):
        return np.asarray(v, f32).reshape(-1, 128).T

    def pack_k(w):
        w = np.asarray(w)
        return np.concatenate([w[k * 128:(k + 1) * 128, :]
                               for k in range(w.shape[0] // 128)], axis=1)

    slots = _routing(inp["langs"])
    # block c selects den row 32*(c%4) of den tile c//4
    onehot = np.zeros((128, 8 * 128), f32)
    for c in range(8):
        onehot[32 * (c % 4), c * 128:(c + 1) * 128] = 1.0
    wq_p = pack_k(qw_f.astype(np_dt))
    wkv_p = pack_k(np.concatenate([kw_f, vw_f], axis=1).astype(np_dt))
    wca = np.concatenate([q2w_f, k2w_f, inp["ca_v_w"]], axis=1).astype(np_dt)

    bias_common = np.zeros((128, BIAS_W), f32)
    for nm, v in [("qb", col128(qb_f)), ("kb", col128(kb_f)),
                  ("vb", np.broadcast_to(vb_f.astype(f32), (128, KVW))),
                  ("ob", col128(inp["sa_o_b"])),
                  ("q2b", col128(q2b_f)), ("k2b", col128(k2b_f)),
                  ("v2b", np.broadcast_to(inp["ca_v_b"].astype(f32), (128, KVW))),
                  ("o2b", col128(inp["ca_o_b"]))]:
        off, w = _BIAS_COLS[nm]
        bias_common[:, off:off + w] = v

    in_maps = []
    for c in range(8):
        b, j = c // 2, c % 2
        e, coef = slots[b][j]
        xTp = pack_k(np.ascontiguousarray(x[b].T))
        # interleave w1/w3 by group: [w1 grp g | w3 grp g] blocks of 512 cols
        gw = GRP * 128
        w13 = np.empty((128, NGRP * DC * 1024), f32)
        for g in range(NGRP):
            for k in range(DC):
                c0 = (g * DC + k) * 1024
                w13[:, c0:c0 + 512] = w1_f[e][k * 128:(k + 1) * 128,
                                              g * 512:(g + 1) * 512]
                w13[:, c0 + 512:c0 + 1024] = w3_f[e][k * 128:(k + 1) * 128,
                                                     g * 512:(g + 1) * 512]
        bt = bias_common.copy()
        for nm, v in [("b1", col128(b1_f[e])), ("b3", col128(b3_f[e]))]:
            off, w = _BIAS_COLS[nm]
            bt[:, off:off + w] = v
        bt[:, _BIAS_COLS["c"][0]] = coef
        m = {
            "encT": pack_k(np.ascontiguousarray(enc[b].T)).astype(np_dt),
            "id128": np.eye(128, dtype=f32).astype(np_dt),
            "onehot": onehot.astype(np_dt),
            "ones_col": np.ones((128, 1), f32).astype(np_dt),
            "ones_row": np.ones((1, 128), f32).astype(np_dt),
            "wq": wq_p, "wkv": wkv_p, "wca": pack_k(wca),
            "ow": pack_k(inp["sa_o_w"].astype(np_dt)),
            "o2w": pack_k(inp["ca_o_w"].astype(np_dt)),
            "biases": bt,
            "w13": w13.astype(np_dt),
            "w2": pack_k(np.ascontiguousarray(inp["moe_w2"][e])).astype(np_dt),
        }
        m["xT_st"] = xTp.astype(np_dt)
        if any(cc == 1 for cc in sa_cls):
            m["maskT"] = np.exp(maskT0[b]).astype(np_dt)
        if any(cc == 1 for cc in ca_cls):
            m["encmaskT"] = np.exp(encmaskT0[b]).astype(np_dt)
        in_maps.append(m)

    kw = {}
    if _TRACE_DIR:
        kw = dict(trace=True, tmpdir=_TRACE_DIR, trace_cores=[0])
    res = bass_utils.run_bass_kernel_spmd(nc, in_maps, core_ids=list(range(8)), **kw)
    global _LAST_EXEC_NS
    _LAST_EXEC_NS = res.exec_time_ns
    return np.stack([
        res.results[2 * b]["out_res"].T
        + res.results[2 * b]["out_ffn"].astype(f32)
        + res.results[2 * b + 1]["out_ffn"].astype(f32)
        for b in range(B)
    ]).astype(f32)



# revision 36
# speedup vs baseline: 1.2566x; 1.0038x over previous
"""MBart MoE decoder layer on 8 trn2 NeuronCores.

Sharding: 8 cores = 8 (sequence, expert-slot) pairs. Core c handles
sequence b=c//2, expert slot j=c%2 (each sequence is lang-routed to at
most 2 distinct experts; routing is computed on the host from `langs`).
Each core computes the full attention path for its sequence (replicated
across the pair) and one expert FFN over all 256 tokens; the host sums
the pair's partial outputs (expert-sharded combine) and transposes back
to token-major. Expert weights are gathered per-core on the host, so a
core only receives the one expert it needs.

On-device layout is feature-major [D, tokens]: projections take weights
as lhsT (feature-major out) or activations as lhsT (token-major out), so
no activation transposes are needed anywhere. LN gains/biases are folded
into the downstream weights on the host; softmax uses transposed scores
[keys, queries] with the attention mask added via an identity-matmul
into PSUM (host classifies each 128x128 mask block as zero / add / skip,
so causal dead blocks are never computed) and denominators accumulated
via a ones-matmul, then broadcast over partitions with a rank-1 matmul
for one full-lane reciprocal per head pair.
"""

import os
import sys
from contextlib import ExitStack

for _p in ("/opt/trn_rl_repo",):
    if _p not in sys.path:
        sys.path.append(_p)

import numpy as np
import ml_dtypes

import concourse.bass as bass
import concourse.tile as tile
import concourse.mybir as mybir
from concourse import bacc, bass_utils

B, S, SK = 4, 256, 512
D, NH, NKV, HD = 1024, 16, 4, 64
DE, NE = 4096, 8
LN_EPS = 1e-5
REP = NH // NKV
DC = D // 128    # 8 feature chunks
FC = DE // 128   # 32 ffn chunks
SC = S // 128    # 2 self-attn key chunks
KC = SK // 128   # 4 cross-attn key chunks
QC = S // 128    # 2 query halves
KVW = NKV * HD   # 256
GRP = 4          # ffn chunks per MoE weight group
NGRP = FC // GRP

MODE = os.environ.get("KERNEL_MM_DTYPE", "bf16")  # "bf16" | "f32r" | "f32"

_CACHE: dict = {}
_TRACE_DIR = None   # set by test harness for profiling runs
_LAST_EXEC_NS = None

# packed attention-weight column layout: qw | kw(dup) | vw
W_Q, W_K, W_V = 0, D, D + 2 * KVW
WPACK = D + 2 * KVW + KVW  # 1792

# packed per-partition bias column layout
_BIAS_COLS = {}
_off = 0
for _n, _w in [("qb", DC), ("kb", 4), ("vb", KVW), ("ob", DC),
               ("q2b", DC), ("k2b", 4), ("v2b", KVW), ("o2b", DC),
               ("b1", FC), ("b3", FC), ("c", 1)]:
    _BIAS_COLS[_n] = (_off, _w)
    _off += _w
BIAS_W = _off


def _build(mode, sa_cls, ca_cls):
    """sa_cls/ca_cls: block classes per (kc, qhalf): 0=no-mask, 1=mask-add,
    2=fully-masked(skip)."""
    st = {"bf16": mybir.dt.bfloat16, "f32r": mybir.dt.float32r,
          "f32": mybir.dt.float32}[mode]
    f32 = mybir.dt.float32
    same_st = mode == "f32"
    A = mybir.ActivationFunctionType
    OP = mybir.AluOpType

    nc = bacc.Bacc("TRN2", target_bir_lowering=False, debug=False, num_devices=8)
    import os as _os
    _SPLIT = _os.environ.get("KERNEL_DMA_SPLIT", "1") == "1"
    eng_b = nc.scalar if _SPLIT else nc.sync
    eng_s = nc.gpsimd if _SPLIT else nc.sync

    def mm(psum, lhsT, rhs, start, stop):
        nc.tensor.matmul(psum, lhsT, rhs, start=start, stop=stop)

    di = {}

    def din(name, shape, dtype=None):
        di[name] = nc.dram_tensor(name, list(shape), dtype or st, kind="ExternalInput")
        return di[name]

    din("xT_st", (128, DC * S))
    din("encT", (128, DC * SK))
    need_samask = any(c == 1 for c in sa_cls)
    need_camask = any(c == 1 for c in ca_cls)
    if need_samask:
        din("maskT", (S, S))
    if need_camask:
        din("encmaskT", (SK, S))
    din("id128", (128, 128))
    din("onehot", (128, 8 * 128))
    din("ones_col", (128, 1))
    din("ones_row", (1, 128))
    din("wq", (128, DC * D))
    din("wkv", (128, DC * (WPACK - D)))
    din("wca", (128, DC * WPACK))
    din("ow", (128, DC * D))
    din("o2w", (128, DC * D))
    din("biases", (128, BIAS_W), f32)
    din("w13", (128, NGRP * DC * 1024))  # per grp, per k: [w1 512 | w3 512]
    din("w2", (128, FC * D))
    out_res = nc.dram_tensor("out_res", [D, S], f32, kind="ExternalOutput")
    out_ffn = nc.dram_tensor("out_ffn", [S, D], st, kind="ExternalOutput")

    with tile.TileContext(nc) as tc, ExitStack() as ctx:
        cp = ctx.enter_context(tc.tile_pool(name="consts", bufs=1))
        pers = ctx.enter_context(tc.tile_pool(name="pers", bufs=1))
        xp = ctx.enter_context(tc.tile_pool(name="xpool", bufs=1))

        ones128 = cp.tile([128, 1], st, tag="ones128", name="ones128")
        eng_b.dma_start(ones128[:], di["ones_col"].ap())
        ones1r = cp.tile([1, 128], st, tag="ones1r", name="ones1r")
        eng_b.dma_start(ones1r[:], di["ones_row"].ap())
        eps_t = cp.tile([128, 1], f32, tag="eps_t", name="eps_t")
        nc.vector.memset(eps_t, LN_EPS)
        id128 = cp.tile([128, 128], st, tag="id128", name="id128")
        eng_b.dma_start(id128[:], di["id128"].ap())
        onehot = cp.tile([128, 8 * 128], st, tag="onehot", name="onehot")
        eng_b.dma_start(onehot[:], di["onehot"].ap())
        maskT = encmaskT = None
        if need_samask:
            maskT = cp.tile([128, SC, S], st, tag="maskT", name="maskT")
            for kc in range(SC):
                eng_b.dma_start(maskT[:, kc, :],
                                  di["maskT"].ap()[kc * 128:(kc + 1) * 128, :])
        if need_camask:
            encmaskT = cp.tile([128, KC, S], st, tag="encmaskT", name="encmaskT")
            for kc in range(KC):
                eng_b.dma_start(encmaskT[:, kc, :],
                                  di["encmaskT"].ap()[kc * 128:(kc + 1) * 128, :])

        bias_t = cp.tile([128, BIAS_W], f32, tag="bias_t", name="bias_t")
        eng_b.dma_start(bias_t[:], di["biases"].ap())

        def bias(nm):
            off, w = _BIAS_COLS[nm]
            return bias_t[:, off:off + w]

        def load_chunks(dram, nchunk, width, tag, pool, dtype=st, engine=None,
                        after=None, inst_out=None):
            t = pool.tile([128, nchunk * width], dtype, tag=tag, name=tag)
            inst = (engine or nc.sync).dma_start(t[:], dram.ap())
            if after is not None:
                tile.add_dep_helper(inst.ins, after, sync=True,
                                    reason="dma priority order")
            if inst_out is not None:
                inst_out.append(inst.ins)
            return [t[:, k * width:(k + 1) * width] for k in range(nchunk)]

        def layernorm(src_f32, src_st, out_tag, pool):
            """src: DC chunks [128,S] f32 (+st copies). Returns DC normalized
            chunks [128,S] st (gain/bias folded downstream by host)."""
            with tc.tile_pool(name=f"{out_tag}_lt", bufs=2) as lp, \
                 tc.tile_pool(name=f"{out_tag}_lp", bufs=1, space="PSUM") as sp, \
                 tc.tile_pool(name=f"{out_tag}_lb", bufs=1, space="PSUM") as bp:
                sum_ps = sp.tile([1, S], f32, tag="lnsum", name="lnsum")
                sq_ps = sp.tile([1, S], f32, tag="lnsq", name="lnsq")
                for k in range(DC):
                    sq = lp.tile([128, S], st, tag="lnsqt", name="lnsqt")
                    nc.vector.tensor_tensor(sq[:], src_st[k][:], src_st[k][:],
                                            OP.mult)
                    mm(sum_ps[:], ones128[:], src_st[k][:], k == 0, k == DC - 1)
                    mm(sq_ps[:], ones128[:], sq[:], k == 0, k == DC - 1)
                s_sb = lp.tile([1, S], st, tag="ln_ssb", name="ln_ssb")
                nc.vector.tensor_single_scalar(s_sb[:], sum_ps[:], 1.0 / D, OP.mult)
                q_sb = lp.tile([1, S], st, tag="ln_qsb", name="ln_qsb")
                nc.vector.tensor_single_scalar(q_sb[:], sq_ps[:], 1.0 / D, OP.mult)
                s_bc = bp.tile([128, S], f32, tag="ln_sbc", name="ln_sbc")
                q_bc = bp.tile([128, S], f32, tag="ln_qbc", name="ln_qbc")
                mm(s_bc[:], ones1r[:], s_sb[:], True, True)   # mean, bcast
                mm(q_bc[:], ones1r[:], q_sb[:], True, True)   # E[x^2], bcast
                # var = E[x^2] - mean^2; rstd/mean*rstd in st so the
                # per-chunk normalize runs in the DVE 16-bit fast mode
                m2 = lp.tile([128, S], f32, tag="ln_m2", name="ln_m2")
                nc.scalar.activation(m2[:], s_bc[:], A.Square)
                var = lp.tile([128, S], f32, tag="ln_var", name="ln_var")
                nc.vector.tensor_sub(var[:], q_bc[:], m2[:])
                v_t = lp.tile([128, S], st, tag="ln_vt", name="ln_vt")
                nc.scalar.activation(v_t[:], var[:], A.Abs_reciprocal_sqrt,
                                     bias=eps_t[:])
                u_t = lp.tile([128, S], st, tag="ln_ut", name="ln_ut")
                nc.vector.tensor_tensor(u_t[:], s_bc[:], v_t[:], OP.mult)
                outs = []
                for k in range(DC):
                    o = pool.tile([128, S], st, tag=f"{out_tag}{k}",
                                  name=f"{out_tag}{k}")
                    nc.vector.tensor_tensor(o[:], src_st[k][:], v_t[:], OP.mult)
                    nc.vector.tensor_sub(o[:], o[:], u_t[:])
                    outs.append(o)
                return outs

        def cast_st(src, tag, pool):
            if same_st:
                return src
            outs = []
            for k, t in enumerate(src):
                o = pool.tile([128, t.shape[-1]], st, tag=f"{tag}{k}",
                              name=f"{tag}{k}")
                nc.vector.tensor_copy(o[:], t[:])
                outs.append(o)
            return outs

        def project_fm(w_slices, rhs_chunks, nout, bias_ap, out_tag, pool,
                       extra=None, out_dt=None, width=None):
            """out^T[dout_chunk] = sum_k w_slices[k][:, m*128:...].T @ rhs[k]."""
            W = width or S
            outs = []
            with tc.tile_pool(name=f"{out_tag}_ps", bufs=3, space="PSUM") as pp:
                for mI in range(nout):
                    ps = pp.tile([128, W], f32, tag="proj", name="proj")
                    for k in range(DC):
                        mm(ps[:], w_slices[k][:, mI * 128:(mI + 1) * 128],
                           rhs_chunks[k][:], k == 0, k == DC - 1)
                    o = pool.tile([128, W], out_dt or st, tag=f"{out_tag}{mI}",
                                  name=f"{out_tag}{mI}")
                    if extra is not None:
                        extra(mI, ps, o)
                    elif bias_ap is not None:
                        nc.vector.tensor_scalar(o[:], ps[:],
                                                bias_ap[:, mI:mI + 1], None,
                                                OP.add)
                    else:
                        nc.vector.tensor_copy(o[:], ps[:])
                    outs.append(o)
            return outs

        def project_tm(act_chunks, w_slices, ntok, bias_bcast, out_tag, pool):
            """token-major V with a ones column appended per kv head:
            out[tok_chunk] = [V_kv | 1] blocks of 65 columns."""
            outs = []
            with tc.tile_pool(name=f"{out_tag}_ps", bufs=3, space="PSUM") as pp:
                for t in range(ntok):
                    ps = pp.tile([128, KVW], f32, tag="projtm", name="projtm")
                    for k in range(DC):
                        mm(ps[:], act_chunks[k][:, t * 128:(t + 1) * 128],
                           w_slices[k][:], k == 0, k == DC - 1)
                    o = pool.tile([128, NKV, HD + 1], st, tag=f"{out_tag}{t}",
                                  name=f"{out_tag}{t}")
                    nc.vector.tensor_add(
                        o[:, :, 0:HD],
                        ps[:].rearrange("p (kv d) -> p kv d", kv=NKV),
                        bias_bcast[:].rearrange("p (kv d) -> p kv d", kv=NKV))
                    for kv in range(NKV):
                        nc.vector.tensor_copy(o[:, kv, HD:HD + 1], ones128[:])
                    outs.append(o)
            return outs

        def attend(qT, kT, vtm, n_kc, mask_tile, cls, out_tag, pool):
            """Baseline-structure attention; mask applied multiplicatively on
            the DVE (host ships exp(mask)) instead of via id128 matmuls."""
            outs = []
            qr = []
            for kc in range(n_kc):
                act = [qh for qh in range(QC) if cls[kc * QC + qh] != 2]
                assert act and act == list(range(act[0], act[-1] + 1))
                qr.append((act[0] * 128, (act[-1] + 1) * 128))
            with tc.tile_pool(name=f"{out_tag}_sp", bufs=3, space="PSUM") as stp, \
                 tc.tile_pool(name=f"{out_tag}_op", bufs=2, space="PSUM") as opp, \
                 tc.tile_pool(name=f"{out_tag}_bp", bufs=1, space="PSUM") as bpp, \
                 tc.tile_pool(name=f"{out_tag}_et", bufs=5) as epool, \
                 tc.tile_pool(name=f"{out_tag}_un", bufs=1) as upool, \
                 tc.tile_pool(name=f"{out_tag}_dt", bufs=1) as dpool:
                # denominators staged at 32-aligned partitions: tile i holds
                # head pairs 4i..4i+3 at rows {0,32,64,96}
                den_t = [dpool.tile([128, 2 * S], st, tag=f"den_t{i}",
                                    name=f"den_t{i}") for i in range(2)]
                for i in range(2):   # unwritten rows must stay finite
                    nc.vector.memset(den_t[i], 1.0)
                o_un = []
                for c in range(DC):
                    o_ps_h = [opp.tile([65, S], f32, tag=f"oph{hh}",
                                       name=f"oph{hh}") for hh in range(2)]
                    kv = (2 * c) // REP      # same kv head for both of the pair
                    for kc in range(n_kc):
                        q0, q1 = qr[kc]
                        adds = [q for q in range(QC) if cls[kc * QC + q] == 1]
                        st_h = []
                        e_h = []
                        for hh in range(2):
                            qh_ap = qT[c][hh * 64:(hh + 1) * 64, :]
                            kh = kT[kv][hh * 64:(hh + 1) * 64, :]
                            st_ps = stp.tile([128, S], f32, tag="st",
                                             name="st")
                            mm(st_ps[:, q0:q1], kh[:, kc * 128:(kc + 1) * 128],
                               qh_ap[:, q0:q1], True, True)
                            st_h.append(st_ps)
                        for hh in range(2):
                            e = epool.tile([128, S], st, tag="e", name="e")
                            nc.scalar.activation(e[:, q0:q1],
                                                 st_h[hh][:, q0:q1], A.Exp)
                            for q in adds:
                                nc.vector.tensor_tensor(
                                    e[:, q * 128:(q + 1) * 128],
                                    e[:, q * 128:(q + 1) * 128],
                                    mask_tile[:, kc, q * 128:(q + 1) * 128],
                                    OP.mult)
                            e_h.append(e)
                        for hh in range(2):
                            mm(o_ps_h[hh][:, q0:q1],
                               vtm[kc][:, kv, :], e_h[hh][:, q0:q1],
                               kc == 0, kc == n_kc - 1)
                    # evacuate unnormalized O + denominators; frees PSUM fast
                    ou = upool.tile([128, S], st, tag=f"un{c}", name=f"un{c}")
                    row = 32 * (c % 4)
                    for hh in range(2):
                        nc.vector.tensor_copy(ou[hh * 64:(hh + 1) * 64, :],
                                              o_ps_h[hh][0:64, :])
                        nc.vector.tensor_copy(
                            den_t[c // 4][row:row + 1, hh * S:(hh + 1) * S],
                            o_ps_h[hh][64:65, :])
                    o_un.append(ou)
                # ONE rsqrt pass for the whole attention (no ACT table thrash)
                den_sq = den_t
                for i in range(2):
                    nc.scalar.activation(den_t[i][:], den_t[i][:],
                                         A.Abs_reciprocal_sqrt)
                    nc.vector.tensor_tensor(den_t[i][:], den_t[i][:],
                                            den_t[i][:], OP.mult)
                for c in range(DC):
                    r_ps = bpp.tile([128, 2 * S], f32, tag="rbc", name="rbc")
                    mm(r_ps[:], onehot[:, c * 128:(c + 1) * 128],
                       den_sq[c // 4][:], True, True)
                    o = pool.tile([128, S], st, tag=f"{out_tag}{c}",
                                  name=f"{out_tag}{c}")
                    for hh in range(2):
                        nc.vector.tensor_tensor(
                            o[hh * 64:(hh + 1) * 64, :],
                            o_un[c][hh * 64:(hh + 1) * 64, :],
                            r_ps[hh * 64:(hh + 1) * 64, hh * S:(hh + 1) * S],
                            OP.mult)
                    outs.append(o)
            return outs

        h1 = [pers.tile([128, S], f32, tag=f"h1T{k}", name=f"h1T{k}")
              for k in range(DC)]
        h2 = [pers.tile([128, S], f32, tag=f"h2T{k}", name=f"h2T{k}")
              for k in range(DC)]

        # w2 fully resident before the MoE starts (4 sliced DMAs on the
        # store queue); gated behind the attention weight loads so the
        # prefetch never starves first-needed transfers
        w2_all = pers.tile([128, FC * D], st, tag="w2_all", name="w2_all")
        GW13 = DC * 1024
        w13e = [pers.tile([128, GW13], st, tag=f"w13e{g}", name=f"w13e{g}")
                for g in range(1)]

        def _issue_w2():
            # sync-queue FIFO after the attention weights: fills the DMA-idle
            # attention window without starving first-needed loads
            for q in range(4):
                w = FC * D // 4
                nc.sync.dma_start(w2_all[:, q * w:(q + 1) * w],
                                  di["w2"].ap()[:, q * w:(q + 1) * w])
            for g in range(1):
                nc.sync.dma_start(w13e[g][:],
                                  di["w13"].ap()[:, g * GW13:(g + 1) * GW13])

        # ---------------- self attention ----------------
        # sync-queue DMA order == need order: consumers' DMA waits are
        # coarsened to later completions on the same lane, so any transfer
        # queued out of need-order delays every later consumer.
        with tc.tile_pool(name="sa_acts", bufs=1) as sa:
            xT_st = load_chunks(di["xT_st"], DC, S, "xTs", sa)
            n1 = layernorm(xT_st, xT_st, "n1T", sa)
            with tc.tile_pool(name="wqkvp", bufs=1) as wp:
                wq_t = load_chunks(di["wq"], DC, D, "wq", wp)
                wkv_t = load_chunks(di["wkv"], DC, WPACK - D, "wkv", wp)
                qT = project_fm(wq_t, n1, DC, bias("qb"), "qT", sa)
                kT = project_fm([t[:, 0:2 * KVW] for t in wkv_t], n1, 4,
                                bias("kb"), "kT", sa)
                v_tm = project_tm(n1, [t[:, 2 * KVW:3 * KVW] for t in wkv_t], SC,
                                  bias("vb"), "v_tm", sa)
            # cross-attn K/V only need encT — loaded and computed here so the
            # PE has filler work during self-attend softmax stalls and the
            # cross weights are queued right after wqkv
            encT = load_chunks(di["encT"], DC, SK, "encT", pers)
            wca = load_chunks(di["wca"], DC, WPACK, "wca", pers)
            k2T = project_fm([t[:, W_K:W_K + 2 * KVW] for t in wca], encT, 4,
                             bias("k2b"), "k2T", pers, width=SK)
            v2_tm = project_tm(encT, [t[:, W_V:W_V + KVW] for t in wca], KC,
                               bias("v2b"), "v2_tm", pers)
            with tc.tile_pool(name="wop", bufs=1) as wp:
                ow_t = load_chunks(di["ow"], DC, D, "ow", wp)
                sa_out = attend(qT, kT, v_tm, SC, maskT, sa_cls, "saT", sa)

                def o_epil(mI, ps, o):
                    nc.vector.scalar_tensor_tensor(o[:], ps[:],
                                                   bias("ob")[:, mI:mI + 1],
                                                   xT_st[mI][:], OP.add, OP.add)
                project_fm(ow_t, sa_out, DC, None, "h1w", _FixedPool(h1),
                           extra=o_epil, out_dt=f32)

        # ---------------- cross attention ----------------
        with tc.tile_pool(name="ca_acts", bufs=1) as ca:
            _issue_w2()   # MoE prefetch: queued after all attention weights
            h1_st = cast_st(h1, "h1s", ca)
            n2 = layernorm(h1, h1_st, "n2T", ca)
            q2T = project_fm([t[:, W_Q:W_Q + D] for t in wca], n2, DC,
                             bias("q2b"), "q2T", ca)
            with tc.tile_pool(name="wo2p", bufs=1) as wp:
                o2w_t = load_chunks(di["o2w"], DC, D, "o2w", wp)
                ca_out = attend(q2T, k2T, v2_tm, KC, encmaskT, ca_cls, "caT", ca)

                def o2_epil(mI, ps, o):
                    nc.vector.scalar_tensor_tensor(o[:], ps[:],
                                                   bias("o2b")[:, mI:mI + 1],
                                                   h1[mI][:], OP.add, OP.add)
                project_fm(o2w_t, ca_out, DC, None, "h2w", _FixedPool(h2),
                           extra=o2_epil, out_dt=f32)

        # residual output (host: out_b = res.T + ffn_j0 + ffn_j1)
        for k in range(DC):
            eng_s.dma_start(out_res.ap()[k * 128:(k + 1) * 128, :], h2[k][:])

        # ---------------- MoE expert ----------------
        with tc.tile_pool(name="moe_acts", bufs=1) as mo:
            h2_st = cast_st(h2, "h2s", mo)
            n3 = layernorm(h2, h2_st, "n3T", mo)

            with tc.tile_pool(name="w13p", bufs=2) as wp, \
                 tc.tile_pool(name="mTp", bufs=4) as mp, \
                 tc.tile_pool(name="gh_ps", bufs=3, space="PSUM") as gp, \
                 tc.tile_pool(name="y_ps", bufs=1, space="PSUM") as yp, \
                 tc.tile_pool(name="gelu_t", bufs=3) as gt, \
                 tc.tile_pool(name="outp", bufs=2) as op_:
                y_ps = [[yp.tile([128, 512], f32, tag=f"y{t}{n}", name=f"y{t}{n}")
                         for n in range(2)] for t in range(QC)]
                gw = GRP * 128
                GW = DC * 1024
                for g in range(NGRP):
                    if g < 1:
                        wgt = w13e[g]
                    else:
                        wgt = wp.tile([128, GW], st, tag="w13g", name="w13g")
                        nc.sync.dma_start(wgt[:],
                                          di["w13"].ap()[:, g * GW:(g + 1) * GW])
                    wg = [wgt[:, k * 1024:(k + 1) * 1024] for k in range(DC)]
                    for mi in range(GRP):
                        mI = g * GRP + mi
                        # one PSUM bank: gelu-arg in [0:S], mult-arg in [S:2S]
                        gh = gp.tile([128, 2 * S], f32, tag="gh", name="gh")
                        for k in range(DC):
                            mm(gh[:, 0:S], wg[k][:, mi * 128:(mi + 1) * 128],
                               n3[k][:], k == 0, k == DC - 1)
                        for k in range(DC):
                            mm(gh[:, S:2 * S],
                               wg[k][:, gw + mi * 128:gw + (mi + 1) * 128],
                               n3[k][:], k == 0, k == DC - 1)
                        ge = gt.tile([128, S], st, tag="ge", name="ge")
                        nc.scalar.activation(ge[:], gh[:, 0:S], A.Gelu,
                                             bias=bias("b1")[:, mI:mI + 1])
                        mT = mp.tile([128, S], st, tag="mT", name="mT")
                        nc.vector.scalar_tensor_tensor(mT[:], gh[:, S:2 * S],
                                                       bias("b3")[:, mI:mI + 1],
                                                       ge[:], OP.add, OP.mult)
                        # fused down-projection: w2 already resident
                        for t in range(QC):
                            for n in range(2):
                                mm(y_ps[t][n][:], mT[:, t * 128:(t + 1) * 128],
                                   w2_all[:, mI * D + n * 512:
                                          mI * D + (n + 1) * 512],
                                   mI == 0, mI == FC - 1)
                for t in range(QC):
                    for n in range(2):
                        o = op_.tile([128, 512], st, tag="o_out", name="o_out")
                        nc.vector.tensor_scalar_mul(o[:], y_ps[t][n][:],
                                                    bias("c")[:, 0:1])
                        eng_s.dma_start(
                            out_ffn.ap()[t * 128:(t + 1) * 128,
                                         n * 512:(n + 1) * 512], o[:])

    nc.compile()
    return nc


class _FixedPool:
    """Adapter letting project_fm write into pre-allocated tiles."""

    def __init__(self, tiles):
        self._tiles = list(tiles)
        self._i = 0

    def tile(self, shape, dtype, tag=None, name=None):
        t = self._tiles[self._i]
        self._i += 1
        return t


def _routing(langs):
    """Per-sequence expert slots [(expert_idx, coef) x2], matching the
    reference: coef[e,b] = any(langs[b]==4+e) * (1/count(langs[b]>3))."""
    langs = np.asarray(langs)
    slots = []
    for b in range(langs.shape[0]):
        row = [int(v) for v in langs[b]]
        cnt = sum(1 for v in row if v > 3)
        rw = 1.0 if cnt == 0 else 1.0 / cnt
        seen = []
        for v in row:
            if v > 3 and 0 <= v - 4 < NE and (v - 4) not in seen:
                seen.append(v - 4)
        sl = [(e, rw) for e in seen]
        while len(sl) < 2:
            sl.append((0, 0.0))
        slots.append(sl[:2])
    return slots


def _mask_classes(maskT, n_kc):
    """Classify each [128 keys x 128 queries] block of a transposed mask:
    0 all-zero (no add), 1 general (add), 2 fully masked (skip compute).
    Keeps at least one active key block per query and contiguous active
    ranges per key chunk."""
    cls = []
    for kc in range(n_kc):
        for qh in range(QC):
            blk = maskT[kc * 128:(kc + 1) * 128, qh * 128:(qh + 1) * 128]
            if np.all(blk == 0):
                cls.append(0)
            elif np.all(blk <= -1e8):
                cls.append(2)
            else:
                cls.append(1)
    for qh in range(QC):
        if all(cls[kc * QC + qh] == 2 for kc in range(n_kc)):
            for kc in range(n_kc):
                cls[kc * QC + qh] = 1
    for kc in range(n_kc):
        act = [q for q in range(QC) if cls[kc * QC + q] != 2]
        if not act or act != list(range(act[0], act[-1] + 1)):
            for q in range(QC):
                if cls[kc * QC + q] == 2:
                    cls[kc * QC + q] = 1
    return tuple(cls)


def kernel(**inputs):
    mode = MODE
    np_dt = ml_dtypes.bfloat16 if mode == "bf16" else np.float32
    f32 = np.float32

    inp = {k: np.asarray(v) for k, v in inputs.items()}
    x = inp["hidden_states"].astype(f32)
    enc = inp["encoder_hidden_states"].astype(f32)
    mask = inp["attention_mask"].astype(f32)
    encmask = inp["encoder_attention_mask"].astype(f32)
    g1, b1 = inp["ln1_g"].astype(f32), inp["ln1_b"].astype(f32)
    g2, b2 = inp["ln2_g"].astype(f32), inp["ln2_b"].astype(f32)
    g3, b3 = inp["ln3_g"].astype(f32), inp["ln3_b"].astype(f32)

    def dup_kv(w):
        return np.concatenate([np.tile(w[:, 64 * j:64 * (j + 1)], (1, 2))
                               for j in range(NKV)], axis=1)

    def dup_kv_b(v):
        return np.concatenate([np.tile(v[64 * j:64 * (j + 1)], 2)
                               for j in range(NKV)])

    sc = HD ** -0.5
    qw_f = g1[:, None] * inp["sa_q_w"] * sc
    qb_f = (b1 @ inp["sa_q_w"] + inp["sa_q_b"]) * sc
    kw_f = dup_kv(g1[:, None] * inp["sa_k_w"])
    kb_f = dup_kv_b(b1 @ inp["sa_k_w"] + inp["sa_k_b"])
    vw_f = g1[:, None] * inp["sa_v_w"]
    vb_f = b1 @ inp["sa_v_w"] + inp["sa_v_b"]
    q2w_f = g2[:, None] * inp["ca_q_w"] * sc
    q2b_f = (b2 @ inp["ca_q_w"] + inp["ca_q_b"]) * sc
    k2w_f = dup_kv(inp["ca_k_w"])
    k2b_f = dup_kv_b(inp["ca_k_b"])
    w1_f = inp["moe_w1"] * g3[None, :, None]
    b1_f = np.einsum("d,edf->ef", b3, inp["moe_w1"]).astype(f32)
    w3_f = inp["moe_w3"] * g3[None, :, None]
    b3_f = np.einsum("d,edf->ef", b3, inp["moe_w3"]).astype(f32)

    maskT0 = np.ascontiguousarray(mask[:, 0].transpose(0, 2, 1))     # [B,S,S]
    encmaskT0 = np.ascontiguousarray(encmask[:, 0].transpose(0, 2, 1))
    sa_cls = _mask_classes(maskT0[0], SC)
    ca_cls = _mask_classes(encmaskT0[0], KC)
    for b in range(1, B):
        if _mask_classes(maskT0[b], SC) != sa_cls or \
           _mask_classes(encmaskT0[b], KC) != ca_cls:
            sa_cls = tuple(1 for _ in range(SC * QC))
            ca_cls = tuple(1 for _ in range(KC * QC))
            break

    key = (mode, sa_cls, ca_cls)
    if key not in _CACHE:
        _CACHE[key] = _build(mode, sa_cls, ca_cls)
    nc = _CACHE[key]

    def col128(v):
        return np.asarray(v, f32).reshape(-1, 128).T

    def pack_k(w):
        w = np.asarray(w)
        return np.concatenate([w[k * 128:(k + 1) * 128, :]
                               for k in range(w.shape[0] // 128)], axis=1)

    slots = _routing(inp["langs"])
    # block c selects den row 32*(c%4) of den tile c//4
    onehot = np.zeros((128, 8 * 128), f32)
    for c in range(8):
        onehot[32 * (c % 4), c * 128:(c + 1) * 128] = 1.0
    wq_p = pack_k(qw_f.astype(np_dt))
    wkv_p = pack_k(np.concatenate([kw_f, vw_f], axis=1).astype(np_dt))
    wca = np.concatenate([q2w_f, k2w_f, inp["ca_v_w"]], axis=1).astype(np_dt)

    bias_common = np.zeros((128, BIAS_W), f32)
    for nm, v in [("qb", col128(qb_f)), ("kb", col128(kb_f)),
                  ("vb", np.broadcast_to(vb_f.astype(f32), (128, KVW))),
                  ("ob", col128(inp["sa_o_b"])),
                  ("q2b", col128(q2b_f)), ("k2b", col128(k2b_f)),
                  ("v2b", np.broadcast_to(inp["ca_v_b"].astype(f32), (128, KVW))),
                  ("o2b", col128(inp["ca_o_b"]))]:
        off, w = _BIAS_COLS[nm]
        bias_common[:, off:off + w] = v

    in_maps = []
    for c in range(8):
        b, j = c // 2, c % 2
        e, coef = slots[b][j]
        xTp = pack_k(np.ascontiguousarray(x[b].T))
        # interleave w1/w3 by group: [w1 grp g | w3 grp g] blocks of 512 cols
        gw = GRP * 128
        w13 = np.empty((128, NGRP * DC * 1024), f32)
        for g in range(NGRP):
            for k in range(DC):
                c0 = (g * DC + k) * 1024
                w13[:, c0:c0 + 512] = w1_f[e][k * 128:(k + 1) * 128,
                                              g * 512:(g + 1) * 512]
                w13[:, c0 + 512:c0 + 1024] = w3_f[e][k * 128:(k + 1) * 128,
                                                     g * 512:(g + 1) * 512]
        bt = bias_common.copy()
        for nm, v in [("b1", col128(b1_f[e])), ("b3", col128(b3_f[e]))]:
            off, w = _BIAS_COLS[nm]
            bt[:, off:off + w] = v
        bt[:, _BIAS_COLS["c"][0]] = coef
        m = {
            "encT": pack_k(np.ascontiguousarray(enc[b].T)).astype(np_dt),
            "id128": np.eye(128, dtype=f32).astype(np_dt),
            "onehot": onehot.astype(np_dt),
            "ones_col": np.ones((128, 1), f32).astype(np_dt),
            "ones_row": np.ones((1, 128), f32).astype(np_dt),
            "wq": wq_p, "wkv": wkv_p, "wca": pack_k(wca),
            "ow": pack_k(inp["sa_o_w"].astype(np_dt)),
            "o2w": pack_k(inp["ca_o_w"].astype(np_dt)),
            "biases": bt,
            "w13": w13.astype(np_dt),
            "w2": pack_k(np.ascontiguousarray(inp["moe_w2"][e])).astype(np_dt),
        }
        m["xT_st"] = xTp.astype(np_dt)
        if any(cc == 1 for cc in sa_cls):
            m["maskT"] = np.exp(maskT0[b]).astype(np_dt)
        if any(cc == 1 for cc in ca_cls):
            m["encmaskT"] = np.exp(encmaskT0[b]).astype(np_dt)
        in_maps.append(m)

    kw = {}
    if _TRACE_DIR:
        kw = dict(trace=True, tmpdir=_TRACE_DIR, trace_cores=[0])
    res = bass_utils.run_bass_kernel_spmd(nc, in_maps, core_ids=list(range(8)), **kw)
    global _LAST_EXEC_NS
    _LAST_EXEC_NS = res.exec_time_ns
    return np.stack([
        res.results[2 * b]["out_res"].T
        + res.results[2 * b]["out_ffn"].astype(f32)
        + res.results[2 * b + 1]["out_ffn"].astype(f32)
        for b in range(B)
    ]).astype(f32)

